# revision 1
# baseline (speedup 1.0000x reference)
"""Bass/Tile kernel for nn_Colorizer (sparse deformable attention colorizer).

Sharding: spatial row-sharding across 8 cores; core i owns output rows
[7i, 7i+7). All refs computed on every core for its rows; the final joint
softmax is additive across refs so each core normalizes locally.

Per-core pipeline:
  A. CV volume (search ref): banded PE matmuls -> CV[pixel, row, dx(105)]
     per pair-group -> SBUF -> DRAM.
  B. Phase-1 gather (static idx): stride-3 rows of CV -> cc0 -> exp ->
     expected offset field -> floor/frac (rounding-mode-agnostic).
  C. Phase-2 gather (dynamic idx): 14x14 CV windows + 14x448 qr0pad runs.
  D. Ref0: bilinear blend -> exp -> B-blur -> DVE contraction -> out0, Z0.
  E. Refs 1/2: transposed banded cc matmuls -> exp*mask -> PSUM-accumulated
     attention matmuls vs pre-transposed qr (ones channel = Z).
  F. Combine: (out12 + out0) / (Z12 + Z0) -> DRAM.
"""
from contextlib import ExitStack

import numpy as np
import ml_dtypes

import concourse.bass as bass

NPBF16 = ml_dtypes.bfloat16
import concourse.mybir as mybir
import concourse.tile as tile

F32 = mybir.dt.float32
I32 = mybir.dt.int32
BF16 = mybir.dt.bfloat16

# ---------------- geometry ----------------
D_SUB, R, C = 4, 6, 32
P13 = 2 * R + 1          # 13
N169 = P13 * P13
DIL_INT = 15
H = W = 56
CF = 64
NCORES = 8
RY = H // NCORES         # 7

DIL = 3
MAXOFF = R * DIL         # 18
DYLO = MAXOFF + R        # 24
NDY = 2 * DYLO + 2       # 50 rows needed for one y-row
WCV = W + NDY - 1        # 105
NROWS_G = NDY + 1        # 51 rows per pair group
HP = H + NDY - 1         # 105
WB = W + 2 * R           # 68
H_SLAB = H + 1           # 57: uniform 51-row pitch for all 4 groups
NRQ = RY + 2 * R         # 19
CC_RUN = 3 * (P13 - 1) + 1   # 37

FLOOR_BIAS = 1024.0
IDX_BIAS = int(FLOOR_BIAS) * WCV + int(FLOOR_BIAS)

GROUPS = [(0, 0, 128), (1, 2, 128), (2, 4, 128), (3, 6, 128)]
PPG = 128  # partitions per group: rows at offsets 0 and 64


def _pad2(a, top, left, hh, ww):
    out = np.zeros(a.shape[:-2] + (hh, ww), a.dtype)
    out[..., top:top + a.shape[-2], left:left + a.shape[-1]] = a
    return out


def host_prep(feats_r, feats_t, quantized_r, ref_index, current_ind):
    feats_r = np.asarray(feats_r, np.float32)
    feats_t = np.asarray(feats_t, np.float32)
    quantized_r = np.asarray(quantized_r, np.float32)
    ri = np.asarray(ref_index).tolist()
    ci = int(current_ind)
    diffs = [ci - int(x) for x in ri]
    nsearch = sum(1 for d in diffs if d > DIL_INT)
    dirates = [min(4, d // DIL_INT + 1) for d in diffs if d > DIL_INT]
    nref = feats_r.shape[0]
    assert nsearch == 1 and dirates[0] == DIL and nref == 3, \
        (nsearch, dirates, nref)

    f1 = feats_t[0]
    f2 = [feats_r[s, 0] for s in range(nref)]
    qr = [quantized_r[s, 0][:, ::D_SUB, ::D_SUB] for s in range(nref)]

    # row-interleaved qr0: QI[r, x, u, c] = qr0pad[r+u, x, c]
    qr0can = np.zeros((HP + 14, HP, C), np.float32)
    qr0can[DYLO:DYLO + H, DYLO:DYLO + W, :] = qr[0].transpose(1, 2, 0)
    qi = np.stack([qr0can[u:u + HP] for u in range(14)], axis=2)  # [HP,HP,14,C]
    qi = qi.reshape(HP * HP * 14 * C, 1)
    qi_b16 = np.ascontiguousarray(qi.astype(NPBF16))

    # f2_0 canvas: rows [-24 .. H+26], cols [-24 .. 80]
    f2p0 = _pad2(f2[0], DYLO, DYLO, H + 2 * DYLO + 3, WCV)
    f2p12 = [_pad2(f2[r], R, R, H + 2 * R, WB) for r in (1, 2)]
    qrpT = []
    for r in (1, 2):
        q = np.zeros((H + 2 * R, WB, C + 1), np.float32)
        q[R:R + H, R:R + W, :C] = qr[r].transpose(1, 2, 0)
        q[:, :, C] = 1.0
        qrpT.append(np.ascontiguousarray(q.transpose(1, 0, 2)))

    ploc128 = np.arange(PPG)
    yloc = (ploc128 >= 64).astype(np.int64)
    xs = np.minimum(ploc128 - 64 * yloc, W - 1)
    ploc = ploc128  # flat pixel slot in CV dram (includes dummy lanes)
    # phase-1 static stream into compact cvcc [p, 13, 105]: x window x+6
    sidx1 = ((ploc * P13) * WCV + xs + R)[:, None]
    # phase-2 CV stream const: row 18+yloc, col x+18; + oi_y*105 + oi_x
    c2cv = (((ploc * NROWS_G + MAXOFF + yloc) * WCV + xs + MAXOFF)
            - IDX_BIAS)[:, None]
    # phase-2 QI stream const (element units): ((y+18+yloc)*105 + x+18)*448
    c2qr = ((((yloc + MAXOFF) * WCV + xs + MAXOFF) - IDX_BIAS) * 448)[:, None]

    gridy = np.tile((np.repeat(np.arange(P13) - R, P13) * DIL)[None, :],
                    (PPG, 1)).astype(np.float32)
    gridx = np.tile((np.tile(np.arange(P13) - R, P13) * DIL)[None, :],
                    (PPG, 1)).astype(np.float32)

    xq = np.arange(WB)[:, None]
    xx = np.arange(W)[None, :]
    maskT = ((xq - xx >= 0) & (xq - xx <= 2 * R)).astype(np.float32)
    maskT_tiled = np.ascontiguousarray(
        np.tile(maskT[:, None, :], (1, P13, 1)).reshape(WB, P13 * W))

    def b16(a):
        return np.ascontiguousarray(a.astype(NPBF16))

    in_maps = []
    for core in range(NCORES):
        y0 = core * RY
        f1pair = np.zeros((CF, 4 * PPG), np.float32)
        for g in range(4):
            f1pair[:, g * PPG:g * PPG + W] = f1[:, y0 + 2 * g, :]
            if 2 * g + 1 < RY:
                f1pair[:, g * PPG + 64:g * PPG + 64 + W] = f1[:, y0 + 2 * g + 1, :]
        m = dict(
            f1=b16(f1[:, y0:y0 + RY, :].reshape(CF, RY * W)),
            f1pair=b16(f1pair),
            f2p0=b16(
                f2p0[:, y0:y0 + H_SLAB, :].reshape(CF, H_SLAB * WCV)),
            f2p1=b16(f2p12[0][:, y0:y0 + NRQ, :].reshape(CF, NRQ * WB)),
            f2p2=b16(f2p12[1][:, y0:y0 + NRQ, :].reshape(CF, NRQ * WB)),
            qrT1=b16(qrpT[0][:, y0:y0 + NRQ, :].reshape(WB, NRQ * (C + 1))),
            qrT2=b16(qrpT[1][:, y0:y0 + NRQ, :].reshape(WB, NRQ * (C + 1))),
            qr0pad=qi_b16,
            idx1=sidx1.astype(np.int32),
            c2cv=c2cv.astype(np.float32),
            c2qr=(c2qr + y0 * WCV * 448).astype(np.float32),
            gridx=gridx, gridy=gridy,
            maskT=b16(maskT_tiled),
        )
        in_maps.append(m)
    return in_maps


INPUT_SPECS = dict(
    f1=([CF, RY * W], BF16), f1pair=([CF, 4 * PPG], BF16),
    f2p0=([CF, H_SLAB * WCV], BF16),
    f2p1=([CF, NRQ * WB], BF16), f2p2=([CF, NRQ * WB], BF16),
    qrT1=([WB, NRQ * (C + 1)], BF16), qrT2=([WB, NRQ * (C + 1)], BF16),
    qr0pad=([HP * HP * 14 * C, 1], BF16),
    idx1=([PPG, 1], I32), c2cv=([PPG, 1], F32), c2qr=([PPG, 1], F32),
    gridx=([PPG, N169], F32), gridy=([PPG, N169], F32),
    maskT=([WB, P13 * W], BF16),
)
OUT_SPEC = ([RY * W, C], F32)


def build_kernel(tc, outs, ins):
    nc = tc.nc
    Exp = mybir.ActivationFunctionType.Exp
    ALU = mybir.AluOpType
    AX = mybir.AxisListType

    with ExitStack() as ctx:
        sb = ctx.enter_context(tc.tile_pool(name="sb", bufs=1))
        sbg = ctx.enter_context(tc.tile_pool(name="sbg", bufs=2))
        sbg3 = ctx.enter_context(tc.tile_pool(name="sbg3", bufs=3))
        ps_cv = ctx.enter_context(tc.tile_pool(name="ps_cv", bufs=2, space="PSUM"))
        ps_cc = ctx.enter_context(tc.tile_pool(name="ps_cc", bufs=2, space="PSUM"))
        ps_out = ctx.enter_context(tc.tile_pool(name="ps_out", bufs=2, space="PSUM"))
        dram = ctx.enter_context(tc.tile_pool(name="dram", bufs=1, space="DRAM"))

        def load(name, dtype=None):
            shape, dt_ = INPUT_SPECS[name]
            t = sb.tile(shape, dtype or dt_, tag=name)
            nc.sync.dma_start(t[:], ins[name])
            return t

        f1_t = load("f1")
        f1pair_t = load("f1pair")
        f2p0_t = load("f2p0")
        f2p12_t = [load("f2p1"), load("f2p2")]
        qrT_t = [load("qrT1"), load("qrT2")]
        idx1_t = load("idx1")
        c2cv_t = load("c2cv")
        c2qr_t = load("c2qr")
        gridx_t = load("gridx")
        gridy_t = load("gridy")
        maskT_t = load("maskT")

        out0_g, z0_g = {}, {}

        # ---------- E/F. refs 1/2 + combine (emitted per group) ----------
        def emit_row(yr):
            op = ps_out.tile([W, C + 1], F32, tag="op")
            first = True
            for r in (0, 1):
                em = sbg.tile([WB, P13 * W], BF16, tag="em")
                for h0, hn in ((0, 7), (7, 6)):
                    ct = ps_cc.tile([WB, 7 * W], F32, tag="ct")
                    for i in range(hn):
                        iy = h0 + i
                        nc.tensor.matmul(
                            ct[:, i * W:(i + 1) * W],
                            lhsT=f2p12_t[r][:, (yr + iy) * WB:(yr + iy + 1) * WB],
                            rhs=f1_t[:, yr * W:(yr + 1) * W],
                            start=True, stop=True)
                    nc.scalar.activation(em[:, h0 * W:(h0 + hn) * W],
                                         ct[:, 0:hn * W], Exp)
                nc.vector.tensor_tensor(em[:], em[:], maskT_t[:], op=ALU.mult)
                for iy in range(P13):
                    nc.tensor.matmul(
                        op[:], lhsT=em[:, iy * W:(iy + 1) * W],
                        rhs=qrT_t[r][:, (yr + iy) * (C + 1):
                                     (yr + iy + 1) * (C + 1)],
                        start=first, stop=(r == 1 and iy == P13 - 1))
                    first = False
            g = yr // 2
            p_lo = 64 * (yr % 2)
            psl = slice(p_lo, p_lo + W)
            den = sbg.tile([W, 2], F32, tag="den")
            nc.vector.tensor_tensor(den[:, 0:1], op[:, C:C + 1],
                                    z0_g[g][psl, N169:N169 + 1], op=ALU.add)
            nc.vector.reciprocal(den[:, 1:2], den[:, 0:1])
            of = sbg.tile([W, C], F32, tag="of")
            nc.vector.tensor_tensor(of[:], op[:, 0:C], out0_g[g][psl, :],
                                    op=ALU.add)
            nc.vector.tensor_scalar(of[:], of[:], den[:, 1:2], None,
                                    op0=ALU.mult)
            nc.sync.dma_start(
                outs["out"].rearrange("(y x) c -> y x c", y=RY)[yr], of[:])


        for g, yg, MP in GROUPS:
            nrow = NROWS_G
            # ---------- A. CV ----------
            cv_sb = sbg.tile([MP, nrow * WCV], BF16, tag="cv_sb")
            lhs = f1pair_t[:, g * PPG:(g + 1) * PPG]
            CH = 8
            ci = 0
            for r0 in range(0, nrow, CH):
                rn = min(CH, nrow - r0)
                pt = ps_cv.tile([MP, CH * 128], F32, tag="cvch")
                for r in range(rn):
                    row = 2 * g + r0 + r
                    nc.tensor.matmul(
                        pt[:, r * 128:r * 128 + WCV],
                        lhsT=lhs, rhs=f2p0_t[:, row * WCV:(row + 1) * WCV],
                        start=True, stop=True)
                dst = cv_sb[:, r0 * WCV:(r0 + rn) * WCV].rearrange(
                    "p (r w) -> p r w", r=rn)
                src = pt[:].rearrange("p (r w) -> p r w", r=CH)[:, 0:rn, 0:WCV]
                if ci % 2 == 0:
                    nc.vector.tensor_copy(dst, src)
                else:
                    nc.scalar.copy(dst, src)
                ci += 1
            cv_dram = dram.tile([MP * nrow * WCV, 1], BF16, tag=f"cvd{g}")
            nc.sync.dma_start(
                cv_dram[:].rearrange("(p f) o -> p (f o)", p=MP), cv_sb[:])
            # compact stride-3 rows for phase-1: cvcc [p, 13, 105]
            cvcc = dram.tile([MP * P13 * WCV, 1], BF16, tag=f"cvcc{g}")
            ccv_w = cvcc[:].rearrange("(p f) o -> p (f o)", p=MP).rearrange(
                "p (i w) -> p i w", i=P13)
            for pl, base in ((slice(0, 64), R), (slice(64, 128), R + 1)):
                sl = cv_sb[pl]
                src = bass.AP(sl.tensor, sl.offset + base * WCV,
                              [sl.ap[0], [3 * WCV, P13], [1, WCV]])
                nc.sync.dma_start(ccv_w[pl], src)

            # ---------- B. phase-1 (single stream per pixel, compact) ----------
            NS1 = 12 * WCV + CC_RUN
            g1 = sbg3.tile([MP, NS1], BF16, tag="g1")
            nc.gpsimd.indirect_dma_start(
                out=g1[:], out_offset=None, in_=cvcc[:],
                in_offset=bass.IndirectOffsetOnAxis(ap=idx1_t[0:MP, :], axis=0))
            cc0 = bass.AP(g1[:].tensor, g1[:].offset,
                          [g1[:].ap[0], [WCV, P13], [3, P13]])
            e1 = sbg.tile([MP, N169 + 1], F32, tag="e1")
            nc.scalar.activation(
                e1[:, 0:N169].rearrange("p (i j) -> p i j", i=P13), cc0, Exp,
                accum_out=e1[:, N169:N169 + 1])
            sc = sbg.tile([MP, 4], F32, tag="sc")
            tmp = sbg.tile([MP, N169], F32, tag="tmp169")
            nc.vector.scalar_tensor_tensor(
                out=tmp[:], in0=e1[:, 0:N169], scalar=0.0, in1=gridx_t[0:MP, :],
                op0=ALU.add, op1=ALU.mult, accum_out=sc[:, 0:1])
            nc.vector.scalar_tensor_tensor(
                out=tmp[:], in0=e1[:, 0:N169], scalar=0.0, in1=gridy_t[0:MP, :],
                op0=ALU.add, op1=ALU.mult, accum_out=sc[:, 1:2])
            offs = sbg.tile([MP, 2], F32, tag="offs")   # [off_x, off_y]
            nc.vector.reciprocal(sc[:, 2:3], e1[:, N169:N169 + 1])
            nc.vector.tensor_tensor(offs[:, 0:1], sc[:, 0:1], sc[:, 2:3],
                                    op=ALU.mult)
            nc.vector.tensor_tensor(offs[:, 1:2], sc[:, 1:2], sc[:, 2:3],
                                    op=ALU.mult)
            nc.vector.tensor_scalar(offs[:], offs[:], float(MAXOFF),
                                    -float(MAXOFF), op0=ALU.min, op1=ALU.max)
            # floor (mode-agnostic): fb = off+1024; fbi=cast; fbf=cast back;
            # fbf -= (fb - fbf < 0); wfrac = fb - fbf; fbi2 = cast(fbf)
            fb = sbg.tile([MP, 2], F32, tag="fb")
            nc.vector.tensor_scalar(fb[:], offs[:], FLOOR_BIAS, None,
                                    op0=ALU.add)
            fbi = sbg.tile([MP, 2], I32, tag="fbi")
            nc.vector.tensor_copy(fbi[:], fb[:])
            fbf = sbg.tile([MP, 2], F32, tag="fbf")
            nc.vector.tensor_copy(fbf[:], fbi[:])
            err = sbg.tile([MP, 2], F32, tag="err")
            nc.vector.tensor_tensor(err[:], fb[:], fbf[:], op=ALU.subtract)
            neg = sbg.tile([MP, 2], F32, tag="neg")
            nc.vector.tensor_scalar(neg[:], err[:], 0.0, None, op0=ALU.is_lt)
            nc.vector.tensor_tensor(fbf[:], fbf[:], neg[:], op=ALU.subtract)
            wfrac = sbg.tile([MP, 2], F32, tag="wfrac")  # [wx, wy]
            nc.vector.tensor_tensor(wfrac[:], fb[:], fbf[:], op=ALU.subtract)
            s2 = sbg.tile([MP, 1], F32, tag="s2")
            nc.vector.scalar_tensor_tensor(
                out=s2[:], in0=fbf[:, 1:2], scalar=float(WCV),
                in1=fbf[:, 0:1], op0=ALU.mult, op1=ALU.add)
            idx2cvf = sbg.tile([MP, 1], F32, tag="idx2cvf")
            nc.vector.tensor_scalar(idx2cvf[:], c2cv_t[0:MP, :], s2[:], None,
                                    op0=ALU.add)
            idx2cv = sbg.tile([MP, 1], I32, tag="idx2cv")
            nc.vector.tensor_copy(idx2cv[:], idx2cvf[:])
            # QI element index: c2qr + (s2 + yg*WCV)*448
            idx2qrf = sbg.tile([MP, 1], F32, tag="idx2qrf")
            nc.vector.tensor_scalar(idx2qrf[:], s2[:], 448.0,
                                    float(yg * WCV * 448),
                                    op0=ALU.mult, op1=ALU.add)
            nc.vector.tensor_tensor(idx2qrf[:], idx2qrf[:], c2qr_t[0:MP, :],
                                    op=ALU.add)
            idx2qr = sbg.tile([MP, 1], I32, tag="idx2qr")
            nc.vector.tensor_copy(idx2qr[:], idx2qrf[:])

            # ---------- C. phase-2 gathers (single stream per pixel) ----------
            NS2 = 13 * WCV + 14
            g2 = sbg3.tile([MP, NS2], BF16, tag="g2")
            nc.gpsimd.indirect_dma_start(
                out=g2[:], out_offset=None, in_=cv_dram[:],
                in_offset=bass.IndirectOffsetOnAxis(ap=idx2cv[:], axis=0))
            qt = sbg3.tile([MP, 14 * 448], BF16, tag="qt")
            nc.gpsimd.indirect_dma_start(
                out=qt[:], out_offset=None, in_=ins["qr0pad"],
                in_offset=bass.IndirectOffsetOnAxis(ap=idx2qr[:], axis=0))

            # ---------- D. ref0 ----------
            ww = sbg.tile([MP, 4], F32, tag="ww")
            om = sbg.tile([MP, 2], F32, tag="om")
            nc.vector.tensor_scalar(om[:], wfrac[:], -1.0, 1.0,
                                    op0=ALU.mult, op1=ALU.add)
            nc.vector.tensor_tensor(ww[:, 0:1], om[:, 1:2], om[:, 0:1],
                                    op=ALU.mult)
            nc.vector.tensor_tensor(ww[:, 1:2], om[:, 1:2], wfrac[:, 0:1],
                                    op=ALU.mult)
            nc.vector.tensor_tensor(ww[:, 2:3], wfrac[:, 1:2], om[:, 0:1],
                                    op=ALU.mult)
            nc.vector.tensor_tensor(ww[:, 3:4], wfrac[:, 1:2], wfrac[:, 0:1],
                                    op=ALU.mult)

            g2v = bass.AP(g2[:].tensor, g2[:].offset,
                          [g2[:].ap[0], [WCV, 14], [1, 14]])
            corr = sbg.tile([MP, N169], F32, tag="corr")
            crv = corr[:].rearrange("p (i j) -> p i j", i=P13)
            nc.vector.tensor_scalar(crv, g2v[:, 0:13, 0:13], ww[:, 0:1], None,
                                    op0=ALU.mult)
            for (sl_u, sl_v, wcol) in (((0, 13), (1, 14), 1),
                                       ((1, 14), (0, 13), 2),
                                       ((1, 14), (1, 14), 3)):
                nc.vector.scalar_tensor_tensor(
                    out=crv, in0=g2v[:, sl_u[0]:sl_u[1], sl_v[0]:sl_v[1]],
                    scalar=ww[:, wcol:wcol + 1], in1=crv,
                    op0=ALU.mult, op1=ALU.add)
            p0 = sbg.tile([MP, N169 + 1], F32, tag="p0")
            nc.scalar.activation(p0[:, 0:N169], corr[:], Exp,
                                 accum_out=p0[:, N169:N169 + 1])
            z0_g[g] = p0
            bb = sbg.tile([MP, 196], BF16, tag="bb")
            nc.vector.memset(bb[:], 0.0)
            bbv = bb[:].rearrange("p (u v) -> p u v", u=14)
            p0v = p0[:, 0:N169].rearrange("p (i j) -> p i j", i=P13)
            nc.vector.tensor_scalar(bbv[:, 0:13, 0:13], p0v, ww[:, 0:1], None,
                                    op0=ALU.mult)
            for (sl_u, sl_v, wcol) in (((0, 13), (1, 14), 1),
                                       ((1, 14), (0, 13), 2),
                                       ((1, 14), (1, 14), 3)):
                dstv = bbv[:, sl_u[0]:sl_u[1], sl_v[0]:sl_v[1]]
                nc.vector.scalar_tensor_tensor(
                    out=dstv, in0=p0v, scalar=ww[:, wcol:wcol + 1], in1=dstv,
                    op0=ALU.mult, op1=ALU.add)
            qtv = bass.AP(qt[:].tensor, qt[:].offset,
                          [qt[:].ap[0], [32, 14], [448, 14], [1, C]])  # (u,v,c)
            bbb = bbv.to_broadcast([MP, 14, 14, C])
            nc.vector.tensor_tensor(qtv, qtv, bbb, op=ALU.mult)
            o0 = sbg.tile([MP, C], F32, tag="o0")
            pr = bass.AP(qt[:].tensor, qt[:].offset,
                         [qt[:].ap[0], [1, C], [32, 14], [448, 14]])
            nc.vector.tensor_reduce(o0[:], pr, axis=AX.XY, op=ALU.add)
            out0_g[g] = o0
            for yr in (2 * g, 2 * g + 1):
                if yr < RY:
                    emit_row(yr)
            if g == 0 and "dbg_qt" in outs:
                nc.sync.dma_start(outs["dbg_qt"], qt[:])
                nc.sync.dma_start(outs["dbg_g2"], g2[:])
                nc.sync.dma_start(outs["dbg_idx2qr"], idx2qr[:])
                nc.sync.dma_start(outs["dbg_idx2cv"], idx2cv[:])
                nc.sync.dma_start(outs["dbg_o0"], o0[:])
                nc.sync.dma_start(outs["dbg_bb"], bb[:])
                nc.sync.dma_start(outs["dbg_offs"], offs[:])
                nc.sync.dma_start(outs["dbg_e1"], e1[:])
                nc.sync.dma_start(outs["dbg_g1"], g1[:])




# ---------------- numpy mirror of one core (debug) ----------------
def core_reference(m):
    m = {k: (np.asarray(v, np.float32) if v.dtype != np.int32 else v)
         for k, v in m.items()}
    f1 = m["f1"].reshape(CF, RY, W)
    f2p0 = m["f2p0"].reshape(CF, H_SLAB, WCV)
    o0full = np.zeros((RY, W, C), np.float32)
    z0full = np.zeros((RY, W, 1), np.float32)
    for g, yg, MP in GROUPS:
        nrow = NROWS_G
        lhs = m["f1pair"][:, g * PPG:(g + 1) * PPG].astype(np.float32)
        cv = np.einsum('cp,crw->prw', lhs, f2p0[:, 2 * g:2 * g + nrow, :])
        cvf = np.ascontiguousarray(cv).reshape(-1)
        yy = (np.arange(MP) >= 64).astype(np.int64)
        rows = (R + yy[:, None, None] + 3 * np.arange(P13)[None, :, None])
        cvcc = np.take_along_axis(
            cv, np.broadcast_to(rows, (MP, P13, WCV)), axis=1)
        ccf = np.ascontiguousarray(cvcc).reshape(-1)
        sidx = m["idx1"][:MP, 0] - (np.arange(MP) * NROWS_G - np.arange(MP) * P13) * WCV
        # device sidx indexes cvcc directly: (p*13)*WCV + x + 6
        sidx = (np.arange(MP) * P13 * WCV
                + np.minimum(np.arange(MP) - 64 * yy, W - 1) + R)
        NS1 = 12 * WCV + CC_RUN
        g1 = ccf[sidx[:, None] + np.arange(NS1)[None, :]]
        cc0 = np.stack([g1[:, i * WCV + 3 * np.arange(P13)]
                        for i in range(P13)], 1).reshape(MP, N169)
        e1 = np.exp(cc0)
        S = e1.sum(1, keepdims=True)
        offx = np.clip((e1 * m["gridx"][:MP]).sum(1, keepdims=True) / S,
                       -MAXOFF, MAXOFF)
        offy = np.clip((e1 * m["gridy"][:MP]).sum(1, keepdims=True) / S,
                       -MAXOFF, MAXOFF)
        fbx = np.floor(offx + FLOOR_BIAS)
        fby = np.floor(offy + FLOOR_BIAS)
        wx = (offx + FLOOR_BIAS) - fbx
        wy = (offy + FLOOR_BIAS) - fby
        s2 = (fby.astype(np.int64) * WCV + fbx.astype(np.int64))
        idx2cv = (m["c2cv"][:MP, 0] + s2[:, 0]).astype(np.int64)
        idx2qr = (m["c2qr"][:MP, 0] + (s2[:, 0] + yg * WCV) * 448).astype(np.int64)
        NS2 = 13 * WCV + 14
        g2s = cvf[idx2cv[:, None] + np.arange(NS2)[None, :]]
        g2 = np.stack([g2s[:, u * WCV:u * WCV + 14] for u in range(14)], 1)
        qrf = m["qr0pad"].reshape(-1)
        qts = qrf[idx2qr[:, None] + np.arange(6272)[None, :]]
        # stream order (v, u, c) -> [MP, u, v, c]
        qt = qts.reshape(MP, 14, 14, C).transpose(0, 2, 1, 3)
        w00 = (1 - wy) * (1 - wx); w01 = (1 - wy) * wx
        w10 = wy * (1 - wx); w11 = wy * wx
        corr = (w00 * g2[:, 0:13, 0:13].reshape(MP, N169)
                + w01 * np.ascontiguousarray(g2[:, 0:13, 1:14]).reshape(MP, N169)
                + w10 * np.ascontiguousarray(g2[:, 1:14, 0:13]).reshape(MP, N169)
                + w11 * np.ascontiguousarray(g2[:, 1:14, 1:14]).reshape(MP, N169))
        p0 = np.exp(corr)
        z0 = p0.sum(1, keepdims=True)
        bb = np.zeros((MP, 14, 14), np.float32)
        p0v = p0.reshape(MP, P13, P13)
        bb[:, 0:13, 0:13] += w00[..., None] * p0v
        bb[:, 0:13, 1:14] += w01[..., None] * p0v
        bb[:, 1:14, 0:13] += w10[..., None] * p0v
        bb[:, 1:14, 1:14] += w11[..., None] * p0v
        o0 = (qt * bb[..., None]).sum((1, 2))
        for yloc in range(2):
            if yg + yloc >= RY:
                continue
            o0full[yg + yloc] = o0[64 * yloc:64 * yloc + W]
            z0full[yg + yloc] = z0[64 * yloc:64 * yloc + W]
    out = np.zeros((RY, W, C), np.float32)
    maskT = m["maskT"].reshape(WB, P13, W)[:, 0, :]
    for yr in range(RY):
        acc = np.zeros((W, C + 1), np.float32)
        for r in range(2):
            f2p = m[f"f2p{r + 1}"].reshape(CF, NRQ, WB)
            qrT = m[f"qrT{r + 1}"].reshape(WB, NRQ, C + 1)
            for iy in range(P13):
                ct = np.einsum('cq,cx->qx', f2p[:, yr + iy, :], f1[:, yr, :])
                em = np.exp(ct) * maskT
                acc += np.einsum('qx,qd->xd', em, qrT[:, yr + iy, :])
        den = acc[:, C:C + 1] + z0full[yr]
        out[yr] = (acc[:, :C] + o0full[yr]) / den
    return out


def full_reference_from_cores(in_maps):
    outs = [core_reference(in_maps[i]) for i in range(NCORES)]
    full = np.stack(outs, 0)            # [8, 7, 56, C]
    return full.reshape(H, W, C).transpose(2, 0, 1)[None]


DEBUG_SPECS = dict(
    dbg_qt=([128, 14 * 448], F32), dbg_g2=([128, 196], F32),
    dbg_idx2qr=([128, 14], I32), dbg_idx2cv=([128, 14], I32),
    dbg_o0=([128, C], F32), dbg_bb=([128, 196], F32),
    dbg_offs=([128, 2], F32), dbg_e1=([128, N169 + 1], F32),
    dbg_g1=([128, P13 * CC_RUN], F32),
)


def build_program(ncores=NCORES, debug=False):
    import concourse.bacc as bacc
    nc = bacc.Bacc("TRN2", target_bir_lowering=False, debug=False,
                   enable_asserts=True, num_devices=ncores)
    ins = {}
    for name, (shape, dt_) in INPUT_SPECS.items():
        ins[name] = nc.dram_tensor(name, shape, dt_, kind="ExternalInput").ap()
    outs = {"out": nc.dram_tensor("out", OUT_SPEC[0], OUT_SPEC[1],
                                  kind="ExternalOutput").ap()}
    if debug:
        for name, (shape, dt_) in DEBUG_SPECS.items():
            outs[name] = nc.dram_tensor(name, shape, dt_,
                                        kind="ExternalOutput").ap()
    with tile.TileContext(nc) as tc:
        build_kernel(tc, outs, ins)
    nc.compile()
    return nc


# ======================= runner =======================
import os as _os


def _build_program():
    import concourse.bacc as bacc
    nc = bacc.Bacc("TRN2", target_bir_lowering=False, debug=False,
                   enable_asserts=True, num_devices=NCORES)
    ins = {}
    for name, (shape, dt_) in INPUT_SPECS.items():
        ins[name] = nc.dram_tensor(name, shape, dt_, kind="ExternalInput").ap()
    outs = {"out": nc.dram_tensor("out", OUT_SPEC[0], OUT_SPEC[1],
                                  kind="ExternalOutput").ap()}
    with tile.TileContext(nc) as tc:
        build_kernel(tc, outs, ins)
    nc.compile()
    return nc


_LAST_RESULT = {}


def kernel(**inputs):
    from concourse.bass_utils import run_bass_kernel_spmd
    from concourse.bass_interp import get_hw_module

    in_maps = host_prep(**inputs)
    nc = _build_program()
    nc.m = get_hw_module(nc.m)
    trace = _os.environ.get("KERNEL_TRACE", "0") == "1"
    res = run_bass_kernel_spmd(
        nc, in_maps, core_ids=list(range(NCORES)), trace=trace)
    _LAST_RESULT["res"] = res
    slabs = [np.asarray(res.results[i]["out"], np.float32).reshape(RY, W, C)
             for i in range(NCORES)]
    full = np.concatenate(slabs, 0)          # [56, 56, 32]
    return np.ascontiguousarray(full.transpose(2, 0, 1)[None])



# revision 6
# speedup vs baseline: 21.9971x; 21.9971x over previous
"""Bass/Tile kernel for nn_Colorizer (sparse deformable attention colorizer).

Sharding: spatial row-sharding across 8 cores; core i owns output rows
[7i, 7i+7). All refs computed on every core for its rows; the final joint
softmax is additive across refs so each core normalizes locally.

Per-core pipeline:
  A. CV volume (search ref): banded PE matmuls -> CV[pixel, row, dx(105)]
     per pair-group -> SBUF -> DRAM.
  B. Phase-1 gather (static idx): stride-3 rows of CV -> cc0 -> exp ->
     expected offset field -> floor/frac (rounding-mode-agnostic).
  C. Phase-2 gather (dynamic idx): 14x14 CV windows + 14x448 qr0pad runs.
  D. Ref0: bilinear blend -> exp -> B-blur -> DVE contraction -> out0, Z0.
  E. Refs 1/2: transposed banded cc matmuls -> exp*mask -> PSUM-accumulated
     attention matmuls vs pre-transposed qr (ones channel = Z).
  F. Combine: (out12 + out0) / (Z12 + Z0) -> DRAM.
"""
from contextlib import ExitStack

import numpy as np
import ml_dtypes

import concourse.bass as bass

NPBF16 = ml_dtypes.bfloat16
import concourse.mybir as mybir
import concourse.tile as tile

F32 = mybir.dt.float32
I32 = mybir.dt.int32
BF16 = mybir.dt.bfloat16

# ---------------- geometry ----------------
D_SUB, R, C = 4, 6, 32
P13 = 2 * R + 1          # 13
N169 = P13 * P13
DIL_INT = 15
H = W = 56
CF = 64
NCORES = 8
RY = H // NCORES         # 7

DIL = 3
MAXOFF = R * DIL         # 18
DYLO = MAXOFF + R        # 24
NDY = 2 * DYLO + 2       # 50 rows needed for one y-row
WCV = W + NDY - 1        # 105
NROWS_G = NDY + 1        # 51 rows per pair group
HP = H + NDY - 1         # 105
WB = W + 2 * R           # 68
H_SLAB = H + 1           # 57: uniform 51-row pitch for all 4 groups
NRQ = RY + 2 * R         # 19
CC_RUN = 3 * (P13 - 1) + 1   # 37

FLOOR_BIAS = 1024.0
IDX_BIAS = int(FLOOR_BIAS) * WCV + int(FLOOR_BIAS)

GROUPS = [(0, 0, 128), (1, 2, 128), (2, 4, 128), (3, 6, 128)]
PPG = 128  # partitions per group: rows at offsets 0 and 64


def _pad2(a, top, left, hh, ww):
    out = np.zeros(a.shape[:-2] + (hh, ww), a.dtype)
    out[..., top:top + a.shape[-2], left:left + a.shape[-1]] = a
    return out


def host_prep(feats_r, feats_t, quantized_r, ref_index, current_ind):
    feats_r = np.asarray(feats_r, np.float32)
    feats_t = np.asarray(feats_t, np.float32)
    quantized_r = np.asarray(quantized_r, np.float32)
    ri = np.asarray(ref_index).tolist()
    ci = int(current_ind)
    diffs = [ci - int(x) for x in ri]
    nsearch = sum(1 for d in diffs if d > DIL_INT)
    dirates = [min(4, d // DIL_INT + 1) for d in diffs if d > DIL_INT]
    nref = feats_r.shape[0]
    assert nsearch == 1 and dirates[0] == DIL and nref == 3, \
        (nsearch, dirates, nref)

    f1 = feats_t[0]
    f2 = [feats_r[s, 0] for s in range(nref)]
    qr = [quantized_r[s, 0][:, ::D_SUB, ::D_SUB] for s in range(nref)]

    # row-interleaved qr0: QI[r, x, u, c] = qr0pad[r+u, x, c]
    qr0can = np.zeros((HP + 14, HP, C), np.float32)
    qr0can[DYLO:DYLO + H, DYLO:DYLO + W, :] = qr[0].transpose(1, 2, 0)
    qi = np.stack([qr0can[u:u + HP] for u in range(14)], axis=2)  # [HP,HP,14,C]
    qi = qi.reshape(1, HP * HP * 14 * C)
    qi_b16 = np.ascontiguousarray(qi.astype(NPBF16))

    # f2_0 canvas: rows [-24 .. H+26], cols [-24 .. 80]
    f2p0 = _pad2(f2[0], DYLO, DYLO, H + 2 * DYLO + 3, WCV)
    f2p12 = [_pad2(f2[r], R, R, H + 2 * R, WB) for r in (1, 2)]
    qrpT = []
    for r in (1, 2):
        q = np.zeros((H + 2 * R, WB, C + 1), np.float32)
        q[R:R + H, R:R + W, :C] = qr[r].transpose(1, 2, 0)
        q[:, :, C] = 1.0
        qrpT.append(np.ascontiguousarray(q.transpose(1, 0, 2)))

    ploc128 = np.arange(PPG)
    yloc = (ploc128 >= 64).astype(np.int64)
    xs = np.minimum(ploc128 - 64 * yloc, W - 1)
    ploc = ploc128  # flat pixel slot in CV dram (includes dummy lanes)
    # phase-1 static stream into compact cvcc [p, 13, 105]: x window x+6
    sidx1 = ((ploc * P13) * WCV + xs + R)[:, None]
    # phase-2 CV stream const: row 18+yloc, col x+18; + oi_y*105 + oi_x
    c2cv = (((ploc * NROWS_G + MAXOFF + yloc) * WCV + xs + MAXOFF)
            - IDX_BIAS)[:, None]
    # phase-2 QI stream const (element units): ((y+18+yloc)*105 + x+18)*448
    c2qr = ((((yloc + MAXOFF) * WCV + xs + MAXOFF) - IDX_BIAS) * 448)[:, None]

    gridy = np.tile((np.repeat(np.arange(P13) - R, P13) * DIL)[None, :],
                    (PPG, 1)).astype(np.float32)
    gridx = np.tile((np.tile(np.arange(P13) - R, P13) * DIL)[None, :],
                    (PPG, 1)).astype(np.float32)

    xq = np.arange(WB)[:, None]
    xx = np.arange(W)[None, :]
    maskT = ((xq - xx >= 0) & (xq - xx <= 2 * R)).astype(np.float32)
    maskT_tiled = np.ascontiguousarray(
        np.tile(maskT[:, None, :], (1, P13, 1)).reshape(WB, P13 * W))

    def b16(a):
        return np.ascontiguousarray(a.astype(NPBF16))

    in_maps = []
    for core in range(NCORES):
        y0 = core * RY
        f1pair = np.zeros((CF, 4 * PPG), np.float32)
        for g in range(4):
            f1pair[:, g * PPG:g * PPG + W] = f1[:, y0 + 2 * g, :]
            if 2 * g + 1 < RY:
                f1pair[:, g * PPG + 64:g * PPG + 64 + W] = f1[:, y0 + 2 * g + 1, :]
        m = dict(
            f1=b16(f1[:, y0:y0 + RY, :].reshape(CF, RY * W)),
            f1pair=b16(f1pair),
            f2p0=b16(
                f2p0[:, y0:y0 + H_SLAB, :].reshape(CF, H_SLAB * WCV)),
            f2p1=b16(f2p12[0][:, y0:y0 + NRQ, :].reshape(CF, NRQ * WB)),
            f2p2=b16(f2p12[1][:, y0:y0 + NRQ, :].reshape(CF, NRQ * WB)),
            qrT1=b16(qrpT[0][:, y0:y0 + NRQ, :].reshape(WB, NRQ * (C + 1))),
            qrT2=b16(qrpT[1][:, y0:y0 + NRQ, :].reshape(WB, NRQ * (C + 1))),
            qr0pad=qi_b16,
            idx1=sidx1.astype(np.int32),
            c2cv=c2cv.astype(np.float32),
            c2qr=(c2qr + y0 * WCV * 448).astype(np.float32),
            gridx=gridx, gridy=gridy,
            maskT=b16(maskT_tiled),
        )
        in_maps.append(m)
    return in_maps


INPUT_SPECS = dict(
    f1=([CF, RY * W], BF16), f1pair=([CF, 4 * PPG], BF16),
    f2p0=([CF, H_SLAB * WCV], BF16),
    f2p1=([CF, NRQ * WB], BF16), f2p2=([CF, NRQ * WB], BF16),
    qrT1=([WB, NRQ * (C + 1)], BF16), qrT2=([WB, NRQ * (C + 1)], BF16),
    qr0pad=([1, HP * HP * 14 * C], BF16),
    idx1=([PPG, 1], I32), c2cv=([PPG, 1], F32), c2qr=([PPG, 1], F32),
    gridx=([PPG, N169], F32), gridy=([PPG, N169], F32),
    maskT=([WB, P13 * W], BF16),
)
OUT_SPEC = ([RY * W, C], F32)


def build_kernel(tc, outs, ins):
    nc = tc.nc
    Exp = mybir.ActivationFunctionType.Exp
    ALU = mybir.AluOpType
    AX = mybir.AxisListType

    with ExitStack() as ctx:
        sb = ctx.enter_context(tc.tile_pool(name="sb", bufs=1))
        sbg = ctx.enter_context(tc.tile_pool(name="sbg", bufs=2))
        sbg3 = ctx.enter_context(tc.tile_pool(name="sbg3", bufs=3))
        ps_cv = ctx.enter_context(tc.tile_pool(name="ps_cv", bufs=2, space="PSUM"))
        ps_cc = ctx.enter_context(tc.tile_pool(name="ps_cc", bufs=2, space="PSUM"))
        ps_out = ctx.enter_context(tc.tile_pool(name="ps_out", bufs=2, space="PSUM"))
        dram = ctx.enter_context(tc.tile_pool(name="dram", bufs=1, space="DRAM"))

        def load(name, dtype=None):
            shape, dt_ = INPUT_SPECS[name]
            t = sb.tile(shape, dtype or dt_, tag=name)
            nc.sync.dma_start(t[:], ins[name])
            return t

        f1_t = load("f1")
        f1pair_t = load("f1pair")
        f2p0_t = load("f2p0")
        f2p12_t = [load("f2p1"), load("f2p2")]
        qrT_t = [load("qrT1"), load("qrT2")]
        idx1_t = load("idx1")
        c2cv_t = load("c2cv")
        c2qr_t = load("c2qr")
        gridx_t = load("gridx")
        gridy_t = load("gridy")
        maskT_t = load("maskT")

        out0_g, z0_g = {}, {}

        # ---------- E/F. refs 1/2 + combine (emitted per group) ----------
        def emit_row(yr):
            op = ps_out.tile([W, C + 1], F32, tag="op")
            first = True
            for r in (0, 1):
                em = sbg.tile([WB, P13 * W], BF16, tag="em")
                for h0, hn in ((0, 7), (7, 6)):
                    ct = ps_cc.tile([WB, 7 * W], F32, tag="ct")
                    for i in range(hn):
                        iy = h0 + i
                        nc.tensor.matmul(
                            ct[:, i * W:(i + 1) * W],
                            lhsT=f2p12_t[r][:, (yr + iy) * WB:(yr + iy + 1) * WB],
                            rhs=f1_t[:, yr * W:(yr + 1) * W],
                            start=True, stop=True)
                    nc.scalar.activation(em[:, h0 * W:(h0 + hn) * W],
                                         ct[:, 0:hn * W], Exp)
                nc.vector.tensor_tensor(em[:], em[:], maskT_t[:], op=ALU.mult)
                for iy in range(P13):
                    nc.tensor.matmul(
                        op[:], lhsT=em[:, iy * W:(iy + 1) * W],
                        rhs=qrT_t[r][:, (yr + iy) * (C + 1):
                                     (yr + iy + 1) * (C + 1)],
                        start=first, stop=(r == 1 and iy == P13 - 1))
                    first = False
            g = yr // 2
            p_lo = 64 * (yr % 2)
            psl = slice(p_lo, p_lo + W)
            den = sbg.tile([W, 2], F32, tag="den")
            nc.vector.tensor_tensor(den[:, 0:1], op[:, C:C + 1],
                                    z0_g[g][psl, N169:N169 + 1], op=ALU.add)
            nc.vector.reciprocal(den[:, 1:2], den[:, 0:1])
            of = sbg.tile([W, C], F32, tag="of")
            nc.vector.tensor_tensor(of[:], op[:, 0:C], out0_g[g][psl, :],
                                    op=ALU.add)
            nc.vector.tensor_scalar(of[:], of[:], den[:, 1:2], None,
                                    op0=ALU.mult)
            nc.sync.dma_start(
                outs["out"].rearrange("(y x) c -> y x c", y=RY)[yr], of[:])


        for g, yg, MP in GROUPS:
            nrow = NROWS_G
            # ---------- A. CV ----------
            cv_sb = sbg.tile([MP, nrow * WCV], BF16, tag="cv_sb")
            lhs = f1pair_t[:, g * PPG:(g + 1) * PPG]
            CH = 8
            ci = 0
            for r0 in range(0, nrow, CH):
                rn = min(CH, nrow - r0)
                pt = ps_cv.tile([MP, CH * 128], F32, tag="cvch")
                for r in range(rn):
                    row = 2 * g + r0 + r
                    nc.tensor.matmul(
                        pt[:, r * 128:r * 128 + WCV],
                        lhsT=lhs, rhs=f2p0_t[:, row * WCV:(row + 1) * WCV],
                        start=True, stop=True)
                dst = cv_sb[:, r0 * WCV:(r0 + rn) * WCV].rearrange(
                    "p (r w) -> p r w", r=rn)
                src = pt[:].rearrange("p (r w) -> p r w", r=CH)[:, 0:rn, 0:WCV]
                if ci % 2 == 0:
                    nc.vector.tensor_copy(dst, src)
                else:
                    nc.scalar.copy(dst, src)
                ci += 1
            cv_dram = dram.tile([1, MP * nrow * WCV], BF16, tag=f"cvd{g}")
            nc.sync.dma_start(
                cv_dram[:].rearrange("o (p f) -> p (o f)", p=MP), cv_sb[:])
            # compact stride-3 rows for phase-1: cvcc [p, 13, 105]
            cvcc = dram.tile([1, MP * P13 * WCV], BF16, tag=f"cvcc{g}")
            ccv_w = cvcc[:].rearrange("o (p i w) -> p i (o w)", p=MP, i=P13)
            for pl, base in ((slice(0, 64), R), (slice(64, 128), R + 1)):
                sl = cv_sb[pl]
                src = bass.AP(sl.tensor, sl.offset + base * WCV,
                              [sl.ap[0], [3 * WCV, P13], [1, WCV]])
                nc.sync.dma_start(ccv_w[pl], src)

            # ---------- B. phase-1 (single stream per pixel, compact) ----------
            NS1 = 12 * WCV + CC_RUN
            g1 = sbg3.tile([MP, NS1], BF16, tag="g1")
            nc.gpsimd.indirect_dma_start(
                out=g1[:], out_offset=None, in_=cvcc[:],
                in_offset=bass.IndirectOffsetOnAxis(ap=idx1_t[0:MP, :], axis=1))
            cc0 = bass.AP(g1[:].tensor, g1[:].offset,
                          [g1[:].ap[0], [WCV, P13], [3, P13]])
            e1 = sbg.tile([MP, N169 + 1], F32, tag="e1")
            nc.scalar.activation(
                e1[:, 0:N169].rearrange("p (i j) -> p i j", i=P13), cc0, Exp,
                accum_out=e1[:, N169:N169 + 1])
            sc = sbg.tile([MP, 4], F32, tag="sc")
            tmp = sbg.tile([MP, N169], F32, tag="tmp169")
            nc.vector.scalar_tensor_tensor(
                out=tmp[:], in0=e1[:, 0:N169], scalar=0.0, in1=gridx_t[0:MP, :],
                op0=ALU.add, op1=ALU.mult, accum_out=sc[:, 0:1])
            nc.vector.scalar_tensor_tensor(
                out=tmp[:], in0=e1[:, 0:N169], scalar=0.0, in1=gridy_t[0:MP, :],
                op0=ALU.add, op1=ALU.mult, accum_out=sc[:, 1:2])
            offs = sbg.tile([MP, 2], F32, tag="offs")   # [off_x, off_y]
            nc.vector.reciprocal(sc[:, 2:3], e1[:, N169:N169 + 1])
            nc.vector.tensor_tensor(offs[:, 0:1], sc[:, 0:1], sc[:, 2:3],
                                    op=ALU.mult)
            nc.vector.tensor_tensor(offs[:, 1:2], sc[:, 1:2], sc[:, 2:3],
                                    op=ALU.mult)
            nc.vector.tensor_scalar(offs[:], offs[:], float(MAXOFF),
                                    -float(MAXOFF), op0=ALU.min, op1=ALU.max)
            # floor (mode-agnostic): fb = off+1024; fbi=cast; fbf=cast back;
            # fbf -= (fb - fbf < 0); wfrac = fb - fbf; fbi2 = cast(fbf)
            fb = sbg.tile([MP, 2], F32, tag="fb")
            nc.vector.tensor_scalar(fb[:], offs[:], FLOOR_BIAS, None,
                                    op0=ALU.add)
            fbi = sbg.tile([MP, 2], I32, tag="fbi")
            nc.vector.tensor_copy(fbi[:], fb[:])
            fbf = sbg.tile([MP, 2], F32, tag="fbf")
            nc.vector.tensor_copy(fbf[:], fbi[:])
            err = sbg.tile([MP, 2], F32, tag="err")
            nc.vector.tensor_tensor(err[:], fb[:], fbf[:], op=ALU.subtract)
            neg = sbg.tile([MP, 2], F32, tag="neg")
            nc.vector.tensor_scalar(neg[:], err[:], 0.0, None, op0=ALU.is_lt)
            nc.vector.tensor_tensor(fbf[:], fbf[:], neg[:], op=ALU.subtract)
            wfrac = sbg.tile([MP, 2], F32, tag="wfrac")  # [wx, wy]
            nc.vector.tensor_tensor(wfrac[:], fb[:], fbf[:], op=ALU.subtract)
            s2 = sbg.tile([MP, 1], F32, tag="s2")
            nc.vector.scalar_tensor_tensor(
                out=s2[:], in0=fbf[:, 1:2], scalar=float(WCV),
                in1=fbf[:, 0:1], op0=ALU.mult, op1=ALU.add)
            idx2cvf = sbg.tile([MP, 1], F32, tag="idx2cvf")
            nc.vector.tensor_scalar(idx2cvf[:], c2cv_t[0:MP, :], s2[:], None,
                                    op0=ALU.add)
            idx2cv = sbg.tile([MP, 1], I32, tag="idx2cv")
            nc.vector.tensor_copy(idx2cv[:], idx2cvf[:])
            # QI element index: c2qr + (s2 + yg*WCV)*448
            idx2qrf = sbg.tile([MP, 1], F32, tag="idx2qrf")
            nc.vector.tensor_scalar(idx2qrf[:], s2[:], 448.0,
                                    float(yg * WCV * 448),
                                    op0=ALU.mult, op1=ALU.add)
            nc.vector.tensor_tensor(idx2qrf[:], idx2qrf[:], c2qr_t[0:MP, :],
                                    op=ALU.add)
            idx2qr = sbg.tile([MP, 1], I32, tag="idx2qr")
            nc.vector.tensor_copy(idx2qr[:], idx2qrf[:])

            # ---------- C. phase-2 gathers (single stream per pixel) ----------
            NS2 = 13 * WCV + 14
            g2 = sbg3.tile([MP, NS2], BF16, tag="g2")
            nc.gpsimd.indirect_dma_start(
                out=g2[:], out_offset=None, in_=cv_dram[:],
                in_offset=bass.IndirectOffsetOnAxis(ap=idx2cv[:], axis=1))
            qt = sbg3.tile([MP, 14 * 448], BF16, tag="qt")
            nc.gpsimd.indirect_dma_start(
                out=qt[:], out_offset=None, in_=ins["qr0pad"],
                in_offset=bass.IndirectOffsetOnAxis(ap=idx2qr[:], axis=1))

            # ---------- D. ref0 ----------
            ww = sbg.tile([MP, 4], F32, tag="ww")
            om = sbg.tile([MP, 2], F32, tag="om")
            nc.vector.tensor_scalar(om[:], wfrac[:], -1.0, 1.0,
                                    op0=ALU.mult, op1=ALU.add)
            nc.vector.tensor_tensor(ww[:, 0:1], om[:, 1:2], om[:, 0:1],
                                    op=ALU.mult)
            nc.vector.tensor_tensor(ww[:, 1:2], om[:, 1:2], wfrac[:, 0:1],
                                    op=ALU.mult)
            nc.vector.tensor_tensor(ww[:, 2:3], wfrac[:, 1:2], om[:, 0:1],
                                    op=ALU.mult)
            nc.vector.tensor_tensor(ww[:, 3:4], wfrac[:, 1:2], wfrac[:, 0:1],
                                    op=ALU.mult)

            g2v = bass.AP(g2[:].tensor, g2[:].offset,
                          [g2[:].ap[0], [WCV, 14], [1, 14]])
            corr = sbg.tile([MP, N169], F32, tag="corr")
            crv = corr[:].rearrange("p (i j) -> p i j", i=P13)
            nc.vector.tensor_scalar(crv, g2v[:, 0:13, 0:13], ww[:, 0:1], None,
                                    op0=ALU.mult)
            for (sl_u, sl_v, wcol) in (((0, 13), (1, 14), 1),
                                       ((1, 14), (0, 13), 2),
                                       ((1, 14), (1, 14), 3)):
                nc.vector.scalar_tensor_tensor(
                    out=crv, in0=g2v[:, sl_u[0]:sl_u[1], sl_v[0]:sl_v[1]],
                    scalar=ww[:, wcol:wcol + 1], in1=crv,
                    op0=ALU.mult, op1=ALU.add)
            p0 = sbg.tile([MP, N169 + 1], F32, tag="p0")
            nc.scalar.activation(p0[:, 0:N169], corr[:], Exp,
                                 accum_out=p0[:, N169:N169 + 1])
            z0_g[g] = p0
            bb = sbg.tile([MP, 196], BF16, tag="bb")
            nc.vector.memset(bb[:], 0.0)
            bbv = bb[:].rearrange("p (u v) -> p u v", u=14)
            p0v = p0[:, 0:N169].rearrange("p (i j) -> p i j", i=P13)
            nc.vector.tensor_scalar(bbv[:, 0:13, 0:13], p0v, ww[:, 0:1], None,
                                    op0=ALU.mult)
            for (sl_u, sl_v, wcol) in (((0, 13), (1, 14), 1),
                                       ((1, 14), (0, 13), 2),
                                       ((1, 14), (1, 14), 3)):
                dstv = bbv[:, sl_u[0]:sl_u[1], sl_v[0]:sl_v[1]]
                nc.vector.scalar_tensor_tensor(
                    out=dstv, in0=p0v, scalar=ww[:, wcol:wcol + 1], in1=dstv,
                    op0=ALU.mult, op1=ALU.add)
            qtv = bass.AP(qt[:].tensor, qt[:].offset,
                          [qt[:].ap[0], [32, 14], [448, 14], [1, C]])  # (u,v,c)
            bbb = bbv.to_broadcast([MP, 14, 14, C])
            nc.vector.tensor_tensor(qtv, qtv, bbb, op=ALU.mult)
            o0 = sbg.tile([MP, C], F32, tag="o0")
            pr = bass.AP(qt[:].tensor, qt[:].offset,
                         [qt[:].ap[0], [1, C], [32, 14], [448, 14]])
            nc.vector.tensor_reduce(o0[:], pr, axis=AX.XY, op=ALU.add)
            out0_g[g] = o0
            for yr in (2 * g, 2 * g + 1):
                if yr < RY:
                    emit_row(yr)
            if g == 0 and "dbg_qt" in outs:
                nc.sync.dma_start(outs["dbg_qt"], qt[:])
                nc.sync.dma_start(outs["dbg_g2"], g2[:])
                nc.sync.dma_start(outs["dbg_idx2qr"], idx2qr[:])
                nc.sync.dma_start(outs["dbg_idx2cv"], idx2cv[:])
                nc.sync.dma_start(outs["dbg_o0"], o0[:])
                nc.sync.dma_start(outs["dbg_bb"], bb[:])
                nc.sync.dma_start(outs["dbg_offs"], offs[:])
                nc.sync.dma_start(outs["dbg_e1"], e1[:])
                nc.sync.dma_start(outs["dbg_g1"], g1[:])




# ---------------- numpy mirror of one core (debug) ----------------
def core_reference(m):
    m = {k: (np.asarray(v, np.float32) if v.dtype != np.int32 else v)
         for k, v in m.items()}
    f1 = m["f1"].reshape(CF, RY, W)
    f2p0 = m["f2p0"].reshape(CF, H_SLAB, WCV)
    o0full = np.zeros((RY, W, C), np.float32)
    z0full = np.zeros((RY, W, 1), np.float32)
    for g, yg, MP in GROUPS:
        nrow = NROWS_G
        lhs = m["f1pair"][:, g * PPG:(g + 1) * PPG].astype(np.float32)
        cv = np.einsum('cp,crw->prw', lhs, f2p0[:, 2 * g:2 * g + nrow, :])
        cvf = np.ascontiguousarray(cv).reshape(-1)
        yy = (np.arange(MP) >= 64).astype(np.int64)
        rows = (R + yy[:, None, None] + 3 * np.arange(P13)[None, :, None])
        cvcc = np.take_along_axis(
            cv, np.broadcast_to(rows, (MP, P13, WCV)), axis=1)
        ccf = np.ascontiguousarray(cvcc).reshape(-1)
        sidx = m["idx1"][:MP, 0] - (np.arange(MP) * NROWS_G - np.arange(MP) * P13) * WCV
        # device sidx indexes cvcc directly: (p*13)*WCV + x + 6
        sidx = (np.arange(MP) * P13 * WCV
                + np.minimum(np.arange(MP) - 64 * yy, W - 1) + R)
        NS1 = 12 * WCV + CC_RUN
        g1 = ccf[sidx[:, None] + np.arange(NS1)[None, :]]
        cc0 = np.stack([g1[:, i * WCV + 3 * np.arange(P13)]
                        for i in range(P13)], 1).reshape(MP, N169)
        e1 = np.exp(cc0)
        S = e1.sum(1, keepdims=True)
        offx = np.clip((e1 * m["gridx"][:MP]).sum(1, keepdims=True) / S,
                       -MAXOFF, MAXOFF)
        offy = np.clip((e1 * m["gridy"][:MP]).sum(1, keepdims=True) / S,
                       -MAXOFF, MAXOFF)
        fbx = np.floor(offx + FLOOR_BIAS)
        fby = np.floor(offy + FLOOR_BIAS)
        wx = (offx + FLOOR_BIAS) - fbx
        wy = (offy + FLOOR_BIAS) - fby
        s2 = (fby.astype(np.int64) * WCV + fbx.astype(np.int64))
        idx2cv = (m["c2cv"][:MP, 0] + s2[:, 0]).astype(np.int64)
        idx2qr = (m["c2qr"][:MP, 0] + (s2[:, 0] + yg * WCV) * 448).astype(np.int64)
        NS2 = 13 * WCV + 14
        g2s = cvf[idx2cv[:, None] + np.arange(NS2)[None, :]]
        g2 = np.stack([g2s[:, u * WCV:u * WCV + 14] for u in range(14)], 1)
        qrf = m["qr0pad"].reshape(-1)
        qts = qrf[idx2qr[:, None] + np.arange(6272)[None, :]]
        # stream order (v, u, c) -> [MP, u, v, c]
        qt = qts.reshape(MP, 14, 14, C).transpose(0, 2, 1, 3)
        w00 = (1 - wy) * (1 - wx); w01 = (1 - wy) * wx
        w10 = wy * (1 - wx); w11 = wy * wx
        corr = (w00 * g2[:, 0:13, 0:13].reshape(MP, N169)
                + w01 * np.ascontiguousarray(g2[:, 0:13, 1:14]).reshape(MP, N169)
                + w10 * np.ascontiguousarray(g2[:, 1:14, 0:13]).reshape(MP, N169)
                + w11 * np.ascontiguousarray(g2[:, 1:14, 1:14]).reshape(MP, N169))
        p0 = np.exp(corr)
        z0 = p0.sum(1, keepdims=True)
        bb = np.zeros((MP, 14, 14), np.float32)
        p0v = p0.reshape(MP, P13, P13)
        bb[:, 0:13, 0:13] += w00[..., None] * p0v
        bb[:, 0:13, 1:14] += w01[..., None] * p0v
        bb[:, 1:14, 0:13] += w10[..., None] * p0v
        bb[:, 1:14, 1:14] += w11[..., None] * p0v
        o0 = (qt * bb[..., None]).sum((1, 2))
        for yloc in range(2):
            if yg + yloc >= RY:
                continue
            o0full[yg + yloc] = o0[64 * yloc:64 * yloc + W]
            z0full[yg + yloc] = z0[64 * yloc:64 * yloc + W]
    out = np.zeros((RY, W, C), np.float32)
    maskT = m["maskT"].reshape(WB, P13, W)[:, 0, :]
    for yr in range(RY):
        acc = np.zeros((W, C + 1), np.float32)
        for r in range(2):
            f2p = m[f"f2p{r + 1}"].reshape(CF, NRQ, WB)
            qrT = m[f"qrT{r + 1}"].reshape(WB, NRQ, C + 1)
            for iy in range(P13):
                ct = np.einsum('cq,cx->qx', f2p[:, yr + iy, :], f1[:, yr, :])
                em = np.exp(ct) * maskT
                acc += np.einsum('qx,qd->xd', em, qrT[:, yr + iy, :])
        den = acc[:, C:C + 1] + z0full[yr]
        out[yr] = (acc[:, :C] + o0full[yr]) / den
    return out


def full_reference_from_cores(in_maps):
    outs = [core_reference(in_maps[i]) for i in range(NCORES)]
    full = np.stack(outs, 0)            # [8, 7, 56, C]
    return full.reshape(H, W, C).transpose(2, 0, 1)[None]


DEBUG_SPECS = dict(
    dbg_qt=([128, 14 * 448], F32), dbg_g2=([128, 196], F32),
    dbg_idx2qr=([128, 14], I32), dbg_idx2cv=([128, 14], I32),
    dbg_o0=([128, C], F32), dbg_bb=([128, 196], F32),
    dbg_offs=([128, 2], F32), dbg_e1=([128, N169 + 1], F32),
    dbg_g1=([128, P13 * CC_RUN], F32),
)


def build_program(ncores=NCORES, debug=False):
    import concourse.bacc as bacc
    nc = bacc.Bacc("TRN2", target_bir_lowering=False, debug=False,
                   enable_asserts=True, num_devices=ncores)
    ins = {}
    for name, (shape, dt_) in INPUT_SPECS.items():
        ins[name] = nc.dram_tensor(name, shape, dt_, kind="ExternalInput").ap()
    outs = {"out": nc.dram_tensor("out", OUT_SPEC[0], OUT_SPEC[1],
                                  kind="ExternalOutput").ap()}
    if debug:
        for name, (shape, dt_) in DEBUG_SPECS.items():
            outs[name] = nc.dram_tensor(name, shape, dt_,
                                        kind="ExternalOutput").ap()
    with tile.TileContext(nc) as tc:
        build_kernel(tc, outs, ins)
    nc.compile()
    return nc


# ======================= runner =======================
import os as _os


def _build_program():
    import concourse.bacc as bacc
    nc = bacc.Bacc("TRN2", target_bir_lowering=False, debug=False,
                   enable_asserts=True, num_devices=NCORES)
    ins = {}
    for name, (shape, dt_) in INPUT_SPECS.items():
        ins[name] = nc.dram_tensor(name, shape, dt_, kind="ExternalInput").ap()
    outs = {"out": nc.dram_tensor("out", OUT_SPEC[0], OUT_SPEC[1],
                                  kind="ExternalOutput").ap()}
    with tile.TileContext(nc) as tc:
        build_kernel(tc, outs, ins)
    nc.compile()
    return nc


_LAST_RESULT = {}


def kernel(**inputs):
    from concourse.bass_utils import run_bass_kernel_spmd
    from concourse.bass_interp import get_hw_module

    in_maps = host_prep(**inputs)
    nc = _build_program()
    nc.m = get_hw_module(nc.m)
    trace = _os.environ.get("KERNEL_TRACE", "0") == "1"
    res = run_bass_kernel_spmd(
        nc, in_maps, core_ids=list(range(NCORES)), trace=trace)
    _LAST_RESULT["res"] = res
    slabs = [np.asarray(res.results[i]["out"], np.float32).reshape(RY, W, C)
             for i in range(NCORES)]
    full = np.concatenate(slabs, 0)          # [56, 56, 32]
    return np.ascontiguousarray(full.transpose(2, 0, 1)[None])



# revision 32
# speedup vs baseline: 25.7350x; 1.1699x over previous
"""Bass/Tile kernel for nn_Colorizer (sparse deformable attention colorizer).

Sharding: spatial row-sharding across 8 cores; core i owns output rows
[7i, 7i+7). The joint softmax over nref*N is additive across refs so each
core normalizes locally.

Per-core pipeline:
  A. CV volume (search ref): 4-row-batched PE matmuls -> PSUM -> bf16 SBUF
     (copies spread over DVE/Act/Pool) -> cv_dram [p, 51, 105].
  B. Phase-1: static strided DMAs pull the stride-3 13x37 runs straight
     from cv_dram -> exp -> expected offset -> floor/frac -> gather idxs.
  C. Phase-2: 14-run CV window gather (196 el) + qr0 QI gather (u-inner
     interleave -> [v,c,u] runs).
  D. Ref0: bilinear blend -> exp -> B-blur (v-major) -> packed bf16
     multiply + fold -> reduce -> o0, Z0.
  E. Refs 1/2 (s-major): per f2 row s one banded cc matmul [68, w_s*56]
     -> exp into packed em -> one mask multiply per ref -> per-s
     attention matmul accumulated into op2 [33, 392] (d-major).
  F. Combine: PE-transpose op2 per row -> (+o0)/(Z) -> staged -> DRAM.
"""
from contextlib import ExitStack

import numpy as np
import ml_dtypes

import concourse.bass as bass

NPBF16 = ml_dtypes.bfloat16
import concourse.mybir as mybir
import concourse.tile as tile

F32 = mybir.dt.float32
I32 = mybir.dt.int32
BF16 = mybir.dt.bfloat16

# ---------------- geometry ----------------
D_SUB, R, C = 4, 6, 32
P13 = 2 * R + 1          # 13
N169 = P13 * P13
DIL_INT = 15
H = W = 56
CF = 64
NCORES = 8
RY = H // NCORES         # 7

DIL = 3
MAXOFF = R * DIL         # 18
DYLO = MAXOFF + R        # 24
NDY = 2 * DYLO + 2       # 50 rows needed for one y-row
WCV = W + NDY - 1        # 105
NROWS_G = NDY + 1        # 51 rows per pair group
HP = H + NDY - 1         # 105
WB = W + 2 * R           # 68
H_SLAB = H + 1           # 57: uniform 51-row pitch for all 4 groups
NRQ = RY + 2 * R         # 19
CC_RUN = 3 * (P13 - 1) + 1   # 37

FLOOR_BIAS = 1024.0
IDX_BIAS = int(FLOOR_BIAS) * WCV + int(FLOOR_BIAS)

GROUPS = [(0, 0, 128), (1, 2, 128), (2, 4, 128), (3, 6, 128)]
PPG = 128  # partitions per group: rows at offsets 0 and 64

# s-major banded attention: for f2 slab row s, valid yr in [LO[s], HI[s]]
S_LO = [max(0, s - (P13 - 1)) for s in range(NRQ)]
S_HI = [min(RY - 1, s) for s in range(NRQ)]
S_W = [(S_HI[s] - S_LO[s] + 1) * W for s in range(NRQ)]
S_OFF = np.concatenate([[0], np.cumsum(S_W)]).tolist()
EM_N = S_OFF[-1]          # 91*56 = 5096

CVP = NROWS_G * WCV       # 5355 per-pixel cv pitch
QT_N = 14 * 448           # qt stream: v(14) x c(32) x u(14)


def _pad2(a, top, left, hh, ww):
    out = np.zeros(a.shape[:-2] + (hh, ww), a.dtype)
    out[..., top:top + a.shape[-2], left:left + a.shape[-1]] = a
    return out


def host_prep(feats_r, feats_t, quantized_r, ref_index, current_ind):
    feats_r = np.asarray(feats_r, np.float32)
    feats_t = np.asarray(feats_t, np.float32)
    quantized_r = np.asarray(quantized_r, np.float32)
    ri = np.asarray(ref_index).tolist()
    ci = int(current_ind)
    diffs = [ci - int(x) for x in ri]
    nsearch = sum(1 for d in diffs if d > DIL_INT)
    dirates = [min(4, d // DIL_INT + 1) for d in diffs if d > DIL_INT]
    nref = feats_r.shape[0]
    assert nsearch == 1 and dirates[0] == DIL and nref == 3, \
        (nsearch, dirates, nref)

    f1 = feats_t[0]
    f2 = [feats_r[s, 0] for s in range(nref)]
    qr = [quantized_r[s, 0][:, ::D_SUB, ::D_SUB] for s in range(nref)]

    # u-inner interleaved qr0: QI[y, x, c, u] = qr0pad[y+u, x, c]
    qr0can = np.zeros((HP + 14, HP, C), np.float32)
    qr0can[DYLO:DYLO + H, DYLO:DYLO + W, :] = qr[0].transpose(1, 2, 0)
    qi = np.stack([qr0can[u:u + HP] for u in range(14)], axis=3)  # [HP,HP,C,14]
    qi = qi.reshape(1, HP * HP * C * 14)
    qi_b16 = np.ascontiguousarray(qi.astype(NPBF16))

    # f2_0 canvas: rows [-24 .. H+26], cols [-24 .. 80]
    f2p0 = _pad2(f2[0], DYLO, DYLO, H + 2 * DYLO + 3, WCV)
    f2p12 = [_pad2(f2[r], R, R, H + 2 * R, WB) for r in (1, 2)]
    qrpT = []
    for r in (1, 2):
        q = np.zeros((H + 2 * R, WB, C + 1), np.float32)
        q[R:R + H, R:R + W, :C] = qr[r].transpose(1, 2, 0)
        q[:, :, C] = 1.0
        qrpT.append(np.ascontiguousarray(q.transpose(1, 0, 2)))

    ploc128 = np.arange(PPG)
    yloc = (ploc128 >= 64).astype(np.int64)
    xs = np.minimum(ploc128 - 64 * yloc, W - 1)
    # phase-2 CV stream const: row 18+yloc, col x+18; + oi_y*105 + oi_x
    c2cv = (((ploc128 * NROWS_G + MAXOFF + yloc) * WCV + xs + MAXOFF)
            - IDX_BIAS)[:, None]
    # phase-2 QI stream const (element units): ((y+18+yloc)*105 + x+18)*448
    c2qr = ((((yloc + MAXOFF) * WCV + xs + MAXOFF) - IDX_BIAS) * 448)[:, None]

    gridy = np.tile((np.repeat(np.arange(P13) - R, P13) * DIL)[None, :],
                    (PPG, 1)).astype(np.float32)
    gridx = np.tile((np.tile(np.arange(P13) - R, P13) * DIL)[None, :],
                    (PPG, 1)).astype(np.float32)
    uconst = np.tile((np.arange(14) * WCV)[None, :], (PPG, 1)).astype(np.float32)

    xq = np.arange(WB)[:, None]
    xx = np.arange(W)[None, :]
    maskT = ((xq - xx >= 0) & (xq - xx <= 2 * R)).astype(np.float32)  # [68,56]
    maskM = np.tile(maskT[:, None, :], (1, EM_N // W, 1)).reshape(WB, EM_N)
    ident33 = np.zeros((PPG, C + 1), np.float32)
    ident33[:C + 1, :] = np.eye(C + 1)

    def b16(a):
        return np.ascontiguousarray(a.astype(NPBF16))

    in_maps = []
    for core in range(NCORES):
        y0 = core * RY
        f1pair = np.zeros((CF, 4 * PPG), np.float32)
        for g in range(4):
            f1pair[:, g * PPG:g * PPG + W] = f1[:, y0 + 2 * g, :]
            if 2 * g + 1 < RY:
                f1pair[:, g * PPG + 64:g * PPG + 64 + W] = f1[:, y0 + 2 * g + 1, :]
        # 64-partition pack: f1 | f1pair | f2p0 | f2p1 | f2p2
        packA = np.concatenate([
            f1[:, y0:y0 + RY, :].reshape(CF, RY * W),
            f1pair,
            f2p0[:, y0:y0 + H_SLAB, :].reshape(CF, H_SLAB * WCV),
            f2p12[0][:, y0:y0 + NRQ, :].reshape(CF, NRQ * WB),
            f2p12[1][:, y0:y0 + NRQ, :].reshape(CF, NRQ * WB),
        ], axis=1)
        # 68-partition pack: qrT1 | qrT2 | maskM
        packB = np.concatenate([
            qrpT[0][:, y0:y0 + NRQ, :].reshape(WB, NRQ * (C + 1)),
            qrpT[1][:, y0:y0 + NRQ, :].reshape(WB, NRQ * (C + 1)),
            maskM,
        ], axis=1)
        # 128-partition f32 pack: gridx | gridy | uconst | c2cv | c2qr | id33
        packC = np.concatenate([
            gridx, gridy, uconst, c2cv, c2qr + y0 * WCV * 448, ident33,
        ], axis=1).astype(np.float32)
        m = dict(
            packA=b16(packA),
            packB=b16(packB),
            packC=packC,
            qr0pad=qi_b16,
        )
        in_maps.append(m)
    return in_maps


# pack offsets (elements)
A_F1 = 0
A_F1PAIR = A_F1 + RY * W
A_F2P0 = A_F1PAIR + 4 * PPG
A_F2P1 = A_F2P0 + H_SLAB * WCV
A_F2P2 = A_F2P1 + NRQ * WB
A_N = A_F2P2 + NRQ * WB
B_QRT1 = 0
B_QRT2 = B_QRT1 + NRQ * (C + 1)
B_MASK = B_QRT2 + NRQ * (C + 1)
B_N = B_MASK + EM_N
C_GX = 0
C_GY = C_GX + N169
C_UC = C_GY + N169
C_CV = C_UC + 14
C_QR = C_CV + 1
C_ID = C_QR + 1
C_N = C_ID + (C + 1)

INPUT_SPECS = dict(
    packA=([CF, A_N], BF16),
    packB=([WB, B_N], BF16),
    packC=([PPG, C_N], F32),
    qr0pad=([1, HP * HP * C * 14], BF16),
)
OUT_SPEC = ([RY * W, C], F32)


def build_kernel(tc, outs, ins):
    nc = tc.nc
    Exp = mybir.ActivationFunctionType.Exp
    Copy = mybir.ActivationFunctionType.Copy
    ALU = mybir.AluOpType
    AX = mybir.AxisListType

    with ExitStack() as ctx:
        sb = ctx.enter_context(tc.tile_pool(name="sb", bufs=1))
        sbg = ctx.enter_context(tc.tile_pool(name="sbg", bufs=2))
        sbe = ctx.enter_context(tc.tile_pool(name="sbe", bufs=2))
        sbq = ctx.enter_context(tc.tile_pool(name="sbq", bufs=2))
        ps_cv = ctx.enter_context(tc.tile_pool(name="ps_cv", bufs=3, space="PSUM"))
        ps_cc = ctx.enter_context(tc.tile_pool(name="ps_cc", bufs=2, space="PSUM"))
        ps_out = ctx.enter_context(tc.tile_pool(name="ps_out", bufs=1, space="PSUM"))
        ps_tr = ctx.enter_context(tc.tile_pool(name="ps_tr", bufs=2, space="PSUM"))
        dram = ctx.enter_context(tc.tile_pool(name="dram", bufs=1, space="DRAM"))

        def load(name):
            shape, dt_ = INPUT_SPECS[name]
            t = sb.tile(shape, dt_, tag=name)
            nc.sync.dma_start(t[:], ins[name])
            return t

        packA_t = load("packA")
        packB_t = load("packB")
        packC_t = load("packC")

        def pa(off, n):
            return packA_t[:, off:off + n]

        f1_t = pa(A_F1, RY * W)
        maskM_t = packB_t[:, B_MASK:B_MASK + EM_N]
        ident_t = packC_t[0:C + 1, C_ID:C_ID + (C + 1)]
        gridx_t = packC_t[:, C_GX:C_GX + N169]
        gridy_t = packC_t[:, C_GY:C_GY + N169]
        uconst_t = packC_t[:, C_UC:C_UC + 14]
        c2cv_t = packC_t[:, C_CV:C_CV + 1]
        c2qr_t = packC_t[:, C_QR:C_QR + 1]

        out0_g, z0_g = {}, {}

        # ---------- A+B+C+D per group ----------
        def emit_group(g, yg, MP):
            nrow = NROWS_G
            cv_sb = sbg.tile([MP, nrow * WCV], BF16, tag="cv_sb")
            lhs = pa(A_F1PAIR + g * PPG, PPG)
            ci = 0
            # 4 slab rows per matmul (420 cols in one PSUM bank)
            for r0 in range(0, nrow, 4):
                rn = min(4, nrow - r0)
                ncol = rn * WCV
                pt = ps_cv.tile([MP, 512], F32, tag="cvch")
                nc.tensor.matmul(
                    pt[:, 0:ncol], lhsT=lhs,
                    rhs=pa(A_F2P0 + (2 * g + r0) * WCV, ncol),
                    start=True, stop=True)
                dst = cv_sb[:, r0 * WCV:(r0 + rn) * WCV]
                if ci % 2 == 0:
                    nc.vector.tensor_copy(dst, pt[:, 0:ncol])
                else:
                    nc.scalar.copy(dst, pt[:, 0:ncol])
                ci += 1
            cv_dram = dram.tile([1, MP * CVP + 8 * WCV], BF16, tag=f"cvd{g}")
            nc.sync.dma_start(
                cv_dram[:, 0:MP * CVP].rearrange("o (p f) -> p (o f)", p=MP),
                cv_sb[:])

            # ---------- B. phase-1: static strided reads of cv ----------
            g1b = sbq.tile([MP, P13 * CC_RUN], BF16, tag="g1b")
            for pl, ybase in ((slice(0, 64), R), (slice(64, 128), R + 1)):
                p0_ = pl.start
                base = p0_ * CVP + ybase * WCV + R
                src = bass.AP(cv_dram[:].tensor, cv_dram[:].offset + base,
                              [[CVP + 1, 64], [3 * WCV, P13], [1, CC_RUN]])
                dst = g1b[pl].rearrange("p (i r) -> p i r", i=P13)
                nc.sync.dma_start(dst, src)
            cc0 = bass.AP(g1b[:].tensor, g1b[:].offset,
                          [g1b[:].ap[0], [CC_RUN, P13], [3, P13]])
            e1 = sbq.tile([MP, N169 + 1], F32, tag="e1")
            nc.scalar.activation(
                e1[:, 0:N169].rearrange("p (i j) -> p i j", i=P13), cc0, Exp,
                accum_out=e1[:, N169:N169 + 1])
            sc = sbq.tile([MP, 4], F32, tag="sc")
            tmp = sbq.tile([MP, N169], F32, tag="tmp169")
            nc.vector.scalar_tensor_tensor(
                out=tmp[:], in0=e1[:, 0:N169], scalar=0.0, in1=gridx_t,
                op0=ALU.add, op1=ALU.mult, accum_out=sc[:, 0:1])
            nc.vector.scalar_tensor_tensor(
                out=tmp[:], in0=e1[:, 0:N169], scalar=0.0, in1=gridy_t,
                op0=ALU.add, op1=ALU.mult, accum_out=sc[:, 1:2])
            offs = sbq.tile([MP, 2], F32, tag="offs")   # [off_x, off_y]
            nc.vector.reciprocal(sc[:, 2:3], e1[:, N169:N169 + 1])
            nc.vector.tensor_tensor(offs[:, 0:1], sc[:, 0:1], sc[:, 2:3],
                                    op=ALU.mult)
            nc.vector.tensor_tensor(offs[:, 1:2], sc[:, 1:2], sc[:, 2:3],
                                    op=ALU.mult)
            nc.vector.tensor_scalar(offs[:], offs[:], float(MAXOFF),
                                    -float(MAXOFF), op0=ALU.min, op1=ALU.max)
            # floor (mode-agnostic) on gpsimd: fb = off+1024; fbi=cast;
            # fbf=cast back; fbf -= (fb - fbf < 0); wfrac = fb - fbf
            fb = sbq.tile([MP, 2], F32, tag="fb")
            nc.gpsimd.tensor_scalar(fb[:], offs[:], FLOOR_BIAS, None,
                                    op0=ALU.add)
            fbi = sbq.tile([MP, 2], I32, tag="fbi")
            nc.gpsimd.tensor_copy(fbi[:], fb[:])
            fbf = sbq.tile([MP, 2], F32, tag="fbf")
            nc.gpsimd.tensor_copy(fbf[:], fbi[:])
            err = sbq.tile([MP, 2], F32, tag="err")
            nc.gpsimd.tensor_tensor(err[:], fb[:], fbf[:], op=ALU.subtract)
            neg = sbq.tile([MP, 2], F32, tag="neg")
            nc.gpsimd.tensor_scalar(neg[:], err[:], 0.0, None, op0=ALU.is_lt)
            nc.gpsimd.tensor_tensor(fbf[:], fbf[:], neg[:], op=ALU.subtract)
            wfrac = sbq.tile([MP, 2], F32, tag="wfrac")  # [wx, wy]
            nc.gpsimd.tensor_tensor(wfrac[:], fb[:], fbf[:], op=ALU.subtract)
            s2 = sbq.tile([MP, 1], F32, tag="s2")
            nc.vector.scalar_tensor_tensor(
                out=s2[:], in0=fbf[:, 1:2], scalar=float(WCV),
                in1=fbf[:, 0:1], op0=ALU.mult, op1=ALU.add)
            i1f = sbq.tile([MP, 1], F32, tag="i1f")
            nc.vector.tensor_scalar(i1f[:], c2cv_t, s2[:], None,
                                    op0=ALU.add)
            idx2cv = sbq.tile([MP, 1], I32, tag="idx2cv")
            nc.gpsimd.tensor_copy(idx2cv[:], i1f[:])
            # QI element index: c2qr + (s2 + yg*WCV)*448
            idx2qrf = sbq.tile([MP, 1], F32, tag="idx2qrf")
            nc.gpsimd.tensor_scalar(idx2qrf[:], s2[:], 448.0,
                                    float(yg * WCV * 448),
                                    op0=ALU.mult, op1=ALU.add)
            nc.gpsimd.tensor_tensor(idx2qrf[:], idx2qrf[:], c2qr_t,
                                    op=ALU.add)
            idx2qr = sbq.tile([MP, 1], I32, tag="idx2qr")
            nc.gpsimd.tensor_copy(idx2qr[:], idx2qrf[:])

            # ---------- C. phase-2 gathers ----------
            NS2 = 13 * WCV + 14
            g2 = sbq.tile([MP, NS2], BF16, tag="g2")
            nc.gpsimd.indirect_dma_start(
                out=g2[:], out_offset=None, in_=cv_dram[:],
                in_offset=bass.IndirectOffsetOnAxis(ap=idx2cv[:], axis=1))
            if g == 0:
                g2_dbg.append(g2)
                g1b_dbg.append(g1b)
                offs_dbg.append(offs)
            qt = sbq.tile([MP, QT_N], BF16, tag="qt")
            nc.gpsimd.indirect_dma_start(
                out=qt[:], out_offset=None, in_=ins["qr0pad"],
                in_offset=bass.IndirectOffsetOnAxis(ap=idx2qr[:], axis=1))

            # ---------- D. ref0 ----------
            ww = sbq.tile([MP, 4], F32, tag="ww")
            om = sbq.tile([MP, 2], F32, tag="om")
            nc.vector.tensor_scalar(om[:], wfrac[:], -1.0, 1.0,
                                    op0=ALU.mult, op1=ALU.add)
            nc.vector.tensor_tensor(ww[:, 0:1], om[:, 1:2], om[:, 0:1],
                                    op=ALU.mult)
            nc.vector.tensor_tensor(ww[:, 1:2], om[:, 1:2], wfrac[:, 0:1],
                                    op=ALU.mult)
            nc.vector.tensor_tensor(ww[:, 2:3], wfrac[:, 1:2], om[:, 0:1],
                                    op=ALU.mult)
            nc.vector.tensor_tensor(ww[:, 3:4], wfrac[:, 1:2], wfrac[:, 0:1],
                                    op=ALU.mult)

            g2v = bass.AP(g2[:].tensor, g2[:].offset,
                          [g2[:].ap[0], [WCV, 14], [1, 14]])
            corr = sbq.tile([MP, N169], F32, tag="corr")
            crv = corr[:].rearrange("p (i j) -> p i j", i=P13)
            nc.vector.tensor_scalar(crv, g2v[:, 0:13, 0:13], ww[:, 0:1], None,
                                    op0=ALU.mult)
            for (sl_u, sl_v, wcol) in (((0, 13), (1, 14), 1),
                                       ((1, 14), (0, 13), 2),
                                       ((1, 14), (1, 14), 3)):
                nc.vector.scalar_tensor_tensor(
                    out=crv, in0=g2v[:, sl_u[0]:sl_u[1], sl_v[0]:sl_v[1]],
                    scalar=ww[:, wcol:wcol + 1], in1=crv,
                    op0=ALU.mult, op1=ALU.add)
            p0 = sb.tile([MP, N169 + 1], F32, tag=f"p0_{g}")
            nc.scalar.activation(p0[:, 0:N169], corr[:], Exp,
                                 accum_out=p0[:, N169:N169 + 1])
            z0_g[g] = p0
            # bilinear blur, v-major: bb[p, v, u]
            bb = sbq.tile([MP, 196], BF16, tag="bb")
            nc.vector.memset(bb[:], 0.0)
            bbv = bb[:].rearrange("p (v u) -> p v u", v=14)
            # p0 viewed as (j=v, i=u): AP dims j outer (stride 1), i inner
            p0ji = bass.AP(p0[:].tensor, p0[:].offset,
                           [p0[:].ap[0], [1, P13], [P13, P13]])
            nc.vector.tensor_scalar(bbv[:, 0:13, 0:13], p0ji, ww[:, 0:1],
                                    None, op0=ALU.mult)
            for (sl_v, sl_u, wcol) in (((1, 14), (0, 13), 1),
                                       ((0, 13), (1, 14), 2),
                                       ((1, 14), (1, 14), 3)):
                dstv = bbv[:, sl_v[0]:sl_v[1], sl_u[0]:sl_u[1]]
                nc.vector.scalar_tensor_tensor(
                    out=dstv, in0=p0ji, scalar=ww[:, wcol:wcol + 1], in1=dstv,
                    op0=ALU.mult, op1=ALU.add)
            # packed bf16 multiply qt *= bb (broadcast over c via 0-stride)
            qtv = bass.AP(qt[:].tensor, qt[:].offset,
                          [qt[:].ap[0], [448, 14], [14, C], [1, 14]])
            bbb = bass.AP(bb[:].tensor, bb[:].offset,
                          [bb[:].ap[0], [14, 14], [0, C], [1, 14]])
            nc.vector.tensor_tensor(qtv, qtv, bbb, op=ALU.mult)
            o0 = sb.tile([MP, C], F32, tag=f"o0_{g}")
            pr = bass.AP(qt[:].tensor, qt[:].offset,
                         [qt[:].ap[0], [14, C], [448, 14], [1, 14]])
            nc.vector.tensor_reduce(o0[:], pr, axis=AX.XY, op=ALU.add)
            out0_g[g] = o0

        # ---------- E. refs 1/2, s-major ----------
        def emit_ref(r):
            em = sbe.tile([WB, EM_N], BF16, tag="em")
            for s in range(NRQ):
                w = S_W[s]
                ct = ps_cc.tile([WB, 512], F32, tag="ct")
                nc.tensor.matmul(
                    ct[:, 0:w],
                    lhsT=pa((A_F2P1, A_F2P2)[r] + s * WB, WB),
                    rhs=f1_t[:, S_LO[s] * W:S_LO[s] * W + w],
                    start=True, stop=True)
                nc.scalar.activation(em[:, S_OFF[s]:S_OFF[s] + w],
                                     ct[:, 0:w], Exp)
            nc.vector.tensor_tensor(em[:], em[:], maskM_t, op=ALU.mult)
            return em

        def emit_attn(r, em, op2, last):
            for s in range(NRQ):
                nc.tensor.matmul(
                    op2[:, S_LO[s] * W:S_LO[s] * W + S_W[s]],
                    lhsT=packB_t[:, (B_QRT1, B_QRT2)[r] + s * (C + 1):
                                 (B_QRT1, B_QRT2)[r] + (s + 1) * (C + 1)],
                    rhs=em[:, S_OFF[s]:S_OFF[s] + S_W[s]],
                    start=False, stop=(last and s == NRQ - 1),
                    skip_group_check=True)

        op2 = ps_out.tile([C + 1, RY * W], F32, tag="op2")
        nc.vector.memset(op2[:], 0.0)

        g2_dbg, g1b_dbg, offs_dbg = [], [], []
        emit_group(0, 0, PPG)
        em0 = emit_ref(0)
        emit_attn(0, em0, op2, last=False)
        emit_group(1, 2, PPG)
        em1 = emit_ref(1)
        emit_attn(1, em1, op2, last=True)
        emit_group(2, 4, PPG)
        emit_group(3, 6, PPG)

        # ---------- F. combine ----------
        ops = sbg.tile([C + 1, RY * W], F32, tag="ops")
        nc.scalar.copy(ops[:], op2[:])
        ofst = sbg.tile([W, RY * C], F32, tag="ofst")
        for yr in range(RY):
            opT = ps_tr.tile([W, C + 1], F32, tag="opT")
            nc.tensor.transpose(opT[:], ops[:, yr * W:(yr + 1) * W], ident_t)
            g = yr // 2
            p_lo = 64 * (yr % 2)
            psl = slice(p_lo, p_lo + W)
            den = sbq.tile([W, 2], F32, tag="den")
            nc.vector.tensor_tensor(den[:, 0:1], opT[:, C:C + 1],
                                    z0_g[g][psl, N169:N169 + 1], op=ALU.add)
            nc.vector.reciprocal(den[:, 1:2], den[:, 0:1])
            of = ofst[:, yr * C:(yr + 1) * C]
            nc.vector.tensor_tensor(of, opT[:, 0:C], out0_g[g][psl, :],
                                    op=ALU.add)
            nc.vector.tensor_scalar(of, of, den[:, 1:2], None,
                                    op0=ALU.mult)
        dst = bass.AP(outs["out"].tensor, outs["out"].offset,
                      [[C, W], [W * C, RY], [1, C]])
        nc.sync.dma_start(dst, ofst[:].rearrange("p (y c) -> p y c", y=RY))

        if "dbg_ops" in outs:
            nc.sync.dma_start(outs["dbg_ops"], ops[:])
            nc.sync.dma_start(outs["dbg_em0"], em0[:])
            nc.sync.dma_start(outs["dbg_o0"], out0_g[0][:])
            nc.sync.dma_start(outs["dbg_p0"], z0_g[0][:])
            nc.sync.dma_start(outs["dbg_g2"], g2_dbg[0][:])
            nc.sync.dma_start(outs["dbg_g1b"], g1b_dbg[0][:])
            nc.sync.dma_start(outs["dbg_offs"], offs_dbg[0][:])


DEBUG_SPECS = dict(
    dbg_ops=([C + 1, RY * W], F32), dbg_em0=([WB, EM_N], BF16),
    dbg_o0=([PPG, C], F32), dbg_p0=([PPG, N169 + 1], F32),
    dbg_g2=([PPG, 13 * WCV + 14], BF16), dbg_g1b=([PPG, P13 * CC_RUN], BF16),
    dbg_offs=([PPG, 2], F32),
)


def build_program(ncores=NCORES, debug=False):
    import concourse.bacc as bacc
    nc = bacc.Bacc("TRN2", target_bir_lowering=False, debug=False,
                   enable_asserts=True, num_devices=ncores)
    ins = {}
    for name, (shape, dt_) in INPUT_SPECS.items():
        ins[name] = nc.dram_tensor(name, shape, dt_, kind="ExternalInput").ap()
    outs = {"out": nc.dram_tensor("out", OUT_SPEC[0], OUT_SPEC[1],
                                  kind="ExternalOutput").ap()}
    if debug:
        for name, (shape, dt_) in DEBUG_SPECS.items():
            outs[name] = nc.dram_tensor(name, shape, dt_,
                                        kind="ExternalOutput").ap()
    with tile.TileContext(nc) as tc:
        build_kernel(tc, outs, ins)
    nc.compile()
    return nc


# ======================= runner =======================
import os as _os

_LAST_RESULT = {}


def kernel(**inputs):
    from concourse.bass_utils import run_bass_kernel_spmd
    from concourse.bass_interp import get_hw_module

    in_maps = host_prep(**inputs)
    nc = build_program()
    nc.m = get_hw_module(nc.m)
    trace = _os.environ.get("KERNEL_TRACE", "0") == "1"
    res = run_bass_kernel_spmd(
        nc, in_maps, core_ids=list(range(NCORES)), trace=trace)
    _LAST_RESULT["res"] = res
    slabs = [np.asarray(res.results[i]["out"], np.float32).reshape(RY, W, C)
             for i in range(NCORES)]
    full = np.concatenate(slabs, 0)          # [56, 56, 32]
    return np.ascontiguousarray(full.transpose(2, 0, 1)[None])


# revision 38
# speedup vs baseline: 26.4771x; 1.0288x over previous
"""Bass/Tile kernel for nn_Colorizer (sparse deformable attention colorizer).

Sharding: spatial row-sharding across 8 cores; core i owns output rows
[7i, 7i+7). The joint softmax over nref*N is additive across refs so each
core normalizes locally.

Per-core pipeline:
  A. CV volume (search ref): 4-row-batched PE matmuls -> PSUM -> bf16 SBUF
     (copies spread over DVE/Act/Pool) -> cv_dram [p, 51, 105].
  B. Phase-1: static strided DMAs pull the stride-3 13x37 runs straight
     from cv_dram -> exp -> expected offset -> floor/frac -> gather idxs.
  C. Phase-2: 14-run CV window gather (196 el) + qr0 QI gather (u-inner
     interleave -> [v,c,u] runs).
  D. Ref0: bilinear blend -> exp -> B-blur (v-major) -> packed bf16
     multiply + fold -> reduce -> o0, Z0.
  E. Refs 1/2 (s-major): per f2 row s one banded cc matmul [68, w_s*56]
     -> exp into packed em -> one mask multiply per ref -> per-s
     attention matmul accumulated into op2 [33, 392] (d-major).
  F. Combine: PE-transpose op2 per row -> (+o0)/(Z) -> staged -> DRAM.
"""
from contextlib import ExitStack

import numpy as np
import ml_dtypes

import concourse.bass as bass

NPBF16 = ml_dtypes.bfloat16
import concourse.mybir as mybir
import concourse.tile as tile

F32 = mybir.dt.float32
I32 = mybir.dt.int32
BF16 = mybir.dt.bfloat16

# ---------------- geometry ----------------
D_SUB, R, C = 4, 6, 32
P13 = 2 * R + 1          # 13
N169 = P13 * P13
DIL_INT = 15
H = W = 56
CF = 64
NCORES = 8
RY = H // NCORES         # 7

DIL = 3
MAXOFF = R * DIL         # 18
DYLO = MAXOFF + R        # 24
NDY = 2 * DYLO + 2       # 50 rows needed for one y-row
WCV = W + NDY - 1        # 105
NROWS_G = NDY + 1        # 51 rows per pair group
HP = H + NDY - 1         # 105
WB = W + 2 * R           # 68
H_SLAB = H + 1           # 57: uniform 51-row pitch for all 4 groups
NRQ = RY + 2 * R         # 19
CC_RUN = 3 * (P13 - 1) + 1   # 37

FLOOR_BIAS = 1024.0
IDX_BIAS = int(FLOOR_BIAS) * WCV + int(FLOOR_BIAS)

GROUPS = [(0, 0, 128), (1, 2, 128), (2, 4, 128), (3, 6, 128)]
PPG = 128  # partitions per group: rows at offsets 0 and 64

# s-major banded attention: for f2 slab row s, valid yr in [LO[s], HI[s]]
S_LO = [max(0, s - (P13 - 1)) for s in range(NRQ)]
S_HI = [min(RY - 1, s) for s in range(NRQ)]
S_W = [(S_HI[s] - S_LO[s] + 1) * W for s in range(NRQ)]
S_OFF = np.concatenate([[0], np.cumsum(S_W)]).tolist()
EM_N = S_OFF[-1]          # 91*56 = 5096

CVP = NROWS_G * WCV       # 5355 per-pixel cv pitch
QT_N = 14 * 448           # qt stream: v(14) x c(32) x u(14)


def _pad2(a, top, left, hh, ww):
    out = np.zeros(a.shape[:-2] + (hh, ww), a.dtype)
    out[..., top:top + a.shape[-2], left:left + a.shape[-1]] = a
    return out


def host_prep(feats_r, feats_t, quantized_r, ref_index, current_ind):
    feats_r = np.asarray(feats_r, np.float32)
    feats_t = np.asarray(feats_t, np.float32)
    quantized_r = np.asarray(quantized_r, np.float32)
    ri = np.asarray(ref_index).tolist()
    ci = int(current_ind)
    diffs = [ci - int(x) for x in ri]
    nsearch = sum(1 for d in diffs if d > DIL_INT)
    dirates = [min(4, d // DIL_INT + 1) for d in diffs if d > DIL_INT]
    nref = feats_r.shape[0]
    assert nsearch == 1 and dirates[0] == DIL and nref == 3, \
        (nsearch, dirates, nref)

    f1 = feats_t[0]
    f2 = [feats_r[s, 0] for s in range(nref)]
    qr = [quantized_r[s, 0][:, ::D_SUB, ::D_SUB] for s in range(nref)]

    # u-inner interleaved qr0: QI[y, x, c, u] = qr0pad[y+u, x, c]
    qr0can = np.zeros((HP + 14, HP, C), np.float32)
    qr0can[DYLO:DYLO + H, DYLO:DYLO + W, :] = qr[0].transpose(1, 2, 0)
    qi = np.stack([qr0can[u:u + HP] for u in range(14)], axis=3)  # [HP,HP,C,14]
    qi = qi.reshape(1, HP * HP * C * 14)
    qi_b16 = np.ascontiguousarray(qi.astype(NPBF16))

    # f2_0 canvas: rows [-24 .. H+26], cols [-24 .. 80]
    f2p0 = _pad2(f2[0], DYLO, DYLO, H + 2 * DYLO + 3, WCV)
    f2p12 = [_pad2(f2[r], R, R, H + 2 * R, WB) for r in (1, 2)]
    qrpT = []
    for r in (1, 2):
        q = np.zeros((H + 2 * R, WB, C + 1), np.float32)
        q[R:R + H, R:R + W, :C] = qr[r].transpose(1, 2, 0)
        q[:, :, C] = 1.0
        qrpT.append(np.ascontiguousarray(q.transpose(1, 0, 2)))

    ploc128 = np.arange(PPG)
    yloc = (ploc128 >= 64).astype(np.int64)
    xs = np.minimum(ploc128 - 64 * yloc, W - 1)
    # phase-2 CV stream const: row 18+yloc, col x+18; + oi_y*105 + oi_x
    c2cv = (((ploc128 * NROWS_G + MAXOFF + yloc) * WCV + xs + MAXOFF)
            - IDX_BIAS)[:, None]
    # phase-2 QI stream const (element units): ((y+18+yloc)*105 + x+18)*448
    c2qr = ((((yloc + MAXOFF) * WCV + xs + MAXOFF) - IDX_BIAS) * 448)[:, None]

    gridy = np.tile((np.repeat(np.arange(P13) - R, P13) * DIL)[None, :],
                    (PPG, 1)).astype(np.float32)
    gridx = np.tile((np.tile(np.arange(P13) - R, P13) * DIL)[None, :],
                    (PPG, 1)).astype(np.float32)
    uconst = np.tile((np.arange(14) * WCV)[None, :], (PPG, 1)).astype(np.float32)

    xq = np.arange(WB)[:, None]
    xx = np.arange(W)[None, :]
    maskT = ((xq - xx >= 0) & (xq - xx <= 2 * R)).astype(np.float32)  # [68,56]
    maskM = np.tile(maskT[:, None, :], (1, EM_N // W, 1)).reshape(WB, EM_N)
    ident33 = np.zeros((PPG, C + 1), np.float32)
    ident33[:C + 1, :] = np.eye(C + 1)

    def b16(a):
        return np.ascontiguousarray(a.astype(NPBF16))

    in_maps = []
    for core in range(NCORES):
        y0 = core * RY
        f1pair = np.zeros((CF, 4 * PPG), np.float32)
        for g in range(4):
            f1pair[:, g * PPG:g * PPG + W] = f1[:, y0 + 2 * g, :]
            if 2 * g + 1 < RY:
                f1pair[:, g * PPG + 64:g * PPG + 64 + W] = f1[:, y0 + 2 * g + 1, :]
        # 64-partition pack: f1 | f1pair | f2p0 | f2p1 | f2p2
        packA = np.concatenate([
            f1[:, y0:y0 + RY, :].reshape(CF, RY * W),
            f1pair,
            f2p0[:, y0:y0 + H_SLAB, :].reshape(CF, H_SLAB * WCV),
            f2p12[0][:, y0:y0 + NRQ, :].reshape(CF, NRQ * WB),
            f2p12[1][:, y0:y0 + NRQ, :].reshape(CF, NRQ * WB),
        ], axis=1)
        # 68-partition pack: qrT1 | qrT2 | maskM
        packB = np.concatenate([
            qrpT[0][:, y0:y0 + NRQ, :].reshape(WB, NRQ * (C + 1)),
            qrpT[1][:, y0:y0 + NRQ, :].reshape(WB, NRQ * (C + 1)),
            maskM,
        ], axis=1)
        # 128-partition f32 pack: gridx | gridy | uconst | c2cv | c2qr | id33
        packC = np.concatenate([
            gridx, gridy, uconst, c2cv, c2qr + y0 * WCV * 448, ident33,
        ], axis=1).astype(np.float32)
        m = dict(
            packA=b16(packA),
            packB=b16(packB),
            packC=packC,
            qr0pad=qi_b16,
        )
        in_maps.append(m)
    return in_maps


# pack offsets (elements)
A_F1 = 0
A_F1PAIR = A_F1 + RY * W
A_F2P0 = A_F1PAIR + 4 * PPG
A_F2P1 = A_F2P0 + H_SLAB * WCV
A_F2P2 = A_F2P1 + NRQ * WB
A_N = A_F2P2 + NRQ * WB
B_QRT1 = 0
B_QRT2 = B_QRT1 + NRQ * (C + 1)
B_MASK = B_QRT2 + NRQ * (C + 1)
B_N = B_MASK + EM_N
C_GX = 0
C_GY = C_GX + N169
C_UC = C_GY + N169
C_CV = C_UC + 14
C_QR = C_CV + 1
C_ID = C_QR + 1
C_N = C_ID + (C + 1)

INPUT_SPECS = dict(
    packA=([CF, A_N], BF16),
    packB=([WB, B_N], BF16),
    packC=([PPG, C_N], F32),
    qr0pad=([1, HP * HP * C * 14], BF16),
)
OUT_SPEC = ([RY * W, C], F32)


def build_kernel(tc, outs, ins):
    nc = tc.nc
    Exp = mybir.ActivationFunctionType.Exp
    Copy = mybir.ActivationFunctionType.Copy
    ALU = mybir.AluOpType
    AX = mybir.AxisListType

    with ExitStack() as ctx:
        sb = ctx.enter_context(tc.tile_pool(name="sb", bufs=1))
        sbg = ctx.enter_context(tc.tile_pool(name="sbg", bufs=2))
        sbe = ctx.enter_context(tc.tile_pool(name="sbe", bufs=2))
        sbq = ctx.enter_context(tc.tile_pool(name="sbq", bufs=2))
        ps_cv = ctx.enter_context(tc.tile_pool(name="ps_cv", bufs=3, space="PSUM"))
        ps_cc = ctx.enter_context(tc.tile_pool(name="ps_cc", bufs=2, space="PSUM"))
        ps_out = ctx.enter_context(tc.tile_pool(name="ps_out", bufs=1, space="PSUM"))
        ps_tr = ctx.enter_context(tc.tile_pool(name="ps_tr", bufs=2, space="PSUM"))
        dram = ctx.enter_context(tc.tile_pool(name="dram", bufs=1, space="DRAM"))

        def load(name):
            shape, dt_ = INPUT_SPECS[name]
            t = sb.tile(shape, dt_, tag=name)
            nc.sync.dma_start(t[:], ins[name])
            return t

        packA_t = load("packA")
        packB_t = load("packB")
        packC_t = load("packC")

        def pa(off, n):
            return packA_t[:, off:off + n]

        f1_t = pa(A_F1, RY * W)
        maskM_t = packB_t[:, B_MASK:B_MASK + EM_N]
        ident_t = packC_t[0:C + 1, C_ID:C_ID + (C + 1)]
        gridx_t = packC_t[:, C_GX:C_GX + N169]
        gridy_t = packC_t[:, C_GY:C_GY + N169]
        uconst_t = packC_t[:, C_UC:C_UC + 14]
        c2cv_t = packC_t[:, C_CV:C_CV + 1]
        c2qr_t = packC_t[:, C_QR:C_QR + 1]

        out0_g, z0_g = {}, {}

        # ---------- stage 1: CV -> phase-1 -> gather launches ----------
        def stage1(g, yg, MP=PPG):
            nrow = NROWS_G
            cv_sb = sbg.tile([MP, nrow * WCV], BF16, tag="cv_sb")
            lhs = pa(A_F1PAIR + g * PPG, PPG)
            ci = 0
            # 4 slab rows per matmul (420 cols in one PSUM bank)
            for r0 in range(0, nrow, 4):
                rn = min(4, nrow - r0)
                ncol = rn * WCV
                pt = ps_cv.tile([MP, 512], F32, tag="cvch")
                nc.tensor.matmul(
                    pt[:, 0:ncol], lhsT=lhs,
                    rhs=pa(A_F2P0 + (2 * g + r0) * WCV, ncol),
                    start=True, stop=True)
                dst = cv_sb[:, r0 * WCV:(r0 + rn) * WCV]
                if ci % 2 == 0:
                    nc.vector.tensor_copy(dst, pt[:, 0:ncol])
                else:
                    nc.scalar.copy(dst, pt[:, 0:ncol])
                ci += 1
            cv_dram = dram.tile([1, MP * CVP + 8 * WCV], BF16, tag=f"cvd{g}")
            nc.sync.dma_start(
                cv_dram[:, 0:MP * CVP].rearrange("o (p f) -> p (o f)", p=MP),
                cv_sb[:])

            # ---------- B. phase-1: static strided reads of cv ----------
            g1b = sbq.tile([MP, P13 * CC_RUN], BF16, tag="g1b")
            for pl, ybase in ((slice(0, 64), R), (slice(64, 128), R + 1)):
                p0_ = pl.start
                base = p0_ * CVP + ybase * WCV + R
                src = bass.AP(cv_dram[:].tensor, cv_dram[:].offset + base,
                              [[CVP + 1, 64], [3 * WCV, P13], [1, CC_RUN]])
                dst = g1b[pl].rearrange("p (i r) -> p i r", i=P13)
                nc.sync.dma_start(dst, src)
            cc0 = bass.AP(g1b[:].tensor, g1b[:].offset,
                          [g1b[:].ap[0], [CC_RUN, P13], [3, P13]])
            e1 = sbq.tile([MP, N169 + 1], F32, tag="e1")
            nc.scalar.activation(
                e1[:, 0:N169].rearrange("p (i j) -> p i j", i=P13), cc0, Exp,
                accum_out=e1[:, N169:N169 + 1])
            sc = sbq.tile([MP, 4], F32, tag="sc")
            tmp = sbq.tile([MP, N169], F32, tag="tmp169")
            nc.vector.scalar_tensor_tensor(
                out=tmp[:], in0=e1[:, 0:N169], scalar=0.0, in1=gridx_t,
                op0=ALU.add, op1=ALU.mult, accum_out=sc[:, 0:1])
            nc.vector.scalar_tensor_tensor(
                out=tmp[:], in0=e1[:, 0:N169], scalar=0.0, in1=gridy_t,
                op0=ALU.add, op1=ALU.mult, accum_out=sc[:, 1:2])
            offs = sbq.tile([MP, 2], F32, tag="offs")   # [off_x, off_y]
            nc.vector.reciprocal(sc[:, 2:3], e1[:, N169:N169 + 1])
            nc.vector.tensor_tensor(offs[:, 0:1], sc[:, 0:1], sc[:, 2:3],
                                    op=ALU.mult)
            nc.vector.tensor_tensor(offs[:, 1:2], sc[:, 1:2], sc[:, 2:3],
                                    op=ALU.mult)
            nc.vector.tensor_scalar(offs[:], offs[:], float(MAXOFF),
                                    -float(MAXOFF), op0=ALU.min, op1=ALU.max)
            # floor (mode-agnostic) on gpsimd: fb = off+1024; fbi=cast;
            # fbf=cast back; fbf -= (fb - fbf < 0); wfrac = fb - fbf
            fb = sbq.tile([MP, 2], F32, tag="fb")
            nc.gpsimd.tensor_scalar(fb[:], offs[:], FLOOR_BIAS, None,
                                    op0=ALU.add)
            fbi = sbq.tile([MP, 2], I32, tag="fbi")
            nc.gpsimd.tensor_copy(fbi[:], fb[:])
            fbf = sbq.tile([MP, 2], F32, tag="fbf")
            nc.gpsimd.tensor_copy(fbf[:], fbi[:])
            err = sbq.tile([MP, 2], F32, tag="err")
            nc.gpsimd.tensor_tensor(err[:], fb[:], fbf[:], op=ALU.subtract)
            neg = sbq.tile([MP, 2], F32, tag="neg")
            nc.gpsimd.tensor_scalar(neg[:], err[:], 0.0, None, op0=ALU.is_lt)
            nc.gpsimd.tensor_tensor(fbf[:], fbf[:], neg[:], op=ALU.subtract)
            wfrac = sbq.tile([MP, 2], F32, tag="wfrac")  # [wx, wy]
            nc.gpsimd.tensor_tensor(wfrac[:], fb[:], fbf[:], op=ALU.subtract)
            s2 = sbq.tile([MP, 1], F32, tag="s2")
            nc.vector.scalar_tensor_tensor(
                out=s2[:], in0=fbf[:, 1:2], scalar=float(WCV),
                in1=fbf[:, 0:1], op0=ALU.mult, op1=ALU.add)
            i1f = sbq.tile([MP, 1], F32, tag="i1f")
            nc.vector.tensor_scalar(i1f[:], c2cv_t, s2[:], None,
                                    op0=ALU.add)
            idx2cv = sbq.tile([MP, 1], I32, tag="idx2cv")
            nc.gpsimd.tensor_copy(idx2cv[:], i1f[:])
            # QI element index: c2qr + (s2 + yg*WCV)*448
            idx2qrf = sbq.tile([MP, 1], F32, tag="idx2qrf")
            nc.gpsimd.tensor_scalar(idx2qrf[:], s2[:], 448.0,
                                    float(yg * WCV * 448),
                                    op0=ALU.mult, op1=ALU.add)
            nc.gpsimd.tensor_tensor(idx2qrf[:], idx2qrf[:], c2qr_t,
                                    op=ALU.add)
            idx2qr = sbq.tile([MP, 1], I32, tag="idx2qr")
            nc.gpsimd.tensor_copy(idx2qr[:], idx2qrf[:])

            # ---------- C. phase-2 gathers ----------
            NS2 = 13 * WCV + 14
            g2 = sbq.tile([MP, NS2], BF16, tag="g2")
            nc.gpsimd.indirect_dma_start(
                out=g2[:], out_offset=None, in_=cv_dram[:],
                in_offset=bass.IndirectOffsetOnAxis(ap=idx2cv[:], axis=1))
            if g == 0:
                g2_dbg.append(g2)
                g1b_dbg.append(g1b)
                offs_dbg.append(offs)
            qt = sbq.tile([MP, QT_N], BF16, tag="qt")
            nc.gpsimd.indirect_dma_start(
                out=qt[:], out_offset=None, in_=ins["qr0pad"],
                in_offset=bass.IndirectOffsetOnAxis(ap=idx2qr[:], axis=1))

            # bilinear corner weights (Pool: tiny ops off the DVE queue)
            ww = sbq.tile([MP, 4], F32, tag="ww")
            om = sbq.tile([MP, 2], F32, tag="om")
            nc.gpsimd.tensor_scalar(om[:], wfrac[:], -1.0, 1.0,
                                    op0=ALU.mult, op1=ALU.add)
            nc.gpsimd.tensor_tensor(ww[:, 0:1], om[:, 1:2], om[:, 0:1],
                                    op=ALU.mult)
            nc.gpsimd.tensor_tensor(ww[:, 1:2], om[:, 1:2], wfrac[:, 0:1],
                                    op=ALU.mult)
            nc.gpsimd.tensor_tensor(ww[:, 2:3], wfrac[:, 1:2], om[:, 0:1],
                                    op=ALU.mult)
            nc.gpsimd.tensor_tensor(ww[:, 3:4], wfrac[:, 1:2], wfrac[:, 0:1],
                                    op=ALU.mult)
            return dict(g=g, MP=MP, g2=g2, qt=qt, ww=ww)

        # ---------- stage 2: bilinear blend -> p0 -> blur -> o0 ----------
        def stage2(st):
            g, MP, g2, qt, ww = st["g"], st["MP"], st["g2"], st["qt"], st["ww"]
            g2v = bass.AP(g2[:].tensor, g2[:].offset,
                          [g2[:].ap[0], [WCV, 14], [1, 14]])
            corr = sbq.tile([MP, N169], F32, tag="corr")
            crv = corr[:].rearrange("p (i j) -> p i j", i=P13)
            nc.vector.tensor_scalar(crv, g2v[:, 0:13, 0:13], ww[:, 0:1], None,
                                    op0=ALU.mult)
            for (sl_u, sl_v, wcol) in (((0, 13), (1, 14), 1),
                                       ((1, 14), (0, 13), 2),
                                       ((1, 14), (1, 14), 3)):
                nc.vector.scalar_tensor_tensor(
                    out=crv, in0=g2v[:, sl_u[0]:sl_u[1], sl_v[0]:sl_v[1]],
                    scalar=ww[:, wcol:wcol + 1], in1=crv,
                    op0=ALU.mult, op1=ALU.add)
            p0 = sb.tile([MP, N169 + 1], F32, tag=f"p0_{g}")
            nc.scalar.activation(p0[:, 0:N169], corr[:], Exp,
                                 accum_out=p0[:, N169:N169 + 1])
            z0_g[g] = p0
            # bilinear blur, v-major: bb[p, v, u]
            bb = sbq.tile([MP, 196], BF16, tag="bb")
            nc.gpsimd.memset(bb[:], 0.0)
            bbv = bb[:].rearrange("p (v u) -> p v u", v=14)
            # p0 viewed as (j=v, i=u): AP dims j outer (stride 1), i inner
            p0ji = bass.AP(p0[:].tensor, p0[:].offset,
                           [p0[:].ap[0], [1, P13], [P13, P13]])
            nc.vector.tensor_scalar(bbv[:, 0:13, 0:13], p0ji, ww[:, 0:1],
                                    None, op0=ALU.mult)
            for (sl_v, sl_u, wcol) in (((1, 14), (0, 13), 1),
                                       ((0, 13), (1, 14), 2),
                                       ((1, 14), (1, 14), 3)):
                dstv = bbv[:, sl_v[0]:sl_v[1], sl_u[0]:sl_u[1]]
                nc.vector.scalar_tensor_tensor(
                    out=dstv, in0=p0ji, scalar=ww[:, wcol:wcol + 1], in1=dstv,
                    op0=ALU.mult, op1=ALU.add)
            # packed bf16 multiply qt *= bb (broadcast over c via 0-stride)
            qtv = bass.AP(qt[:].tensor, qt[:].offset,
                          [qt[:].ap[0], [448, 14], [14, C], [1, 14]])
            bbb = bass.AP(bb[:].tensor, bb[:].offset,
                          [bb[:].ap[0], [14, 14], [0, C], [1, 14]])
            nc.vector.tensor_tensor(qtv, qtv, bbb, op=ALU.mult)
            # fold v twice: 14 -> 7 -> 4 rows, then reduce keep c
            nc.vector.tensor_tensor(qt[:, 0:7 * 448], qt[:, 0:7 * 448],
                                    qt[:, 7 * 448:14 * 448], op=ALU.add)
            nc.vector.tensor_tensor(qt[:, 0:3 * 448], qt[:, 0:3 * 448],
                                    qt[:, 4 * 448:7 * 448], op=ALU.add)
            o0 = sb.tile([MP, C], F32, tag=f"o0_{g}")
            pr = bass.AP(qt[:].tensor, qt[:].offset,
                         [qt[:].ap[0], [14, C], [448, 4], [1, 14]])
            nc.vector.tensor_reduce(o0[:], pr, axis=AX.XY, op=ALU.add)
            out0_g[g] = o0

        # ---------- E. refs 1/2, s-major ----------
        def emit_ref(r):
            em = sbe.tile([WB, EM_N], BF16, tag="em")
            for s in range(NRQ):
                w = S_W[s]
                ct = ps_cc.tile([WB, 512], F32, tag="ct")
                nc.tensor.matmul(
                    ct[:, 0:w],
                    lhsT=pa((A_F2P1, A_F2P2)[r] + s * WB, WB),
                    rhs=f1_t[:, S_LO[s] * W:S_LO[s] * W + w],
                    start=True, stop=True)
                nc.scalar.activation(em[:, S_OFF[s]:S_OFF[s] + w],
                                     ct[:, 0:w], Exp)
            return em

        def emit_mask(em):
            nc.vector.tensor_tensor(em[:], em[:], maskM_t, op=ALU.mult)

        def emit_attn(r, em, op2, last):
            for s in range(NRQ):
                nc.tensor.matmul(
                    op2[:, S_LO[s] * W:S_LO[s] * W + S_W[s]],
                    lhsT=packB_t[:, (B_QRT1, B_QRT2)[r] + s * (C + 1):
                                 (B_QRT1, B_QRT2)[r] + (s + 1) * (C + 1)],
                    rhs=em[:, S_OFF[s]:S_OFF[s] + S_W[s]],
                    start=False, stop=(last and s == NRQ - 1),
                    skip_group_check=True)

        # ---------- F. combine ----------
        ofst = sbg.tile([W, RY * C], F32, tag="ofst")

        def combine_rows(ops, yrs):
            for yr in yrs:
                opT = ps_tr.tile([W, C + 1], F32, tag="opT")
                nc.tensor.transpose(opT[:], ops[:, yr * W:(yr + 1) * W],
                                    ident_t)
                g = yr // 2
                p_lo = 64 * (yr % 2)
                psl = slice(p_lo, p_lo + W)
                den = sbq.tile([W, 2], F32, tag="den")
                nc.vector.tensor_tensor(den[:, 0:1], opT[:, C:C + 1],
                                        z0_g[g][psl, N169:N169 + 1],
                                        op=ALU.add)
                nc.vector.reciprocal(den[:, 1:2], den[:, 0:1])
                of = ofst[:, yr * C:(yr + 1) * C]
                nc.vector.tensor_tensor(of, opT[:, 0:C], out0_g[g][psl, :],
                                        op=ALU.add)
                nc.vector.tensor_scalar(of, of, den[:, 1:2], None,
                                        op0=ALU.mult)

        # ---------- schedule ----------
        op2 = ps_out.tile([C + 1, RY * W], F32, tag="op2")
        nc.vector.memset(op2[:], 0.0)

        g2_dbg, g1b_dbg, offs_dbg = [], [], []
        st0 = stage1(0, 0)
        em0 = emit_ref(0)
        st1 = stage1(1, 2)
        emit_mask(em0)
        stage2(st0)
        emit_attn(0, em0, op2, last=False)
        st2 = stage1(2, 4)
        em1 = emit_ref(1)
        emit_mask(em1)
        stage2(st1)
        st3 = stage1(3, 6)
        emit_attn(1, em1, op2, last=True)
        ops = sbg.tile([C + 1, RY * W], F32, tag="ops")
        nc.scalar.copy(ops[:], op2[:])
        combine_rows(ops, (0, 1, 2, 3))
        stage2(st2)
        combine_rows(ops, (4, 5))
        stage2(st3)
        combine_rows(ops, (6,))
        dst = bass.AP(outs["out"].tensor, outs["out"].offset,
                      [[C, W], [W * C, RY], [1, C]])
        nc.sync.dma_start(dst, ofst[:].rearrange("p (y c) -> p y c", y=RY))

        if "dbg_ops" in outs:
            nc.sync.dma_start(outs["dbg_ops"], ops[:])
            nc.sync.dma_start(outs["dbg_em0"], em0[:])
            nc.sync.dma_start(outs["dbg_o0"], out0_g[0][:])
            nc.sync.dma_start(outs["dbg_p0"], z0_g[0][:])
            nc.sync.dma_start(outs["dbg_g2"], g2_dbg[0][:])
            nc.sync.dma_start(outs["dbg_g1b"], g1b_dbg[0][:])
            nc.sync.dma_start(outs["dbg_offs"], offs_dbg[0][:])


DEBUG_SPECS = dict(
    dbg_ops=([C + 1, RY * W], F32), dbg_em0=([WB, EM_N], BF16),
    dbg_o0=([PPG, C], F32), dbg_p0=([PPG, N169 + 1], F32),
    dbg_g2=([PPG, 13 * WCV + 14], BF16), dbg_g1b=([PPG, P13 * CC_RUN], BF16),
    dbg_offs=([PPG, 2], F32),
)


def build_program(ncores=NCORES, debug=False):
    import concourse.bacc as bacc
    nc = bacc.Bacc("TRN2", target_bir_lowering=False, debug=False,
                   enable_asserts=True, num_devices=ncores)
    ins = {}
    for name, (shape, dt_) in INPUT_SPECS.items():
        ins[name] = nc.dram_tensor(name, shape, dt_, kind="ExternalInput").ap()
    outs = {"out": nc.dram_tensor("out", OUT_SPEC[0], OUT_SPEC[1],
                                  kind="ExternalOutput").ap()}
    if debug:
        for name, (shape, dt_) in DEBUG_SPECS.items():
            outs[name] = nc.dram_tensor(name, shape, dt_,
                                        kind="ExternalOutput").ap()
    with tile.TileContext(nc) as tc:
        build_kernel(tc, outs, ins)
    nc.compile()
    return nc


# ======================= runner =======================
import os as _os

_LAST_RESULT = {}


def kernel(**inputs):
    from concourse.bass_utils import run_bass_kernel_spmd
    from concourse.bass_interp import get_hw_module

    in_maps = host_prep(**inputs)
    nc = build_program()
    nc.m = get_hw_module(nc.m)
    trace = _os.environ.get("KERNEL_TRACE", "0") == "1"
    res = run_bass_kernel_spmd(
        nc, in_maps, core_ids=list(range(NCORES)), trace=trace)
    _LAST_RESULT["res"] = res
    slabs = [np.asarray(res.results[i]["out"], np.float32).reshape(RY, W, C)
             for i in range(NCORES)]
    full = np.concatenate(slabs, 0)          # [56, 56, 32]
    return np.ascontiguousarray(full.transpose(2, 0, 1)[None])


# revision 50
# speedup vs baseline: 29.9820x; 1.1324x over previous
"""Bass/Tile kernel for nn_Colorizer (sparse deformable attention colorizer).

Sharding: spatial row-sharding across 8 cores; core i owns output rows
[7i, 7i+7). The joint softmax over nref*N is additive across refs so each
core normalizes locally.

Per-core pipeline:
  A. CV volume (search ref): 4-row-batched PE matmuls -> PSUM -> bf16 SBUF
     (copies spread over DVE/Act/Pool) -> cv_dram [p, 51, 105].
  B. Phase-1: static strided DMAs pull the stride-3 13x37 runs straight
     from cv_dram -> exp -> expected offset -> floor/frac -> gather idxs.
  C. Phase-2: 14-run CV window gather (196 el) + qr0 QI gather (u-inner
     interleave -> [v,c,u] runs).
  D. Ref0: bilinear blend -> exp -> B-blur (v-major) -> packed bf16
     multiply + fold -> reduce -> o0, Z0.
  E. Refs 1/2 (s-major): per f2 row s one banded cc matmul [68, w_s*56]
     -> exp into packed em -> one mask multiply per ref -> per-s
     attention matmul accumulated into op2 [33, 392] (d-major).
  F. Combine: PE-transpose op2 per row -> (+o0)/(Z) -> staged -> DRAM.
"""
from contextlib import ExitStack

import numpy as np
import ml_dtypes

import concourse.bass as bass

NPBF16 = ml_dtypes.bfloat16
import concourse.mybir as mybir
import concourse.tile as tile

F32 = mybir.dt.float32
I32 = mybir.dt.int32
BF16 = mybir.dt.bfloat16

# ---------------- geometry ----------------
D_SUB, R, C = 4, 6, 32
P13 = 2 * R + 1          # 13
N169 = P13 * P13
DIL_INT = 15
H = W = 56
CF = 64
NCORES = 8
RY = H // NCORES         # 7

DIL = 3
MAXOFF = R * DIL         # 18
DYLO = MAXOFF + R        # 24
NDY = 2 * DYLO + 2       # 50 rows needed for one y-row
WCV = W + NDY - 1        # 105
NROWS_G = NDY + 1        # 51 rows per pair group
HP = H + NDY - 1         # 105
WB = W + 2 * R           # 68
H_SLAB = H + 1           # 57: uniform 51-row pitch for all 4 groups
NRQ = RY + 2 * R         # 19
CC_RUN = 3 * (P13 - 1) + 1   # 37

FLOOR_BIAS = 1024.0
IDX_BIAS = int(FLOOR_BIAS) * WCV + int(FLOOR_BIAS)

GROUPS = [(0, 0, 128), (1, 2, 128), (2, 4, 128), (3, 6, 128)]
PPG = 128  # partitions per group: rows at offsets 0 and 64

# s-major banded attention: for f2 slab row s, valid yr in [LO[s], HI[s]]
S_LO = [max(0, s - (P13 - 1)) for s in range(NRQ)]
S_HI = [min(RY - 1, s) for s in range(NRQ)]
S_W = [(S_HI[s] - S_LO[s] + 1) * W for s in range(NRQ)]
S_OFF = np.concatenate([[0], np.cumsum(S_W)]).tolist()
EM_N = S_OFF[-1]          # 91*56 = 5096

CVP = NROWS_G * WCV       # 5355 per-pixel cv pitch
QT_N = 14 * 448           # qt stream: v(14) x c(32) x u(14)


def _pad2(a, top, left, hh, ww):
    out = np.zeros(a.shape[:-2] + (hh, ww), a.dtype)
    out[..., top:top + a.shape[-2], left:left + a.shape[-1]] = a
    return out


def host_prep(feats_r, feats_t, quantized_r, ref_index, current_ind):
    feats_r = np.asarray(feats_r, np.float32)
    feats_t = np.asarray(feats_t, np.float32)
    quantized_r = np.asarray(quantized_r, np.float32)
    ri = np.asarray(ref_index).tolist()
    ci = int(current_ind)
    diffs = [ci - int(x) for x in ri]
    nsearch = sum(1 for d in diffs if d > DIL_INT)
    dirates = [min(4, d // DIL_INT + 1) for d in diffs if d > DIL_INT]
    nref = feats_r.shape[0]
    assert nsearch == 1 and dirates[0] == DIL and nref == 3, \
        (nsearch, dirates, nref)

    f1 = feats_t[0]
    f2 = [feats_r[s, 0] for s in range(nref)]
    qr = [quantized_r[s, 0][:, ::D_SUB, ::D_SUB] for s in range(nref)]

    # u-inner interleaved qr0: QI[y, x, c, u] = qr0pad[y+u, x, c]
    qr0can = np.zeros((HP + 14, HP, C), np.float32)
    qr0can[DYLO:DYLO + H, DYLO:DYLO + W, :] = qr[0].transpose(1, 2, 0)
    qi = np.stack([qr0can[u:u + HP] for u in range(14)], axis=3)  # [HP,HP,C,14]
    qi = qi.reshape(1, HP * HP * C * 14)
    qi_b16 = np.ascontiguousarray(qi.astype(NPBF16))

    # f2_0 canvas: rows [-24 .. H+26], cols [-24 .. 80]
    f2p0 = _pad2(f2[0], DYLO, DYLO, H + 2 * DYLO + 3, WCV)
    f2p12 = [_pad2(f2[r], R, R, H + 2 * R, WB) for r in (1, 2)]
    qrpT = []
    for r in (1, 2):
        q = np.zeros((H + 2 * R, WB, C + 1), np.float32)
        q[R:R + H, R:R + W, :C] = qr[r].transpose(1, 2, 0)
        q[:, :, C] = 1.0
        qrpT.append(np.ascontiguousarray(q.transpose(1, 0, 2)))

    ploc128 = np.arange(PPG)
    yloc = (ploc128 >= 64).astype(np.int64)
    xs = np.minimum(ploc128 - 64 * yloc, W - 1)
    # phase-2 CV stream const: row 18+yloc, col x+18; + oi_y*105 + oi_x
    c2cv = (((ploc128 * NROWS_G + MAXOFF + yloc) * WCV + xs + MAXOFF)
            - IDX_BIAS)[:, None]
    # phase-2 QI stream const (element units): ((y+18+yloc)*105 + x+18)*448
    c2qr = ((((yloc + MAXOFF) * WCV + xs + MAXOFF) - IDX_BIAS) * 448)[:, None]

    gridy = np.tile((np.repeat(np.arange(P13) - R, P13) * DIL)[None, :],
                    (PPG, 1)).astype(np.float32)
    gridx = np.tile((np.tile(np.arange(P13) - R, P13) * DIL)[None, :],
                    (PPG, 1)).astype(np.float32)
    uconst = np.tile((np.arange(14) * WCV)[None, :], (PPG, 1)).astype(np.float32)

    xq = np.arange(WB)[:, None]
    xx = np.arange(W)[None, :]
    maskT = ((xq - xx >= 0) & (xq - xx <= 2 * R)).astype(np.float32)  # [68,56]
    maskM = np.tile(maskT[:, None, :], (1, EM_N // W, 1)).reshape(WB, EM_N)
    ident33 = np.zeros((PPG, C + 1), np.float32)
    ident33[:C + 1, :] = np.eye(C + 1)

    def b16(a):
        return np.ascontiguousarray(a.astype(NPBF16))

    in_maps = []
    for core in range(NCORES):
        y0 = core * RY
        f1pair = np.zeros((CF, 4 * PPG), np.float32)
        for g in range(4):
            f1pair[:, g * PPG:g * PPG + W] = f1[:, y0 + 2 * g, :]
            if 2 * g + 1 < RY:
                f1pair[:, g * PPG + 64:g * PPG + 64 + W] = f1[:, y0 + 2 * g + 1, :]
        # 64-partition pack: f1 | f1pair | f2p0 | f2p1 | f2p2
        packA = np.concatenate([
            f1[:, y0:y0 + RY, :].reshape(CF, RY * W),
            f1pair,
            f2p0[:, y0:y0 + H_SLAB, :].reshape(CF, H_SLAB * WCV),
            f2p12[0][:, y0:y0 + NRQ, :].reshape(CF, NRQ * WB),
            f2p12[1][:, y0:y0 + NRQ, :].reshape(CF, NRQ * WB),
        ], axis=1)
        # 68-partition pack: qrT1 | qrT2 | maskM
        packB = np.concatenate([
            qrpT[0][:, y0:y0 + NRQ, :].reshape(WB, NRQ * (C + 1)),
            qrpT[1][:, y0:y0 + NRQ, :].reshape(WB, NRQ * (C + 1)),
            maskM,
        ], axis=1)
        # 128-partition f32 pack: gridx | gridy | uconst | c2cv | c2qr | id33
        packC = np.concatenate([
            gridx, gridy, uconst, c2cv, c2qr + y0 * WCV * 448, ident33,
        ], axis=1).astype(np.float32)
        m = dict(
            packA=b16(packA),
            packB=b16(packB),
            packC=packC,
            qr0pad=qi_b16,
        )
        in_maps.append(m)
    return in_maps


# pack offsets (elements)
A_F1 = 0
A_F1PAIR = A_F1 + RY * W
A_F2P0 = A_F1PAIR + 4 * PPG
A_F2P1 = A_F2P0 + H_SLAB * WCV
A_F2P2 = A_F2P1 + NRQ * WB
A_N = A_F2P2 + NRQ * WB
B_QRT1 = 0
B_QRT2 = B_QRT1 + NRQ * (C + 1)
B_MASK = B_QRT2 + NRQ * (C + 1)
B_N = B_MASK + EM_N
C_GX = 0
C_GY = C_GX + N169
C_UC = C_GY + N169
C_CV = C_UC + 14
C_QR = C_CV + 1
C_ID = C_QR + 1
C_N = C_ID + (C + 1)

INPUT_SPECS = dict(
    packA=([CF, A_N], BF16),
    packB=([WB, B_N], BF16),
    packC=([PPG, C_N], F32),
    qr0pad=([1, HP * HP * C * 14], BF16),
)
OUT_SPEC = ([RY * W, C], F32)


def build_kernel(tc, outs, ins):
    nc = tc.nc
    Exp = mybir.ActivationFunctionType.Exp
    Copy = mybir.ActivationFunctionType.Copy
    ALU = mybir.AluOpType
    AX = mybir.AxisListType

    with ExitStack() as ctx:
        sb = ctx.enter_context(tc.tile_pool(name="sb", bufs=1))
        sbg = ctx.enter_context(tc.tile_pool(name="sbg", bufs=2))
        sbe = ctx.enter_context(tc.tile_pool(name="sbe", bufs=2))
        sbq = ctx.enter_context(tc.tile_pool(name="sbq", bufs=4))
        ps_cv = ctx.enter_context(tc.tile_pool(name="ps_cv", bufs=3, space="PSUM"))
        ps_cc = ctx.enter_context(tc.tile_pool(name="ps_cc", bufs=2, space="PSUM"))
        ps_out = ctx.enter_context(tc.tile_pool(name="ps_out", bufs=1, space="PSUM"))
        ps_tr = ctx.enter_context(tc.tile_pool(name="ps_tr", bufs=2, space="PSUM"))
        dram = ctx.enter_context(tc.tile_pool(name="dram", bufs=1, space="DRAM"))

        def load(name):
            shape, dt_ = INPUT_SPECS[name]
            t = sb.tile(shape, dt_, tag=name)
            nc.sync.dma_start(t[:], ins[name])
            return t

        packA_t = load("packA")
        packB_t = load("packB")
        packC_t = load("packC")

        def pa(off, n):
            return packA_t[:, off:off + n]

        f1_t = pa(A_F1, RY * W)
        maskM_t = packB_t[:, B_MASK:B_MASK + EM_N]
        ident_t = packC_t[0:C + 1, C_ID:C_ID + (C + 1)]
        gridx_t = packC_t[:, C_GX:C_GX + N169]
        gridy_t = packC_t[:, C_GY:C_GY + N169]
        uconst_t = packC_t[:, C_UC:C_UC + 14]
        c2cv_t = packC_t[:, C_CV:C_CV + 1]
        c2qr_t = packC_t[:, C_QR:C_QR + 1]

        out0_g, z0_g = {}, {}

        # ---------- stage 0: CV volume -> DRAM + phase-1 window reads ----
        def stage0(g, MP=PPG):
            nrow = NROWS_G
            cv_sb = sbg.tile([MP, nrow * WCV], BF16, tag="cv_sb")
            lhs = pa(A_F1PAIR + g * PPG, PPG)
            ci = 0
            # 4 slab rows per matmul (420 cols in one PSUM bank)
            for r0 in range(0, nrow, 4):
                rn = min(4, nrow - r0)
                ncol = rn * WCV
                pt = ps_cv.tile([MP, 512], F32, tag="cvch")
                nc.tensor.matmul(
                    pt[:, 0:ncol], lhsT=lhs,
                    rhs=pa(A_F2P0 + (2 * g + r0) * WCV, ncol),
                    start=True, stop=True)
                dst = cv_sb[:, r0 * WCV:(r0 + rn) * WCV]
                if ci % 2 == 0:
                    nc.vector.tensor_copy(dst, pt[:, 0:ncol])
                else:
                    nc.scalar.copy(dst, pt[:, 0:ncol])
                ci += 1
            cv_dram = dram.tile([1, MP * CVP + 8 * WCV], BF16, tag=f"cvd{g}")
            nc.sync.dma_start(
                cv_dram[:, 0:MP * CVP].rearrange("o (p f) -> p (o f)", p=MP),
                cv_sb[:])

            # ---------- B. phase-1: static strided reads of cv ----------
            g1b = sbq.tile([MP, P13 * CC_RUN], BF16, tag="g1b")
            for pl, ybase in ((slice(0, 64), R), (slice(64, 128), R + 1)):
                p0_ = pl.start
                base = p0_ * CVP + ybase * WCV + R
                src = bass.AP(cv_dram[:].tensor, cv_dram[:].offset + base,
                              [[CVP + 1, 64], [3 * WCV, P13], [1, CC_RUN]])
                dst = g1b[pl].rearrange("p (i r) -> p i r", i=P13)
                nc.sync.dma_start(dst, src)
            return dict(g=g, MP=MP, g1b=g1b, cv_dram=cv_dram)

        # ---------- stage 1: softmax offsets -> gather launches ----------
        def stage1(s0, yg):
            g, MP, g1b, cv_dram = s0["g"], s0["MP"], s0["g1b"], s0["cv_dram"]
            cc0 = bass.AP(g1b[:].tensor, g1b[:].offset,
                          [g1b[:].ap[0], [CC_RUN, P13], [3, P13]])
            e1 = sbq.tile([MP, N169 + 1], F32, tag="e1")
            nc.scalar.activation(
                e1[:, 0:N169].rearrange("p (i j) -> p i j", i=P13), cc0, Exp,
                accum_out=e1[:, N169:N169 + 1])
            sc = sbq.tile([MP, 4], F32, tag="sc")
            tmp = sbq.tile([MP, N169], F32, tag="tmp169")
            nc.vector.scalar_tensor_tensor(
                out=tmp[:], in0=e1[:, 0:N169], scalar=0.0, in1=gridx_t,
                op0=ALU.add, op1=ALU.mult, accum_out=sc[:, 0:1])
            nc.vector.scalar_tensor_tensor(
                out=tmp[:], in0=e1[:, 0:N169], scalar=0.0, in1=gridy_t,
                op0=ALU.add, op1=ALU.mult, accum_out=sc[:, 1:2])
            offs = sbq.tile([MP, 2], F32, tag="offs")   # [off_x, off_y]
            nc.vector.reciprocal(sc[:, 2:3], e1[:, N169:N169 + 1])
            nc.vector.tensor_tensor(offs[:, 0:1], sc[:, 0:1], sc[:, 2:3],
                                    op=ALU.mult)
            nc.vector.tensor_tensor(offs[:, 1:2], sc[:, 1:2], sc[:, 2:3],
                                    op=ALU.mult)
            nc.vector.tensor_scalar(offs[:], offs[:], float(MAXOFF),
                                    -float(MAXOFF), op0=ALU.min, op1=ALU.max)
            # floor (mode-agnostic): fb = off+1024; fbi=cast;
            # fbf=cast back; fbf -= (fb - fbf < 0); wfrac = fb - fbf
            fb = sbq.tile([MP, 2], F32, tag="fb")
            nc.vector.tensor_scalar(fb[:], offs[:], FLOOR_BIAS, None,
                                    op0=ALU.add)
            fbi = sbq.tile([MP, 2], I32, tag="fbi")
            nc.vector.tensor_copy(fbi[:], fb[:])
            fbf = sbq.tile([MP, 2], F32, tag="fbf")
            nc.vector.tensor_copy(fbf[:], fbi[:])
            err = sbq.tile([MP, 2], F32, tag="err")
            nc.vector.tensor_tensor(err[:], fb[:], fbf[:], op=ALU.subtract)
            neg = sbq.tile([MP, 2], F32, tag="neg")
            nc.vector.tensor_scalar(neg[:], err[:], 0.0, None, op0=ALU.is_lt)
            nc.vector.tensor_tensor(fbf[:], fbf[:], neg[:], op=ALU.subtract)
            wfrac = sbq.tile([MP, 2], F32, tag="wfrac")  # [wx, wy]
            nc.vector.tensor_tensor(wfrac[:], fb[:], fbf[:], op=ALU.subtract)
            s2 = sbq.tile([MP, 1], F32, tag="s2")
            nc.vector.scalar_tensor_tensor(
                out=s2[:], in0=fbf[:, 1:2], scalar=float(WCV),
                in1=fbf[:, 0:1], op0=ALU.mult, op1=ALU.add)
            i1f = sbq.tile([MP, 1], F32, tag="i1f")
            nc.vector.tensor_scalar(i1f[:], c2cv_t, s2[:], None,
                                    op0=ALU.add)
            idx2cv = sbq.tile([MP, 1], I32, tag="idx2cv")
            nc.gpsimd.tensor_copy(idx2cv[:], i1f[:])
            # QI element index: c2qr + (s2 + yg*WCV)*448
            idx2qrf = sbq.tile([MP, 1], F32, tag="idx2qrf")
            nc.gpsimd.tensor_scalar(idx2qrf[:], s2[:], 448.0,
                                    float(yg * WCV * 448),
                                    op0=ALU.mult, op1=ALU.add)
            nc.gpsimd.tensor_tensor(idx2qrf[:], idx2qrf[:], c2qr_t,
                                    op=ALU.add)
            idx2qr = sbq.tile([MP, 1], I32, tag="idx2qr")
            nc.gpsimd.tensor_copy(idx2qr[:], idx2qrf[:])

            # ---------- C. phase-2 gathers ----------
            NS2 = 13 * WCV + 14
            g2 = sbq.tile([MP, NS2], BF16, tag="g2")
            nc.gpsimd.indirect_dma_start(
                out=g2[:], out_offset=None, in_=cv_dram[:],
                in_offset=bass.IndirectOffsetOnAxis(ap=idx2cv[:], axis=1))
            if g == 0:
                g2_dbg.append(g2)
                g1b_dbg.append(g1b)
                offs_dbg.append(offs)
            qt = sbq.tile([MP, QT_N], BF16, tag="qt")
            nc.gpsimd.indirect_dma_start(
                out=qt[:], out_offset=None, in_=ins["qr0pad"],
                in_offset=bass.IndirectOffsetOnAxis(ap=idx2qr[:], axis=1))

            # bilinear corner weights
            ww = sbq.tile([MP, 4], F32, tag="ww")
            om = sbq.tile([MP, 2], F32, tag="om")
            nc.vector.tensor_scalar(om[:], wfrac[:], -1.0, 1.0,
                                    op0=ALU.mult, op1=ALU.add)
            nc.vector.tensor_tensor(ww[:, 0:1], om[:, 1:2], om[:, 0:1],
                                    op=ALU.mult)
            nc.vector.tensor_tensor(ww[:, 1:2], om[:, 1:2], wfrac[:, 0:1],
                                    op=ALU.mult)
            nc.vector.tensor_tensor(ww[:, 2:3], wfrac[:, 1:2], om[:, 0:1],
                                    op=ALU.mult)
            nc.vector.tensor_tensor(ww[:, 3:4], wfrac[:, 1:2], wfrac[:, 0:1],
                                    op=ALU.mult)
            return dict(g=g, MP=MP, g2=g2, qt=qt, ww=ww)

        # ---------- stage 2: bilinear blend -> p0 -> blur -> o0 ----------
        def stage2(st):
            g, MP, g2, qt, ww = st["g"], st["MP"], st["g2"], st["qt"], st["ww"]
            g2v = bass.AP(g2[:].tensor, g2[:].offset,
                          [g2[:].ap[0], [WCV, 14], [1, 14]])
            corr = sbq.tile([MP, N169], F32, tag="corr")
            crv = corr[:].rearrange("p (i j) -> p i j", i=P13)
            nc.vector.tensor_scalar(crv, g2v[:, 0:13, 0:13], ww[:, 0:1], None,
                                    op0=ALU.mult)
            for (sl_u, sl_v, wcol) in (((0, 13), (1, 14), 1),
                                       ((1, 14), (0, 13), 2),
                                       ((1, 14), (1, 14), 3)):
                nc.vector.scalar_tensor_tensor(
                    out=crv, in0=g2v[:, sl_u[0]:sl_u[1], sl_v[0]:sl_v[1]],
                    scalar=ww[:, wcol:wcol + 1], in1=crv,
                    op0=ALU.mult, op1=ALU.add)
            p0 = sb.tile([MP, N169 + 1], F32, tag=f"p0_{g}")
            nc.scalar.activation(p0[:, 0:N169], corr[:], Exp,
                                 accum_out=p0[:, N169:N169 + 1])
            z0_g[g] = p0
            # bilinear blur, v-major: bb[p, v, u]
            bb = sbq.tile([MP, 196], BF16, tag="bb")
            nc.gpsimd.memset(bb[:], 0.0)
            bbv = bb[:].rearrange("p (v u) -> p v u", v=14)
            # p0 viewed as (j=v, i=u): AP dims j outer (stride 1), i inner
            p0ji = bass.AP(p0[:].tensor, p0[:].offset,
                           [p0[:].ap[0], [1, P13], [P13, P13]])
            nc.vector.tensor_scalar(bbv[:, 0:13, 0:13], p0ji, ww[:, 0:1],
                                    None, op0=ALU.mult)
            for (sl_v, sl_u, wcol) in (((1, 14), (0, 13), 1),
                                       ((0, 13), (1, 14), 2),
                                       ((1, 14), (1, 14), 3)):
                dstv = bbv[:, sl_v[0]:sl_v[1], sl_u[0]:sl_u[1]]
                nc.vector.scalar_tensor_tensor(
                    out=dstv, in0=p0ji, scalar=ww[:, wcol:wcol + 1], in1=dstv,
                    op0=ALU.mult, op1=ALU.add)
            # packed bf16 multiply qt *= bb (broadcast over c via 0-stride)
            qtv = bass.AP(qt[:].tensor, qt[:].offset,
                          [qt[:].ap[0], [448, 14], [14, C], [1, 14]])
            bbb = bass.AP(bb[:].tensor, bb[:].offset,
                          [bb[:].ap[0], [14, 14], [0, C], [1, 14]])
            nc.vector.tensor_tensor(qtv, qtv, bbb, op=ALU.mult)
            # fold v twice: 14 -> 7 -> 4 rows, then reduce keep c
            nc.vector.tensor_tensor(qt[:, 0:7 * 448], qt[:, 0:7 * 448],
                                    qt[:, 7 * 448:14 * 448], op=ALU.add)
            nc.vector.tensor_tensor(qt[:, 0:3 * 448], qt[:, 0:3 * 448],
                                    qt[:, 4 * 448:7 * 448], op=ALU.add)
            o0 = sb.tile([MP, C], F32, tag=f"o0_{g}")
            pr = bass.AP(qt[:].tensor, qt[:].offset,
                         [qt[:].ap[0], [14, C], [448, 4], [1, 14]])
            nc.vector.tensor_reduce(o0[:], pr, axis=AX.XY, op=ALU.add)
            out0_g[g] = o0

        # ---------- E. refs 1/2, s-major ----------
        def emit_ref(r):
            em = sbe.tile([WB, EM_N], BF16, tag="em")
            for s in range(NRQ):
                w = S_W[s]
                ct = ps_cc.tile([WB, 512], F32, tag="ct")
                nc.tensor.matmul(
                    ct[:, 0:w],
                    lhsT=pa((A_F2P1, A_F2P2)[r] + s * WB, WB),
                    rhs=f1_t[:, S_LO[s] * W:S_LO[s] * W + w],
                    start=True, stop=True)
                nc.scalar.activation(em[:, S_OFF[s]:S_OFF[s] + w],
                                     ct[:, 0:w], Exp)
            return em

        def emit_mask(em):
            nc.vector.tensor_tensor(em[:], em[:], maskM_t, op=ALU.mult)

        def emit_attn(r, em, op2, last):
            for s in range(NRQ):
                nc.tensor.matmul(
                    op2[:, S_LO[s] * W:S_LO[s] * W + S_W[s]],
                    lhsT=packB_t[:, (B_QRT1, B_QRT2)[r] + s * (C + 1):
                                 (B_QRT1, B_QRT2)[r] + (s + 1) * (C + 1)],
                    rhs=em[:, S_OFF[s]:S_OFF[s] + S_W[s]],
                    start=False, stop=(last and s == NRQ - 1),
                    skip_group_check=True)

        # ---------- F. combine ----------
        ofst = sbg.tile([W, RY * C], F32, tag="ofst")

        def combine_rows(ops, yrs):
            for yr in yrs:
                opT = ps_tr.tile([W, C + 1], F32, tag="opT")
                nc.tensor.transpose(opT[:], ops[:, yr * W:(yr + 1) * W],
                                    ident_t)
                g = yr // 2
                p_lo = 64 * (yr % 2)
                psl = slice(p_lo, p_lo + W)
                den = sbq.tile([W, 2], F32, tag="den")
                nc.vector.tensor_tensor(den[:, 0:1], opT[:, C:C + 1],
                                        z0_g[g][psl, N169:N169 + 1],
                                        op=ALU.add)
                nc.vector.reciprocal(den[:, 1:2], den[:, 0:1])
                of = ofst[:, yr * C:(yr + 1) * C]
                nc.vector.tensor_tensor(of, opT[:, 0:C], out0_g[g][psl, :],
                                        op=ALU.add)
                nc.vector.tensor_scalar(of, of, den[:, 1:2], None,
                                        op0=ALU.mult)

        # ---------- schedule ----------
        op2 = ps_out.tile([C + 1, RY * W], F32, tag="op2")
        nc.vector.memset(op2[:], 0.0)

        g2_dbg, g1b_dbg, offs_dbg = [], [], []
        s00 = stage0(0)
        s01 = stage0(1)
        s02 = stage0(2)
        s03 = stage0(3)
        st0 = stage1(s00, 0)
        st1 = stage1(s01, 2)
        st2 = stage1(s02, 4)
        st3 = stage1(s03, 6)
        em0 = emit_ref(0)
        stage2(st0)
        emit_mask(em0)
        emit_attn(0, em0, op2, last=False)
        stage2(st1)
        em1 = emit_ref(1)
        emit_mask(em1)
        emit_attn(1, em1, op2, last=True)
        stage2(st2)
        ops = sbg.tile([C + 1, RY * W], F32, tag="ops")
        nc.scalar.copy(ops[:], op2[:])
        combine_rows(ops, (0, 1, 2, 3, 4, 5))
        stage2(st3)
        combine_rows(ops, (6,))
        dst = bass.AP(outs["out"].tensor, outs["out"].offset,
                      [[C, W], [W * C, RY], [1, C]])
        nc.sync.dma_start(dst, ofst[:].rearrange("p (y c) -> p y c", y=RY))

        if "dbg_ops" in outs:
            nc.sync.dma_start(outs["dbg_ops"], ops[:])
            nc.sync.dma_start(outs["dbg_em0"], em0[:])
            nc.sync.dma_start(outs["dbg_o0"], out0_g[0][:])
            nc.sync.dma_start(outs["dbg_p0"], z0_g[0][:])
            nc.sync.dma_start(outs["dbg_g2"], g2_dbg[0][:])
            nc.sync.dma_start(outs["dbg_g1b"], g1b_dbg[0][:])
            nc.sync.dma_start(outs["dbg_offs"], offs_dbg[0][:])


DEBUG_SPECS = dict(
    dbg_ops=([C + 1, RY * W], F32), dbg_em0=([WB, EM_N], BF16),
    dbg_o0=([PPG, C], F32), dbg_p0=([PPG, N169 + 1], F32),
    dbg_g2=([PPG, 13 * WCV + 14], BF16), dbg_g1b=([PPG, P13 * CC_RUN], BF16),
    dbg_offs=([PPG, 2], F32),
)


def build_program(ncores=NCORES, debug=False):
    import concourse.bacc as bacc
    nc = bacc.Bacc("TRN2", target_bir_lowering=False, debug=False,
                   enable_asserts=True, num_devices=ncores)
    ins = {}
    for name, (shape, dt_) in INPUT_SPECS.items():
        ins[name] = nc.dram_tensor(name, shape, dt_, kind="ExternalInput").ap()
    outs = {"out": nc.dram_tensor("out", OUT_SPEC[0], OUT_SPEC[1],
                                  kind="ExternalOutput").ap()}
    if debug:
        for name, (shape, dt_) in DEBUG_SPECS.items():
            outs[name] = nc.dram_tensor(name, shape, dt_,
                                        kind="ExternalOutput").ap()
    with tile.TileContext(nc) as tc:
        build_kernel(tc, outs, ins)
    nc.compile()
    return nc


# ======================= runner =======================
import os as _os

_LAST_RESULT = {}


def kernel(**inputs):
    from concourse.bass_utils import run_bass_kernel_spmd
    from concourse.bass_interp import get_hw_module

    in_maps = host_prep(**inputs)
    nc = build_program()
    nc.m = get_hw_module(nc.m)
    trace = _os.environ.get("KERNEL_TRACE", "0") == "1"
    res = run_bass_kernel_spmd(
        nc, in_maps, core_ids=list(range(NCORES)), trace=trace)
    _LAST_RESULT["res"] = res
    slabs = [np.asarray(res.results[i]["out"], np.float32).reshape(RY, W, C)
             for i in range(NCORES)]
    full = np.concatenate(slabs, 0)          # [56, 56, 32]
    return np.ascontiguousarray(full.transpose(2, 0, 1)[None])


# revision 61
# speedup vs baseline: 31.1373x; 1.0385x over previous
"""Bass/Tile kernel for nn_Colorizer (sparse deformable attention colorizer).

Sharding: spatial row-sharding across 8 cores; core i owns output rows
[7i, 7i+7). The joint softmax over nref*N is additive across refs so each
core normalizes locally.

Per-core pipeline:
  A. CV volume (search ref): 4-row-batched PE matmuls -> PSUM -> bf16 SBUF
     (copies spread over DVE/Act/Pool) -> cv_dram [p, 51, 105].
  B. Phase-1: static strided DMAs pull the stride-3 13x37 runs straight
     from cv_dram -> exp -> expected offset -> floor/frac -> gather idxs.
  C. Phase-2: 14-run CV window gather (196 el) + qr0 QI gather (u-inner
     interleave -> [v,c,u] runs).
  D. Ref0: bilinear blend -> exp -> B-blur (v-major) -> packed bf16
     multiply + fold -> reduce -> o0, Z0.
  E. Refs 1/2 (s-major): per f2 row s one banded cc matmul [68, w_s*56]
     -> exp into packed em -> one mask multiply per ref -> per-s
     attention matmul accumulated into op2 [33, 392] (d-major).
  F. Combine: PE-transpose op2 per row -> (+o0)/(Z) -> staged -> DRAM.
"""
from contextlib import ExitStack

import numpy as np
import ml_dtypes

import concourse.bass as bass

NPBF16 = ml_dtypes.bfloat16
import concourse.mybir as mybir
import concourse.tile as tile

F32 = mybir.dt.float32
I32 = mybir.dt.int32
BF16 = mybir.dt.bfloat16

# ---------------- geometry ----------------
D_SUB, R, C = 4, 6, 32
P13 = 2 * R + 1          # 13
N169 = P13 * P13
DIL_INT = 15
H = W = 56
CF = 64
NCORES = 8
RY = H // NCORES         # 7

DIL = 3
MAXOFF = R * DIL         # 18
DYLO = MAXOFF + R        # 24
NDY = 2 * DYLO + 2       # 50 rows needed for one y-row
WCV = W + NDY - 1        # 105
NROWS_G = NDY + 1        # 51 rows per pair group
HP = H + NDY - 1         # 105
WB = W + 2 * R           # 68
H_SLAB = H + 1           # 57: uniform 51-row pitch for all 4 groups
NRQ = RY + 2 * R         # 19
CC_RUN = 3 * (P13 - 1) + 1   # 37

FLOOR_BIAS = 1024.0
IDX_BIAS = int(FLOOR_BIAS) * WCV + int(FLOOR_BIAS)

GROUPS = [(0, 0, 128), (1, 2, 128), (2, 4, 128), (3, 6, 128)]
PPG = 128  # partitions per group: rows at offsets 0 and 64

# s-major banded attention: for f2 slab row s, valid yr in [LO[s], HI[s]]
S_LO = [max(0, s - (P13 - 1)) for s in range(NRQ)]
S_HI = [min(RY - 1, s) for s in range(NRQ)]
S_W = [(S_HI[s] - S_LO[s] + 1) * W for s in range(NRQ)]
S_OFF = np.concatenate([[0], np.cumsum(S_W)]).tolist()
EM_N = S_OFF[-1]          # 91*56 = 5096

CVP = NROWS_G * WCV       # 5355 per-pixel cv pitch
QT_N = 14 * 448           # qt stream: v(14) x c(32) x u(14)


def _pad2(a, top, left, hh, ww):
    out = np.zeros(a.shape[:-2] + (hh, ww), a.dtype)
    out[..., top:top + a.shape[-2], left:left + a.shape[-1]] = a
    return out


def host_prep(feats_r, feats_t, quantized_r, ref_index, current_ind):
    feats_r = np.asarray(feats_r, np.float32)
    feats_t = np.asarray(feats_t, np.float32)
    quantized_r = np.asarray(quantized_r, np.float32)
    ri = np.asarray(ref_index).tolist()
    ci = int(current_ind)
    diffs = [ci - int(x) for x in ri]
    nsearch = sum(1 for d in diffs if d > DIL_INT)
    dirates = [min(4, d // DIL_INT + 1) for d in diffs if d > DIL_INT]
    nref = feats_r.shape[0]
    assert nsearch == 1 and dirates[0] == DIL and nref == 3, \
        (nsearch, dirates, nref)

    f1 = feats_t[0]
    f2 = [feats_r[s, 0] for s in range(nref)]
    qr = [quantized_r[s, 0][:, ::D_SUB, ::D_SUB] for s in range(nref)]

    # u-inner interleaved qr0: QI[y, x, c, u] = qr0pad[y+u, x, c]
    qr0can = np.zeros((HP + 14, HP, C), np.float32)
    qr0can[DYLO:DYLO + H, DYLO:DYLO + W, :] = qr[0].transpose(1, 2, 0)
    qi = np.stack([qr0can[u:u + HP] for u in range(14)], axis=3)  # [HP,HP,C,14]
    qi = qi.reshape(1, HP * HP * C * 14)
    qi_b16 = np.ascontiguousarray(qi.astype(NPBF16))

    # f2_0 canvas: rows [-24 .. H+26], cols [-24 .. 80]
    f2p0 = _pad2(f2[0], DYLO, DYLO, H + 2 * DYLO + 3, WCV)
    f2p12 = [_pad2(f2[r], R, R, H + 2 * R, WB) for r in (1, 2)]
    qrpT = []
    for r in (1, 2):
        q = np.zeros((H + 2 * R, WB, C + 1), np.float32)
        q[R:R + H, R:R + W, :C] = qr[r].transpose(1, 2, 0)
        q[:, :, C] = 1.0
        qrpT.append(np.ascontiguousarray(q.transpose(1, 0, 2)))

    ploc128 = np.arange(PPG)
    yloc = (ploc128 >= 64).astype(np.int64)
    xs = np.minimum(ploc128 - 64 * yloc, W - 1)
    # phase-2 CV stream const: row 18+yloc, col x+18; + oi_y*105 + oi_x
    c2cv = (((ploc128 * NROWS_G + MAXOFF + yloc) * WCV + xs + MAXOFF)
            - IDX_BIAS)[:, None]
    # phase-2 QI stream const (element units): ((y+18+yloc)*105 + x+18)*448
    c2qr = ((((yloc + MAXOFF) * WCV + xs + MAXOFF) - IDX_BIAS) * 448)[:, None]

    gridy = np.tile((np.repeat(np.arange(P13) - R, P13) * DIL)[None, :],
                    (PPG, 1)).astype(np.float32)
    gridx = np.tile((np.tile(np.arange(P13) - R, P13) * DIL)[None, :],
                    (PPG, 1)).astype(np.float32)
    uconst = np.tile((np.arange(14) * WCV)[None, :], (PPG, 1)).astype(np.float32)

    xq = np.arange(WB)[:, None]
    xx = np.arange(W)[None, :]
    maskT = ((xq - xx >= 0) & (xq - xx <= 2 * R)).astype(np.float32)  # [68,56]
    maskM = np.tile(maskT[:, None, :], (1, EM_N // W, 1)).reshape(WB, EM_N)
    ident33 = np.zeros((PPG, C + 1), np.float32)
    ident33[:C + 1, :] = np.eye(C + 1)

    def b16(a):
        return np.ascontiguousarray(a.astype(NPBF16))

    in_maps = []
    for core in range(NCORES):
        y0 = core * RY
        f1pair = np.zeros((CF, 4 * PPG), np.float32)
        for g in range(4):
            f1pair[:, g * PPG:g * PPG + W] = f1[:, y0 + 2 * g, :]
            if 2 * g + 1 < RY:
                f1pair[:, g * PPG + 64:g * PPG + 64 + W] = f1[:, y0 + 2 * g + 1, :]
        # 64-partition packs: A (CV-critical) = f1pair | f2p0;
        # D = f1 | f2p1 | f2p2
        packA = np.concatenate([
            f1pair,
            f2p0[:, y0:y0 + H_SLAB, :].reshape(CF, H_SLAB * WCV),
        ], axis=1)
        packD = np.concatenate([
            f1[:, y0:y0 + RY, :].reshape(CF, RY * W),
            f2p12[0][:, y0:y0 + NRQ, :].reshape(CF, NRQ * WB),
            f2p12[1][:, y0:y0 + NRQ, :].reshape(CF, NRQ * WB),
        ], axis=1)
        # 68-partition pack: qrT1 | qrT2 | maskM
        packB = np.concatenate([
            qrpT[0][:, y0:y0 + NRQ, :].reshape(WB, NRQ * (C + 1)),
            qrpT[1][:, y0:y0 + NRQ, :].reshape(WB, NRQ * (C + 1)),
            maskM,
        ], axis=1)
        # 128-partition f32 pack: gridx | gridy | uconst | c2cv | c2qr | id33
        packC = np.concatenate([
            gridx, gridy, uconst, c2cv, c2qr + y0 * WCV * 448, ident33,
        ], axis=1).astype(np.float32)
        m = dict(
            packA=b16(packA),
            packB=b16(packB),
            packC=packC,
            packD=b16(packD),
            qr0pad=qi_b16,
        )
        in_maps.append(m)
    return in_maps


# pack offsets (elements)
A_F1PAIR = 0
A_F2P0 = A_F1PAIR + 4 * PPG
A_N = A_F2P0 + H_SLAB * WCV
D_F1 = 0
D_F2P1 = D_F1 + RY * W
D_F2P2 = D_F2P1 + NRQ * WB
D_N = D_F2P2 + NRQ * WB
B_QRT1 = 0
B_QRT2 = B_QRT1 + NRQ * (C + 1)
B_MASK = B_QRT2 + NRQ * (C + 1)
B_N = B_MASK + EM_N
C_GX = 0
C_GY = C_GX + N169
C_UC = C_GY + N169
C_CV = C_UC + 14
C_QR = C_CV + 1
C_ID = C_QR + 1
C_N = C_ID + (C + 1)

INPUT_SPECS = dict(
    packA=([CF, A_N], BF16),
    packB=([WB, B_N], BF16),
    packC=([PPG, C_N], F32),
    packD=([CF, D_N], BF16),
    qr0pad=([1, HP * HP * C * 14], BF16),
)
OUT_SPEC = ([RY * W, C], F32)


def build_kernel(tc, outs, ins):
    nc = tc.nc
    Exp = mybir.ActivationFunctionType.Exp
    Copy = mybir.ActivationFunctionType.Copy
    ALU = mybir.AluOpType
    AX = mybir.AxisListType

    with ExitStack() as ctx:
        sb = ctx.enter_context(tc.tile_pool(name="sb", bufs=1))
        sbg = ctx.enter_context(tc.tile_pool(name="sbg", bufs=2))
        sbe = ctx.enter_context(tc.tile_pool(name="sbe", bufs=2))
        sbq = ctx.enter_context(tc.tile_pool(name="sbq", bufs=4))
        ps_cv = ctx.enter_context(tc.tile_pool(name="ps_cv", bufs=3, space="PSUM"))
        ps_cc = ctx.enter_context(tc.tile_pool(name="ps_cc", bufs=2, space="PSUM"))
        ps_out = ctx.enter_context(tc.tile_pool(name="ps_out", bufs=1, space="PSUM"))
        ps_tr = ctx.enter_context(tc.tile_pool(name="ps_tr", bufs=1, space="PSUM"))
        dram = ctx.enter_context(tc.tile_pool(name="dram", bufs=1, space="DRAM"))

        def load(name):
            shape, dt_ = INPUT_SPECS[name]
            t = sb.tile(shape, dt_, tag=name)
            nc.sync.dma_start(t[:], ins[name])
            return t

        packA_t = load("packA")
        packC_t = load("packC")
        packD_t = load("packD")
        packB_t = load("packB")

        def pa(off, n):
            return packA_t[:, off:off + n]

        def pd(off, n):
            return packD_t[:, off:off + n]

        f1_t = pd(D_F1, RY * W)
        maskM_t = packB_t[:, B_MASK:B_MASK + EM_N]
        ident_t = packC_t[0:C + 1, C_ID:C_ID + (C + 1)]
        gridx_t = packC_t[:, C_GX:C_GX + N169]
        gridy_t = packC_t[:, C_GY:C_GY + N169]
        uconst_t = packC_t[:, C_UC:C_UC + 14]
        c2cv_t = packC_t[:, C_CV:C_CV + 1]
        c2qr_t = packC_t[:, C_QR:C_QR + 1]

        out0_g, z0_g = {}, {}

        # ---------- stage 0: CV volume -> DRAM + phase-1 window reads ----
        def stage0(g, MP=PPG):
            nrow = NROWS_G
            cv_sb = sbg.tile([MP, nrow * WCV], BF16, tag="cv_sb")
            lhs = pa(A_F1PAIR + g * PPG, PPG)
            ci = 0
            # 4 slab rows per matmul (420 cols in one PSUM bank)
            for r0 in range(0, nrow, 4):
                rn = min(4, nrow - r0)
                ncol = rn * WCV
                pt = ps_cv.tile([MP, 512], F32, tag="cvch")
                nc.tensor.matmul(
                    pt[:, 0:ncol], lhsT=lhs,
                    rhs=pa(A_F2P0 + (2 * g + r0) * WCV, ncol),
                    start=True, stop=True)
                dst = cv_sb[:, r0 * WCV:(r0 + rn) * WCV]
                if ci % 2 == 0:
                    nc.vector.tensor_copy(dst, pt[:, 0:ncol])
                else:
                    nc.scalar.copy(dst, pt[:, 0:ncol])
                ci += 1
            cv_dram = dram.tile([1, MP * CVP + 8 * WCV], BF16, tag=f"cvd{g}")
            nc.sync.dma_start(
                cv_dram[:, 0:MP * CVP].rearrange("o (p f) -> p (o f)", p=MP),
                cv_sb[:])

            # ---------- B. phase-1: static strided reads of cv ----------
            g1b = sbq.tile([MP, P13 * CC_RUN], BF16, tag="g1b")
            for pl, ybase in ((slice(0, 64), R), (slice(64, 128), R + 1)):
                p0_ = pl.start
                base = p0_ * CVP + ybase * WCV + R
                src = bass.AP(cv_dram[:].tensor, cv_dram[:].offset + base,
                              [[CVP + 1, 64], [3 * WCV, P13], [1, CC_RUN]])
                dst = g1b[pl].rearrange("p (i r) -> p i r", i=P13)
                nc.sync.dma_start(dst, src)
            return dict(g=g, MP=MP, g1b=g1b, cv_dram=cv_dram)

        # ---------- stage 1: softmax offsets -> gather launches ----------
        def stage1(s0, yg):
            g, MP, g1b, cv_dram = s0["g"], s0["MP"], s0["g1b"], s0["cv_dram"]
            cc0 = bass.AP(g1b[:].tensor, g1b[:].offset,
                          [g1b[:].ap[0], [CC_RUN, P13], [3, P13]])
            e1 = sbq.tile([MP, N169 + 1], F32, tag="e1")
            nc.scalar.activation(
                e1[:, 0:N169].rearrange("p (i j) -> p i j", i=P13), cc0, Exp,
                accum_out=e1[:, N169:N169 + 1])
            sc = sbq.tile([MP, 4], F32, tag="sc")
            tmp = sbq.tile([MP, N169], F32, tag="tmp169")
            nc.vector.scalar_tensor_tensor(
                out=tmp[:], in0=e1[:, 0:N169], scalar=0.0, in1=gridx_t,
                op0=ALU.add, op1=ALU.mult, accum_out=sc[:, 0:1])
            nc.vector.scalar_tensor_tensor(
                out=tmp[:], in0=e1[:, 0:N169], scalar=0.0, in1=gridy_t,
                op0=ALU.add, op1=ALU.mult, accum_out=sc[:, 1:2])
            offs = sbq.tile([MP, 2], F32, tag="offs")   # [off_x, off_y]
            nc.vector.reciprocal(sc[:, 2:3], e1[:, N169:N169 + 1])
            nc.vector.tensor_tensor(offs[:, 0:1], sc[:, 0:1], sc[:, 2:3],
                                    op=ALU.mult)
            nc.vector.tensor_tensor(offs[:, 1:2], sc[:, 1:2], sc[:, 2:3],
                                    op=ALU.mult)
            nc.vector.tensor_scalar(offs[:], offs[:], float(MAXOFF),
                                    -float(MAXOFF), op0=ALU.min, op1=ALU.max)
            # floor (mode-agnostic): fb = off+1024; fbi=cast;
            # fbf=cast back; fbf -= (fb - fbf < 0); wfrac = fb - fbf
            fb = sbq.tile([MP, 2], F32, tag="fb")
            nc.vector.tensor_scalar(fb[:], offs[:], FLOOR_BIAS, None,
                                    op0=ALU.add)
            fbi = sbq.tile([MP, 2], I32, tag="fbi")
            nc.vector.tensor_copy(fbi[:], fb[:])
            fbf = sbq.tile([MP, 2], F32, tag="fbf")
            nc.vector.tensor_copy(fbf[:], fbi[:])
            err = sbq.tile([MP, 2], F32, tag="err")
            nc.vector.tensor_tensor(err[:], fb[:], fbf[:], op=ALU.subtract)
            neg = sbq.tile([MP, 2], F32, tag="neg")
            nc.vector.tensor_scalar(neg[:], err[:], 0.0, None, op0=ALU.is_lt)
            nc.vector.tensor_tensor(fbf[:], fbf[:], neg[:], op=ALU.subtract)
            wfrac = sbq.tile([MP, 2], F32, tag="wfrac")  # [wx, wy]
            nc.vector.tensor_tensor(wfrac[:], fb[:], fbf[:], op=ALU.subtract)
            s2 = sbq.tile([MP, 1], F32, tag="s2")
            nc.vector.scalar_tensor_tensor(
                out=s2[:], in0=fbf[:, 1:2], scalar=float(WCV),
                in1=fbf[:, 0:1], op0=ALU.mult, op1=ALU.add)
            i1f = sbq.tile([MP, 1], F32, tag="i1f")
            nc.vector.tensor_scalar(i1f[:], c2cv_t, s2[:], None,
                                    op0=ALU.add)
            idx2cv = sbq.tile([MP, 1], I32, tag="idx2cv")
            nc.gpsimd.tensor_copy(idx2cv[:], i1f[:])
            # QI element index: c2qr + (s2 + yg*WCV)*448
            idx2qrf = sbq.tile([MP, 1], F32, tag="idx2qrf")
            nc.gpsimd.tensor_scalar(idx2qrf[:], s2[:], 448.0,
                                    float(yg * WCV * 448),
                                    op0=ALU.mult, op1=ALU.add)
            nc.gpsimd.tensor_tensor(idx2qrf[:], idx2qrf[:], c2qr_t,
                                    op=ALU.add)
            idx2qr = sbq.tile([MP, 1], I32, tag="idx2qr")
            nc.gpsimd.tensor_copy(idx2qr[:], idx2qrf[:])

            # ---------- C. phase-2 gathers ----------
            NS2 = 13 * WCV + 14
            g2 = sbq.tile([MP, NS2], BF16, tag="g2")
            nc.gpsimd.indirect_dma_start(
                out=g2[:], out_offset=None, in_=cv_dram[:],
                in_offset=bass.IndirectOffsetOnAxis(ap=idx2cv[:], axis=1))
            if g == 0:
                g2_dbg.append(g2)
                g1b_dbg.append(g1b)
                offs_dbg.append(offs)
            qt = sbq.tile([MP, QT_N], BF16, tag="qt")
            nc.gpsimd.indirect_dma_start(
                out=qt[:], out_offset=None, in_=ins["qr0pad"],
                in_offset=bass.IndirectOffsetOnAxis(ap=idx2qr[:], axis=1))

            # bilinear corner weights
            ww = sbq.tile([MP, 4], F32, tag="ww")
            om = sbq.tile([MP, 2], F32, tag="om")
            nc.vector.tensor_scalar(om[:], wfrac[:], -1.0, 1.0,
                                    op0=ALU.mult, op1=ALU.add)
            nc.vector.tensor_tensor(ww[:, 0:1], om[:, 1:2], om[:, 0:1],
                                    op=ALU.mult)
            nc.vector.tensor_tensor(ww[:, 1:2], om[:, 1:2], wfrac[:, 0:1],
                                    op=ALU.mult)
            nc.vector.tensor_tensor(ww[:, 2:3], wfrac[:, 1:2], om[:, 0:1],
                                    op=ALU.mult)
            nc.vector.tensor_tensor(ww[:, 3:4], wfrac[:, 1:2], wfrac[:, 0:1],
                                    op=ALU.mult)
            return dict(g=g, MP=MP, g2=g2, qt=qt, ww=ww)

        # ---------- stage 2: bilinear blend -> p0 -> blur -> o0 ----------
        def stage2(st):
            g, MP, g2, qt, ww = st["g"], st["MP"], st["g2"], st["qt"], st["ww"]
            g2v = bass.AP(g2[:].tensor, g2[:].offset,
                          [g2[:].ap[0], [WCV, 14], [1, 14]])
            corr = sbq.tile([MP, N169], F32, tag="corr")
            crv = corr[:].rearrange("p (i j) -> p i j", i=P13)
            nc.vector.tensor_scalar(crv, g2v[:, 0:13, 0:13], ww[:, 0:1], None,
                                    op0=ALU.mult)
            for (sl_u, sl_v, wcol) in (((0, 13), (1, 14), 1),
                                       ((1, 14), (0, 13), 2),
                                       ((1, 14), (1, 14), 3)):
                nc.vector.scalar_tensor_tensor(
                    out=crv, in0=g2v[:, sl_u[0]:sl_u[1], sl_v[0]:sl_v[1]],
                    scalar=ww[:, wcol:wcol + 1], in1=crv,
                    op0=ALU.mult, op1=ALU.add)
            p0 = sb.tile([MP, N169 + 1], F32, tag=f"p0_{g}")
            nc.scalar.activation(p0[:, 0:N169], corr[:], Exp,
                                 accum_out=p0[:, N169:N169 + 1])
            z0_g[g] = p0
            # bilinear blur, v-major: bb[p, v, u]
            bb = sbq.tile([MP, 196], BF16, tag="bb")
            nc.gpsimd.memset(bb[:], 0.0)
            bbv = bb[:].rearrange("p (v u) -> p v u", v=14)
            # p0 viewed as (j=v, i=u): AP dims j outer (stride 1), i inner
            p0ji = bass.AP(p0[:].tensor, p0[:].offset,
                           [p0[:].ap[0], [1, P13], [P13, P13]])
            nc.vector.tensor_scalar(bbv[:, 0:13, 0:13], p0ji, ww[:, 0:1],
                                    None, op0=ALU.mult)
            for (sl_v, sl_u, wcol) in (((1, 14), (0, 13), 1),
                                       ((0, 13), (1, 14), 2),
                                       ((1, 14), (1, 14), 3)):
                dstv = bbv[:, sl_v[0]:sl_v[1], sl_u[0]:sl_u[1]]
                nc.vector.scalar_tensor_tensor(
                    out=dstv, in0=p0ji, scalar=ww[:, wcol:wcol + 1], in1=dstv,
                    op0=ALU.mult, op1=ALU.add)
            # packed bf16 multiply qt *= bb (broadcast over c via 0-stride)
            qtv = bass.AP(qt[:].tensor, qt[:].offset,
                          [qt[:].ap[0], [448, 14], [14, C], [1, 14]])
            bbb = bass.AP(bb[:].tensor, bb[:].offset,
                          [bb[:].ap[0], [14, 14], [0, C], [1, 14]])
            nc.vector.tensor_tensor(qtv, qtv, bbb, op=ALU.mult)
            # fold v: 14 -> 7 -> 4 -> 2 -> 1 rows, then reduce keep c
            nc.vector.tensor_tensor(qt[:, 0:7 * 448], qt[:, 0:7 * 448],
                                    qt[:, 7 * 448:14 * 448], op=ALU.add)
            nc.vector.tensor_tensor(qt[:, 0:3 * 448], qt[:, 0:3 * 448],
                                    qt[:, 4 * 448:7 * 448], op=ALU.add)
            nc.vector.tensor_tensor(qt[:, 0:2 * 448], qt[:, 0:2 * 448],
                                    qt[:, 2 * 448:4 * 448], op=ALU.add)
            nc.vector.tensor_tensor(qt[:, 0:448], qt[:, 0:448],
                                    qt[:, 448:2 * 448], op=ALU.add)
            o0 = sb.tile([MP, C], F32, tag=f"o0_{g}")
            pr = bass.AP(qt[:].tensor, qt[:].offset,
                         [qt[:].ap[0], [14, C], [1, 14]])
            nc.vector.tensor_reduce(o0[:], pr, axis=AX.X, op=ALU.add)
            out0_g[g] = o0

        # ---------- E. refs 1/2, s-major ----------
        def emit_ref(r):
            em = sbe.tile([WB, EM_N], BF16, tag="em")
            for s in range(NRQ):
                w = S_W[s]
                ct = ps_cc.tile([WB, 512], F32, tag="ct")
                nc.tensor.matmul(
                    ct[:, 0:w],
                    lhsT=pd((D_F2P1, D_F2P2)[r] + s * WB, WB),
                    rhs=f1_t[:, S_LO[s] * W:S_LO[s] * W + w],
                    start=True, stop=True)
                nc.scalar.activation(em[:, S_OFF[s]:S_OFF[s] + w],
                                     ct[:, 0:w], Exp)
            return em

        def emit_mask(em):
            nc.vector.tensor_tensor(em[:], em[:], maskM_t, op=ALU.mult)

        def emit_attn(r, em, op2, last):
            for s in range(NRQ):
                nc.tensor.matmul(
                    op2[:, S_LO[s] * W:S_LO[s] * W + S_W[s]],
                    lhsT=packB_t[:, (B_QRT1, B_QRT2)[r] + s * (C + 1):
                                 (B_QRT1, B_QRT2)[r] + (s + 1) * (C + 1)],
                    rhs=em[:, S_OFF[s]:S_OFF[s] + S_W[s]],
                    start=False, stop=(last and s == NRQ - 1),
                    skip_group_check=True)

        # ---------- F. combine ----------
        ofst = sbg.tile([W, RY * C], F32, tag="ofst")

        def combine_rows(ops, yrs):
            for yr in yrs:
                opT = ps_tr.tile([W, C + 1], F32, tag="opT")
                nc.tensor.transpose(opT[:], ops[:, yr * W:(yr + 1) * W],
                                    ident_t)
                g = yr // 2
                p_lo = 64 * (yr % 2)
                psl = slice(p_lo, p_lo + W)
                den = sbq.tile([W, 2], F32, tag="den")
                nc.vector.tensor_tensor(den[:, 0:1], opT[:, C:C + 1],
                                        z0_g[g][psl, N169:N169 + 1],
                                        op=ALU.add)
                nc.vector.reciprocal(den[:, 1:2], den[:, 0:1])
                of = ofst[:, yr * C:(yr + 1) * C]
                nc.vector.tensor_tensor(of, opT[:, 0:C], out0_g[g][psl, :],
                                        op=ALU.add)
                nc.vector.tensor_scalar(of, of, den[:, 1:2], None,
                                        op0=ALU.mult)

        # ---------- schedule ----------
        op2 = ps_out.tile([C + 1, RY * W], F32, tag="op2")
        nc.vector.memset(op2[:], 0.0)

        g2_dbg, g1b_dbg, offs_dbg = [], [], []
        s00 = stage0(0)
        s01 = stage0(1)
        s02 = stage0(2)
        s03 = stage0(3)
        st0 = stage1(s00, 0)
        st1 = stage1(s01, 2)
        st2 = stage1(s02, 4)
        st3 = stage1(s03, 6)
        em0 = emit_ref(0)
        stage2(st0)
        emit_mask(em0)
        emit_attn(0, em0, op2, last=False)
        stage2(st1)
        em1 = emit_ref(1)
        emit_mask(em1)
        emit_attn(1, em1, op2, last=True)
        stage2(st2)
        ops = sbg.tile([C + 1, RY * W], F32, tag="ops")
        nc.scalar.copy(ops[:], op2[:])
        combine_rows(ops, (0, 1, 2, 3, 4, 5))
        stage2(st3)
        combine_rows(ops, (6,))
        dst = bass.AP(outs["out"].tensor, outs["out"].offset,
                      [[C, W], [W * C, RY], [1, C]])
        nc.sync.dma_start(dst, ofst[:].rearrange("p (y c) -> p y c", y=RY))

        if "dbg_ops" in outs:
            nc.sync.dma_start(outs["dbg_ops"], ops[:])
            nc.sync.dma_start(outs["dbg_em0"], em0[:])
            nc.sync.dma_start(outs["dbg_o0"], out0_g[0][:])
            nc.sync.dma_start(outs["dbg_p0"], z0_g[0][:])
            nc.sync.dma_start(outs["dbg_g2"], g2_dbg[0][:])
            nc.sync.dma_start(outs["dbg_g1b"], g1b_dbg[0][:])
            nc.sync.dma_start(outs["dbg_offs"], offs_dbg[0][:])


DEBUG_SPECS = dict(
    dbg_ops=([C + 1, RY * W], F32), dbg_em0=([WB, EM_N], BF16),
    dbg_o0=([PPG, C], F32), dbg_p0=([PPG, N169 + 1], F32),
    dbg_g2=([PPG, 13 * WCV + 14], BF16), dbg_g1b=([PPG, P13 * CC_RUN], BF16),
    dbg_offs=([PPG, 2], F32),
)


def build_program(ncores=NCORES, debug=False):
    import concourse.bacc as bacc
    nc = bacc.Bacc("TRN2", target_bir_lowering=False, debug=False,
                   enable_asserts=True, num_devices=ncores)
    ins = {}
    for name, (shape, dt_) in INPUT_SPECS.items():
        ins[name] = nc.dram_tensor(name, shape, dt_, kind="ExternalInput").ap()
    outs = {"out": nc.dram_tensor("out", OUT_SPEC[0], OUT_SPEC[1],
                                  kind="ExternalOutput").ap()}
    if debug:
        for name, (shape, dt_) in DEBUG_SPECS.items():
            outs[name] = nc.dram_tensor(name, shape, dt_,
                                        kind="ExternalOutput").ap()
    with tile.TileContext(nc) as tc:
        build_kernel(tc, outs, ins)
    nc.compile()
    return nc


# ======================= runner =======================
import os as _os

_LAST_RESULT = {}


def kernel(**inputs):
    from concourse.bass_utils import run_bass_kernel_spmd
    from concourse.bass_interp import get_hw_module

    in_maps = host_prep(**inputs)
    nc = build_program()
    nc.m = get_hw_module(nc.m)
    trace = _os.environ.get("KERNEL_TRACE", "0") == "1"
    res = run_bass_kernel_spmd(
        nc, in_maps, core_ids=list(range(NCORES)), trace=trace)
    _LAST_RESULT["res"] = res
    slabs = [np.asarray(res.results[i]["out"], np.float32).reshape(RY, W, C)
             for i in range(NCORES)]
    full = np.concatenate(slabs, 0)          # [56, 56, 32]
    return np.ascontiguousarray(full.transpose(2, 0, 1)[None])


# revision 70
# speedup vs baseline: 31.3077x; 1.0055x over previous
"""Bass/Tile kernel for nn_Colorizer (sparse deformable attention colorizer).

Sharding: spatial row-sharding across 8 cores; core i owns output rows
[7i, 7i+7). The joint softmax over nref*N is additive across refs so each
core normalizes locally.

Per-core pipeline:
  A. CV volume (search ref): 4-row-batched PE matmuls -> PSUM -> bf16 SBUF
     (copies spread over DVE/Act/Pool) -> cv_dram [p, 51, 105].
  B. Phase-1: static strided DMAs pull the stride-3 13x37 runs straight
     from cv_dram -> exp -> expected offset -> floor/frac -> gather idxs.
  C. Phase-2: 14-run CV window gather (196 el) + qr0 QI gather (u-inner
     interleave -> [v,c,u] runs).
  D. Ref0: bilinear blend -> exp -> B-blur (v-major) -> packed bf16
     multiply + fold -> reduce -> o0, Z0.
  E. Refs 1/2 (s-major): per f2 row s one banded cc matmul [68, w_s*56]
     -> exp into packed em -> one mask multiply per ref -> per-s
     attention matmul accumulated into op2 [33, 392] (d-major).
  F. Combine: PE-transpose op2 per row -> (+o0)/(Z) -> staged -> DRAM.
"""
from contextlib import ExitStack

import numpy as np
import ml_dtypes

import concourse.bass as bass

NPBF16 = ml_dtypes.bfloat16
import concourse.mybir as mybir
import concourse.tile as tile

F32 = mybir.dt.float32
I32 = mybir.dt.int32
BF16 = mybir.dt.bfloat16

# ---------------- geometry ----------------
D_SUB, R, C = 4, 6, 32
P13 = 2 * R + 1          # 13
N169 = P13 * P13
DIL_INT = 15
H = W = 56
CF = 64
NCORES = 8
RY = H // NCORES         # 7

DIL = 3
MAXOFF = R * DIL         # 18
DYLO = MAXOFF + R        # 24
NDY = 2 * DYLO + 2       # 50 rows needed for one y-row
WCV = W + NDY - 1        # 105
NROWS_G = NDY + 1        # 51 rows per pair group
HP = H + NDY - 1         # 105
WB = W + 2 * R           # 68
H_SLAB = H + 1           # 57: uniform 51-row pitch for all 4 groups
NRQ = RY + 2 * R         # 19
CC_RUN = 3 * (P13 - 1) + 1   # 37

FLOOR_BIAS = 1024.0
IDX_BIAS = int(FLOOR_BIAS) * WCV + int(FLOOR_BIAS)

GROUPS = [(0, 0, 128), (1, 2, 128), (2, 4, 128), (3, 6, 128)]
PPG = 128  # partitions per group: rows at offsets 0 and 64

# s-major banded attention: for f2 slab row s, valid yr in [LO[s], HI[s]]
S_LO = [max(0, s - (P13 - 1)) for s in range(NRQ)]
S_HI = [min(RY - 1, s) for s in range(NRQ)]
S_W = [(S_HI[s] - S_LO[s] + 1) * W for s in range(NRQ)]
S_OFF = np.concatenate([[0], np.cumsum(S_W)]).tolist()
EM_N = S_OFF[-1]          # 91*56 = 5096

CVP = NROWS_G * WCV       # 5355 per-pixel cv pitch
QT_N = 14 * 448           # qt stream: v(14) x c(32) x u(14)


def _pad2(a, top, left, hh, ww):
    out = np.zeros(a.shape[:-2] + (hh, ww), a.dtype)
    out[..., top:top + a.shape[-2], left:left + a.shape[-1]] = a
    return out


def host_prep(feats_r, feats_t, quantized_r, ref_index, current_ind):
    feats_r = np.asarray(feats_r, np.float32)
    feats_t = np.asarray(feats_t, np.float32)
    quantized_r = np.asarray(quantized_r, np.float32)
    ri = np.asarray(ref_index).tolist()
    ci = int(current_ind)
    diffs = [ci - int(x) for x in ri]
    nsearch = sum(1 for d in diffs if d > DIL_INT)
    dirates = [min(4, d // DIL_INT + 1) for d in diffs if d > DIL_INT]
    nref = feats_r.shape[0]
    assert nsearch == 1 and dirates[0] == DIL and nref == 3, \
        (nsearch, dirates, nref)

    f1 = feats_t[0]
    f2 = [feats_r[s, 0] for s in range(nref)]
    qr = [quantized_r[s, 0][:, ::D_SUB, ::D_SUB] for s in range(nref)]

    # u-inner interleaved qr0: QI[y, x, c, u] = qr0pad[y+u, x, c]
    qr0can = np.zeros((HP + 14, HP, C), np.float32)
    qr0can[DYLO:DYLO + H, DYLO:DYLO + W, :] = qr[0].transpose(1, 2, 0)
    qi = np.stack([qr0can[u:u + HP] for u in range(14)], axis=3)  # [HP,HP,C,14]
    qi = qi.reshape(1, HP * HP * C * 14)
    qi_b16 = np.ascontiguousarray(qi.astype(NPBF16))

    # f2_0 canvas: rows [-24 .. H+26], cols [-24 .. 80]
    f2p0 = _pad2(f2[0], DYLO, DYLO, H + 2 * DYLO + 3, WCV)
    f2p12 = [_pad2(f2[r], R, R, H + 2 * R, WB) for r in (1, 2)]
    qrpT = []
    for r in (1, 2):
        q = np.zeros((H + 2 * R, WB, C + 1), np.float32)
        q[R:R + H, R:R + W, :C] = qr[r].transpose(1, 2, 0)
        q[:, :, C] = 1.0
        qrpT.append(np.ascontiguousarray(q.transpose(1, 0, 2)))

    ploc128 = np.arange(PPG)
    yloc = (ploc128 >= 64).astype(np.int64)
    xs = np.minimum(ploc128 - 64 * yloc, W - 1)
    # phase-2 CV stream const: row 18+yloc, col x+18; + oi_y*105 + oi_x
    c2cv = (((ploc128 * NROWS_G + MAXOFF + yloc) * WCV + xs + MAXOFF)
            - IDX_BIAS)[:, None]
    # phase-2 QI stream const (element units): ((y+18+yloc)*105 + x+18)*448
    c2qr = ((((yloc + MAXOFF) * WCV + xs + MAXOFF) - IDX_BIAS) * 448)[:, None]

    gridy = np.tile((np.repeat(np.arange(P13) - R, P13) * DIL)[None, :],
                    (PPG, 1)).astype(np.float32)
    gridx = np.tile((np.tile(np.arange(P13) - R, P13) * DIL)[None, :],
                    (PPG, 1)).astype(np.float32)
    uconst = np.tile((np.arange(14) * WCV)[None, :], (PPG, 1)).astype(np.float32)

    xq = np.arange(WB)[:, None]
    xx = np.arange(W)[None, :]
    maskT = ((xq - xx >= 0) & (xq - xx <= 2 * R)).astype(np.float32)  # [68,56]
    maskM = np.tile(maskT[:, None, :], (1, EM_N // W, 1)).reshape(WB, EM_N)
    ident33 = np.zeros((PPG, C + 1), np.float32)
    ident33[:C + 1, :] = np.eye(C + 1)

    def b16(a):
        return np.ascontiguousarray(a.astype(NPBF16))

    in_maps = []
    for core in range(NCORES):
        y0 = core * RY
        f1pair = np.zeros((CF, 4 * PPG), np.float32)
        for g in range(4):
            f1pair[:, g * PPG:g * PPG + W] = f1[:, y0 + 2 * g, :]
            if 2 * g + 1 < RY:
                f1pair[:, g * PPG + 64:g * PPG + 64 + W] = f1[:, y0 + 2 * g + 1, :]
        # 64-partition packs: A (CV-critical) = f1pair | f2p0;
        # D = f1 | f2p1 | f2p2
        packA = np.concatenate([
            f1pair,
            f2p0[:, y0:y0 + H_SLAB, :].reshape(CF, H_SLAB * WCV),
        ], axis=1)
        packD = np.concatenate([
            f1[:, y0:y0 + RY, :].reshape(CF, RY * W),
            f2p12[0][:, y0:y0 + NRQ, :].reshape(CF, NRQ * WB),
            f2p12[1][:, y0:y0 + NRQ, :].reshape(CF, NRQ * WB),
        ], axis=1)
        # 68-partition pack: qrT1 | qrT2 | maskM
        packB = np.concatenate([
            qrpT[0][:, y0:y0 + NRQ, :].reshape(WB, NRQ * (C + 1)),
            qrpT[1][:, y0:y0 + NRQ, :].reshape(WB, NRQ * (C + 1)),
            maskM,
        ], axis=1)
        # 128-partition f32 pack: gridx | gridy | uconst | c2cv | c2qr | id33
        packC = np.concatenate([
            gridx, gridy, uconst, c2cv, c2qr + y0 * WCV * 448, ident33,
        ], axis=1).astype(np.float32)
        m = dict(
            packA=b16(packA),
            packB=b16(packB),
            packC=packC,
            packD=b16(packD),
            qr0pad=qi_b16,
        )
        in_maps.append(m)
    return in_maps


# pack offsets (elements)
A_F1PAIR = 0
A_F2P0 = A_F1PAIR + 4 * PPG
A_N = A_F2P0 + H_SLAB * WCV
D_F1 = 0
D_F2P1 = D_F1 + RY * W
D_F2P2 = D_F2P1 + NRQ * WB
D_N = D_F2P2 + NRQ * WB
B_QRT1 = 0
B_QRT2 = B_QRT1 + NRQ * (C + 1)
B_MASK = B_QRT2 + NRQ * (C + 1)
B_N = B_MASK + EM_N
C_GX = 0
C_GY = C_GX + N169
C_UC = C_GY + N169
C_CV = C_UC + 14
C_QR = C_CV + 1
C_ID = C_QR + 1
C_N = C_ID + (C + 1)

INPUT_SPECS = dict(
    packA=([CF, A_N], BF16),
    packB=([WB, B_N], BF16),
    packC=([PPG, C_N], F32),
    packD=([CF, D_N], BF16),
    qr0pad=([1, HP * HP * C * 14], BF16),
)
OUT_SPEC = ([RY * W, C], F32)


def build_kernel(tc, outs, ins):
    nc = tc.nc
    Exp = mybir.ActivationFunctionType.Exp
    Copy = mybir.ActivationFunctionType.Copy
    ALU = mybir.AluOpType
    AX = mybir.AxisListType

    with ExitStack() as ctx:
        sb = ctx.enter_context(tc.tile_pool(name="sb", bufs=1))
        sbg = ctx.enter_context(tc.tile_pool(name="sbg", bufs=2))
        sbe = ctx.enter_context(tc.tile_pool(name="sbe", bufs=2))
        sbq = ctx.enter_context(tc.tile_pool(name="sbq", bufs=4))
        ps_cv = ctx.enter_context(tc.tile_pool(name="ps_cv", bufs=3, space="PSUM"))
        ps_cc = ctx.enter_context(tc.tile_pool(name="ps_cc", bufs=2, space="PSUM"))
        ps_out = ctx.enter_context(tc.tile_pool(name="ps_out", bufs=1, space="PSUM"))
        ps_tr = ctx.enter_context(tc.tile_pool(name="ps_tr", bufs=1, space="PSUM"))
        dram = ctx.enter_context(tc.tile_pool(name="dram", bufs=1, space="DRAM"))

        def load(name):
            shape, dt_ = INPUT_SPECS[name]
            t = sb.tile(shape, dt_, tag=name)
            nc.sync.dma_start(t[:], ins[name])
            return t

        packA_t = load("packA")
        packC_t = load("packC")
        packD_t = load("packD")
        packB_t = load("packB")

        def pa(off, n):
            return packA_t[:, off:off + n]

        def pd(off, n):
            return packD_t[:, off:off + n]

        f1_t = pd(D_F1, RY * W)
        maskM_t = packB_t[:, B_MASK:B_MASK + EM_N]
        ident_t = packC_t[0:C + 1, C_ID:C_ID + (C + 1)]
        gridx_t = packC_t[:, C_GX:C_GX + N169]
        gridy_t = packC_t[:, C_GY:C_GY + N169]
        uconst_t = packC_t[:, C_UC:C_UC + 14]
        c2cv_t = packC_t[:, C_CV:C_CV + 1]
        c2qr_t = packC_t[:, C_QR:C_QR + 1]

        out0_g, z0_g = {}, {}

        # ---------- stage 0: CV volume -> DRAM + phase-1 window reads ----
        def stage0(g, MP=PPG):
            nrow = NROWS_G
            cv_sb = sbg.tile([MP, nrow * WCV], BF16, tag="cv_sb")
            lhs = pa(A_F1PAIR + g * PPG, PPG)
            ci = 0
            # 4 slab rows per matmul (420 cols in one PSUM bank)
            for r0 in range(0, nrow, 4):
                rn = min(4, nrow - r0)
                ncol = rn * WCV
                pt = ps_cv.tile([MP, 512], F32, tag="cvch")
                nc.tensor.matmul(
                    pt[:, 0:ncol], lhsT=lhs,
                    rhs=pa(A_F2P0 + (2 * g + r0) * WCV, ncol),
                    start=True, stop=True)
                dst = cv_sb[:, r0 * WCV:(r0 + rn) * WCV]
                if ci % 2 == 0:
                    nc.vector.tensor_copy(dst, pt[:, 0:ncol])
                else:
                    nc.scalar.copy(dst, pt[:, 0:ncol])
                ci += 1
            cv_dram = dram.tile([1, MP * CVP + 8 * WCV], BF16, tag=f"cvd{g}")
            nc.sync.dma_start(
                cv_dram[:, 0:MP * CVP].rearrange("o (p f) -> p (o f)", p=MP),
                cv_sb[:])

            # ---------- B. phase-1: static strided reads of cv ----------
            g1b = sbq.tile([MP, P13 * CC_RUN], BF16, tag="g1b")
            for pl, ybase in ((slice(0, 64), R), (slice(64, 128), R + 1)):
                p0_ = pl.start
                base = p0_ * CVP + ybase * WCV + R
                src = bass.AP(cv_dram[:].tensor, cv_dram[:].offset + base,
                              [[CVP + 1, 64], [3 * WCV, P13], [1, CC_RUN]])
                dst = g1b[pl].rearrange("p (i r) -> p i r", i=P13)
                nc.sync.dma_start(dst, src)
            return dict(g=g, MP=MP, g1b=g1b, cv_dram=cv_dram)

        # ---------- stage 1: softmax offsets -> gather launches ----------
        def stage1(s0, yg):
            g, MP, g1b, cv_dram = s0["g"], s0["MP"], s0["g1b"], s0["cv_dram"]
            cc0 = bass.AP(g1b[:].tensor, g1b[:].offset,
                          [g1b[:].ap[0], [CC_RUN, P13], [3, P13]])
            e1 = sbq.tile([MP, N169 + 1], F32, tag="e1")
            nc.scalar.activation(
                e1[:, 0:N169].rearrange("p (i j) -> p i j", i=P13), cc0, Exp,
                accum_out=e1[:, N169:N169 + 1])
            sc = sbq.tile([MP, 4], F32, tag="sc")
            tmp = sbq.tile([MP, N169], F32, tag="tmp169")
            nc.vector.scalar_tensor_tensor(
                out=tmp[:], in0=e1[:, 0:N169], scalar=0.0, in1=gridx_t,
                op0=ALU.add, op1=ALU.mult, accum_out=sc[:, 0:1])
            nc.vector.scalar_tensor_tensor(
                out=tmp[:], in0=e1[:, 0:N169], scalar=0.0, in1=gridy_t,
                op0=ALU.add, op1=ALU.mult, accum_out=sc[:, 1:2])
            offs = sbq.tile([MP, 2], F32, tag="offs")   # [off_x, off_y]
            nc.vector.reciprocal(sc[:, 2:3], e1[:, N169:N169 + 1])
            nc.vector.tensor_tensor(offs[:, 0:1], sc[:, 0:1], sc[:, 2:3],
                                    op=ALU.mult)
            nc.vector.tensor_tensor(offs[:, 1:2], sc[:, 1:2], sc[:, 2:3],
                                    op=ALU.mult)
            nc.vector.tensor_scalar(offs[:], offs[:], float(MAXOFF),
                                    -float(MAXOFF), op0=ALU.min, op1=ALU.max)
            # floor (mode-agnostic): fb = off+1024; fbi=cast;
            # fbf=cast back; fbf -= (fb - fbf < 0); wfrac = fb - fbf
            fb = sbq.tile([MP, 2], F32, tag="fb")
            nc.vector.tensor_scalar(fb[:], offs[:], FLOOR_BIAS, None,
                                    op0=ALU.add)
            fbi = sbq.tile([MP, 2], I32, tag="fbi")
            nc.vector.tensor_copy(fbi[:], fb[:])
            fbf = sbq.tile([MP, 2], F32, tag="fbf")
            nc.vector.tensor_copy(fbf[:], fbi[:])
            err = sbq.tile([MP, 2], F32, tag="err")
            nc.vector.tensor_tensor(err[:], fb[:], fbf[:], op=ALU.subtract)
            neg = sbq.tile([MP, 2], F32, tag="neg")
            nc.vector.tensor_scalar(neg[:], err[:], 0.0, None, op0=ALU.is_lt)
            nc.vector.tensor_tensor(fbf[:], fbf[:], neg[:], op=ALU.subtract)
            wfrac = sbq.tile([MP, 2], F32, tag="wfrac")  # [wx, wy]
            nc.vector.tensor_tensor(wfrac[:], fb[:], fbf[:], op=ALU.subtract)
            s2 = sbq.tile([MP, 1], F32, tag="s2")
            nc.vector.scalar_tensor_tensor(
                out=s2[:], in0=fbf[:, 1:2], scalar=float(WCV),
                in1=fbf[:, 0:1], op0=ALU.mult, op1=ALU.add)
            i1f = sbq.tile([MP, 1], F32, tag="i1f")
            nc.vector.tensor_scalar(i1f[:], c2cv_t, s2[:], None,
                                    op0=ALU.add)
            idx2cv = sbq.tile([MP, 1], I32, tag="idx2cv")
            nc.gpsimd.tensor_copy(idx2cv[:], i1f[:])
            # QI element index: c2qr + (s2 + yg*WCV)*448
            idx2qrf = sbq.tile([MP, 1], F32, tag="idx2qrf")
            nc.gpsimd.tensor_scalar(idx2qrf[:], s2[:], 448.0,
                                    float(yg * WCV * 448),
                                    op0=ALU.mult, op1=ALU.add)
            nc.gpsimd.tensor_tensor(idx2qrf[:], idx2qrf[:], c2qr_t,
                                    op=ALU.add)
            idx2qr = sbq.tile([MP, 1], I32, tag="idx2qr")
            nc.gpsimd.tensor_copy(idx2qr[:], idx2qrf[:])

            # ---------- C. phase-2 gathers ----------
            NS2 = 13 * WCV + 14
            g2 = sbq.tile([MP, NS2], BF16, tag="g2")
            nc.gpsimd.indirect_dma_start(
                out=g2[:], out_offset=None, in_=cv_dram[:],
                in_offset=bass.IndirectOffsetOnAxis(ap=idx2cv[:], axis=1))
            if g == 0:
                g2_dbg.append(g2)
                g1b_dbg.append(g1b)
                offs_dbg.append(offs)
            qt = sbq.tile([MP, QT_N], BF16, tag="qt")
            nc.gpsimd.indirect_dma_start(
                out=qt[:], out_offset=None, in_=ins["qr0pad"],
                in_offset=bass.IndirectOffsetOnAxis(ap=idx2qr[:], axis=1))

            # bilinear corner weights
            ww = sbq.tile([MP, 4], F32, tag="ww")
            om = sbq.tile([MP, 2], F32, tag="om")
            nc.vector.tensor_scalar(om[:], wfrac[:], -1.0, 1.0,
                                    op0=ALU.mult, op1=ALU.add)
            nc.vector.tensor_tensor(ww[:, 0:1], om[:, 1:2], om[:, 0:1],
                                    op=ALU.mult)
            nc.vector.tensor_tensor(ww[:, 1:2], om[:, 1:2], wfrac[:, 0:1],
                                    op=ALU.mult)
            nc.vector.tensor_tensor(ww[:, 2:3], wfrac[:, 1:2], om[:, 0:1],
                                    op=ALU.mult)
            nc.vector.tensor_tensor(ww[:, 3:4], wfrac[:, 1:2], wfrac[:, 0:1],
                                    op=ALU.mult)
            bb = sbq.tile([MP, 196], BF16, tag="bb")
            nc.gpsimd.memset(bb[:], 0.0)
            return dict(g=g, MP=MP, g2=g2, qt=qt, ww=ww, bb=bb)

        # ---------- stage 2: bilinear blend -> p0 -> blur -> o0 ----------
        def stage2(st):
            g, MP, g2, qt, ww = st["g"], st["MP"], st["g2"], st["qt"], st["ww"]
            bb = st["bb"]
            g2v = bass.AP(g2[:].tensor, g2[:].offset,
                          [g2[:].ap[0], [WCV, 14], [1, 14]])
            corr = sbq.tile([MP, N169], F32, tag="corr")
            crv = corr[:].rearrange("p (i j) -> p i j", i=P13)
            nc.vector.tensor_scalar(crv, g2v[:, 0:13, 0:13], ww[:, 0:1], None,
                                    op0=ALU.mult)
            for (sl_u, sl_v, wcol) in (((0, 13), (1, 14), 1),
                                       ((1, 14), (0, 13), 2),
                                       ((1, 14), (1, 14), 3)):
                nc.vector.scalar_tensor_tensor(
                    out=crv, in0=g2v[:, sl_u[0]:sl_u[1], sl_v[0]:sl_v[1]],
                    scalar=ww[:, wcol:wcol + 1], in1=crv,
                    op0=ALU.mult, op1=ALU.add)
            p0 = sb.tile([MP, N169 + 1], F32, tag=f"p0_{g}")
            nc.scalar.activation(p0[:, 0:N169], corr[:], Exp,
                                 accum_out=z0cat[:, g:g + 1])
            z0_g[g] = p0
            # bilinear blur, v-major: bb[p, v, u]
            bbv = bb[:].rearrange("p (v u) -> p v u", v=14)
            # p0 viewed as (j=v, i=u): AP dims j outer (stride 1), i inner
            p0ji = bass.AP(p0[:].tensor, p0[:].offset,
                           [p0[:].ap[0], [1, P13], [P13, P13]])
            nc.vector.tensor_scalar(bbv[:, 0:13, 0:13], p0ji, ww[:, 0:1],
                                    None, op0=ALU.mult)
            for (sl_v, sl_u, wcol) in (((1, 14), (0, 13), 1),
                                       ((0, 13), (1, 14), 2),
                                       ((1, 14), (1, 14), 3)):
                dstv = bbv[:, sl_v[0]:sl_v[1], sl_u[0]:sl_u[1]]
                nc.vector.scalar_tensor_tensor(
                    out=dstv, in0=p0ji, scalar=ww[:, wcol:wcol + 1], in1=dstv,
                    op0=ALU.mult, op1=ALU.add)
            # packed bf16 multiply qt *= bb (broadcast over c via 0-stride)
            qtv = bass.AP(qt[:].tensor, qt[:].offset,
                          [qt[:].ap[0], [448, 14], [14, C], [1, 14]])
            bbb = bass.AP(bb[:].tensor, bb[:].offset,
                          [bb[:].ap[0], [14, 14], [0, C], [1, 14]])
            nc.vector.tensor_tensor(qtv, qtv, bbb, op=ALU.mult)
            # fold v: 14 -> 7 -> 4 -> 2 -> 1 rows, then reduce keep c
            nc.vector.tensor_tensor(qt[:, 0:7 * 448], qt[:, 0:7 * 448],
                                    qt[:, 7 * 448:14 * 448], op=ALU.add)
            nc.vector.tensor_tensor(qt[:, 0:3 * 448], qt[:, 0:3 * 448],
                                    qt[:, 4 * 448:7 * 448], op=ALU.add)
            nc.vector.tensor_tensor(qt[:, 0:2 * 448], qt[:, 0:2 * 448],
                                    qt[:, 2 * 448:4 * 448], op=ALU.add)
            nc.vector.tensor_tensor(qt[:, 0:448], qt[:, 0:448],
                                    qt[:, 448:2 * 448], op=ALU.add)
            pr = bass.AP(qt[:].tensor, qt[:].offset,
                         [qt[:].ap[0], [14, C], [1, 14]])
            nc.vector.tensor_reduce(o0cat[:, g * C:(g + 1) * C], pr,
                                    axis=AX.X, op=ALU.add)

        # ---------- E. refs 1/2, s-major ----------
        def emit_ref(r):
            em = sbe.tile([WB, EM_N], BF16, tag="em")
            for s in range(NRQ):
                w = S_W[s]
                ct = ps_cc.tile([WB, 512], F32, tag="ct")
                nc.tensor.matmul(
                    ct[:, 0:w],
                    lhsT=pd((D_F2P1, D_F2P2)[r] + s * WB, WB),
                    rhs=f1_t[:, S_LO[s] * W:S_LO[s] * W + w],
                    start=True, stop=True)
                nc.scalar.activation(em[:, S_OFF[s]:S_OFF[s] + w],
                                     ct[:, 0:w], Exp)
            return em

        def emit_mask(em):
            nc.vector.tensor_tensor(em[:], em[:], maskM_t, op=ALU.mult)

        def emit_attn(r, em, op2, last):
            for s in range(NRQ):
                nc.tensor.matmul(
                    op2[:, S_LO[s] * W:S_LO[s] * W + S_W[s]],
                    lhsT=packB_t[:, (B_QRT1, B_QRT2)[r] + s * (C + 1):
                                 (B_QRT1, B_QRT2)[r] + (s + 1) * (C + 1)],
                    rhs=em[:, S_OFF[s]:S_OFF[s] + S_W[s]],
                    start=False, stop=(last and s == NRQ - 1),
                    skip_group_check=True)

        # ---------- F. combine (batched) ----------
        ofst = sbg.tile([W, RY * C], F32, tag="ofst")
        o0cat = sb.tile([PPG, 4 * C], F32, tag="o0cat")
        z0cat = sb.tile([PPG, 4], F32, tag="z0cat")

        def combine_all(ops):
            opTall = ps_tr.tile([W, RY * (C + 1)], F32, tag="opTall")
            for yr in range(RY):
                nc.tensor.transpose(opTall[:, yr * (C + 1):
                                           (yr + 1) * (C + 1)],
                                    ops[:, yr * W:(yr + 1) * W], ident_t)
            ot = opTall[:]
            CC1 = C + 1
            for par, yrs in ((0, (0, 2, 4, 6)), (64, (1, 3, 5))):
                n = len(yrs)
                psl = slice(par, par + W)
                num = bass.AP(ot.tensor, ot.offset + yrs[0] * CC1,
                              [ot.ap[0], [2 * CC1, n], [1, C]])
                dnm = bass.AP(ot.tensor, ot.offset + yrs[0] * CC1 + C,
                              [ot.ap[0], [2 * CC1, n]])
                o0s = bass.AP(o0cat[psl].tensor, o0cat[psl].offset,
                              [o0cat[psl].ap[0], [C, n], [1, C]])
                z0s = z0cat[psl, 0:n]
                den = sbq.tile([W, 8], F32, tag="den")
                nc.vector.tensor_tensor(den[:, 0:n], dnm, z0s, op=ALU.add)
                nc.vector.reciprocal(den[:, 4:4 + n], den[:, 0:n])
                ofv = bass.AP(ofst[:].tensor,
                              ofst[:].offset + yrs[0] * C,
                              [ofst[:].ap[0], [2 * C, n], [1, C]])
                nc.vector.tensor_tensor(ofv, num, o0s, op=ALU.add)
                denb = bass.AP(den[:].tensor, den[:].offset + 4,
                               [den[:].ap[0], [1, n], [0, C]])
                nc.vector.tensor_tensor(ofv, ofv, denb, op=ALU.mult)

        # ---------- schedule ----------
        op2 = ps_out.tile([C + 1, RY * W], F32, tag="op2")
        nc.vector.memset(op2[:], 0.0)

        g2_dbg, g1b_dbg, offs_dbg = [], [], []
        s00 = stage0(0)
        s01 = stage0(1)
        s02 = stage0(2)
        s03 = stage0(3)
        st0 = stage1(s00, 0)
        st1 = stage1(s01, 2)
        st2 = stage1(s02, 4)
        st3 = stage1(s03, 6)
        em0 = emit_ref(0)
        stage2(st0)
        emit_mask(em0)
        emit_attn(0, em0, op2, last=False)
        stage2(st1)
        em1 = emit_ref(1)
        emit_mask(em1)
        emit_attn(1, em1, op2, last=True)
        stage2(st2)
        ops = sbg.tile([C + 1, RY * W], F32, tag="ops")
        nc.scalar.copy(ops[:], op2[:])
        stage2(st3)
        combine_all(ops)
        dst = bass.AP(outs["out"].tensor, outs["out"].offset,
                      [[C, W], [W * C, RY], [1, C]])
        nc.sync.dma_start(dst, ofst[:].rearrange("p (y c) -> p y c", y=RY))

        if "dbg_ops" in outs:
            nc.sync.dma_start(outs["dbg_ops"], ops[:])
            nc.sync.dma_start(outs["dbg_em0"], em0[:])
            nc.sync.dma_start(outs["dbg_o0"], o0cat[:, 0:C])
            nc.sync.dma_start(outs["dbg_p0"], z0_g[0][:])
            nc.sync.dma_start(outs["dbg_g2"], g2_dbg[0][:])
            nc.sync.dma_start(outs["dbg_g1b"], g1b_dbg[0][:])
            nc.sync.dma_start(outs["dbg_offs"], offs_dbg[0][:])


DEBUG_SPECS = dict(
    dbg_ops=([C + 1, RY * W], F32), dbg_em0=([WB, EM_N], BF16),
    dbg_o0=([PPG, C], F32), dbg_p0=([PPG, N169 + 1], F32),
    dbg_g2=([PPG, 13 * WCV + 14], BF16), dbg_g1b=([PPG, P13 * CC_RUN], BF16),
    dbg_offs=([PPG, 2], F32),
)


def build_program(ncores=NCORES, debug=False):
    import concourse.bacc as bacc
    nc = bacc.Bacc("TRN2", target_bir_lowering=False, debug=False,
                   enable_asserts=True, num_devices=ncores)
    ins = {}
    for name, (shape, dt_) in INPUT_SPECS.items():
        ins[name] = nc.dram_tensor(name, shape, dt_, kind="ExternalInput").ap()
    outs = {"out": nc.dram_tensor("out", OUT_SPEC[0], OUT_SPEC[1],
                                  kind="ExternalOutput").ap()}
    if debug:
        for name, (shape, dt_) in DEBUG_SPECS.items():
            outs[name] = nc.dram_tensor(name, shape, dt_,
                                        kind="ExternalOutput").ap()
    with tile.TileContext(nc) as tc:
        build_kernel(tc, outs, ins)
    nc.compile()
    return nc


# ======================= runner =======================
import os as _os

_LAST_RESULT = {}


def kernel(**inputs):
    from concourse.bass_utils import run_bass_kernel_spmd
    from concourse.bass_interp import get_hw_module

    in_maps = host_prep(**inputs)
    nc = build_program()
    nc.m = get_hw_module(nc.m)
    trace = _os.environ.get("KERNEL_TRACE", "0") == "1"
    res = run_bass_kernel_spmd(
        nc, in_maps, core_ids=list(range(NCORES)), trace=trace)
    _LAST_RESULT["res"] = res
    slabs = [np.asarray(res.results[i]["out"], np.float32).reshape(RY, W, C)
             for i in range(NCORES)]
    full = np.concatenate(slabs, 0)          # [56, 56, 32]
    return np.ascontiguousarray(full.transpose(2, 0, 1)[None])
